# revision 1
# baseline (speedup 1.0000x reference)
"""Trainium2 Bass kernel for nn_MultiHeadAttention_47579647705431.

Multi-head attention (8 heads, dim 512, seq 1024, batch 16) with:
  - shared key/query linear (key_query_same=True: q and k both use Wk/bk)
  - causal (or arbitrary block-structured) mask
  - SimpleKT zero_pad: attention row 0 zeroed => out[:, 0, :] = bo

Sharding: data-parallel over batch across 8 NeuronCores (2 batches/core).

Per-core pipeline (all matmuls bf16, fp32 PSUM):
  1. kp/qp = Wk.T-stationary projections -> feature-major [o, n] bf16
  2. vp    = token-major projection [n, o] bf16 with interleaved ones
             columns (stride-65) providing the softmax denominator column
  3. per (b, hp, c): scores^T st [t, s] via K=64 row-packed matmuls;
     exp on ACT (scale 1/8 folded, several j-blocks packed per
     activation); causal/diagonal masking as a 0/1 multiply on DVE
  4. AV with SWAPPED operands: stationary = ex [t, s-block], moving =
     vp [t, 65] -> av PSUM [s, 2, 65] per (b, i, hp).  Cost = 65 free
     columns per (i, j, head) instead of 512 -- half the PE cycles of
     the stationary-vp form, and the denominator lands per-partition so
     normalization is a DVE reciprocal + broadcast multiply (no PE
     broadcast matmuls, no mask identity matmuls).
  5. ct_t token-major [s, 512] per (b, i) -> feature-major ct_i
     [128, 4, 128] via one XBAR dma_start_transpose
  6. out projection per (b, i) (ct_i-stationary) -> [128, 512] f32 -> DRAM

The walrus build here supports ONE sync wait per instruction; Tile emits
more. legalize_waits() hoists extra waits onto same-engine NoOps.
"""

import os
from contextlib import ExitStack

import numpy as np
import ml_dtypes

import concourse.bass as bass
import concourse.mybir as mybir
import concourse.tile as tile
from concourse.bass_utils import run_bass_kernel_spmd

F32 = mybir.dt.float32
BF16 = mybir.dt.bfloat16
BF = ml_dtypes.bfloat16

B, S, D, H, DH = 16, 1024, 512, 8, 64
NCORES = 8
BL = B // NCORES          # batches per core
N = BL * S                # tokens per core
NB = S // 128             # 128-blocks per sequence (8)
HP = H // 2               # head pairs (= o-blocks of 128)
NCH = S // 512            # 512-chunks per sequence (2)

LAST_SIM_NS = None
LAST_EXEC_NS = None


def legalize_waits(nc):
    """Split multi-wait instructions: keep one wait, hoist the rest onto
    preceding same-engine NoOps (this walrus encodes 1 wait/instruction)."""
    for f in nc.m.functions:
        for blk in f.blocks:
            il = blk.instructions
            i = 0
            while i < len(il):
                inst = il[i]
                si = inst.sync_info
                if si is not None and si.on_wait and len(si.on_wait) > 1:
                    waits = list(si.on_wait)
                    for j, w in enumerate(waits[:-1]):
                        nop = mybir.InstNoOp(
                            name=f"{inst.name}-hw{j}",
                            sync_info=mybir.SyncInfo(on_wait=[w], on_update=[]),
                            bass_nofuse=True,
                            engine=inst.engine,
                        )
                        il.insert(i, nop)
                        i += 1
                    si.on_wait = waits[-1:]
                i += 1


def _classify_mask(mask2d):
    """Classify 128x128 blocks of the [S, S] bool mask (query s, key t).

    Returns (status[j][i], patterns) in scores-transposed coords:
    j = key(t) block, i = query(s) block. status: -1 skip, -2 full,
    >=0 index into patterns (multiplicative bf16 0/1 [t, s] blocks).
    """
    status = [[-1] * NB for _ in range(NB)]
    patterns = []
    pat_idx = {}
    for j in range(NB):
        for i in range(NB):
            blk = mask2d[i * 128:(i + 1) * 128, j * 128:(j + 1) * 128]  # [s, t]
            if blk.all():
                status[j][i] = -2
            elif not blk.any():
                status[j][i] = -1
            else:
                mul = np.where(blk.T, 1.0, 0.0).astype(BF)  # [t, s]
                key = mul.tobytes()
                if key not in pat_idx:
                    pat_idx[key] = len(patterns)
                    patterns.append(mul)
                status[j][i] = pat_idx[key]
    return status, patterns


def _plan_chunks(status, patterns):
    """Per (c, j): suffix run of non-skip query blocks within chunk c.

    Returns plan[c][j] = (w, mixes) where w = run width and mixes =
    [(col_offset_in_region, pattern_id), ...] for mixed blocks. Also
    first_j[c]. Asserts the suffix-nested structure the kernel relies on.
    """
    plan = [[None] * NB for _ in range(NCH)]
    first_j = [None] * NCH
    for c in range(NCH):
        i_lo, i_hi = 4 * c, 4 * c + 4
        prev_w = None
        for j in range(NB):
            sts = [status[j][i] for i in range(i_lo, i_hi)]
            nz = [k for k, s in enumerate(sts) if s != -1]
            if not nz:
                plan[c][j] = (0, [])
                continue
            # must be a contiguous suffix of the chunk
            if nz != list(range(nz[0], 4)):
                raise NotImplementedError("mask block structure not suffix-contiguous")
            w = 128 * len(nz)
            if prev_w is not None and w > prev_w:
                raise NotImplementedError("mask runs not nested over key blocks")
            prev_w = w
            mixes = [((k - nz[0]) * 128, sts[k]) for k in nz if sts[k] >= 0]
            plan[c][j] = (w, mixes)
            if first_j[c] is None:
                first_j[c] = j
    return plan, first_j


def _pack_js(plan, c):
    """Greedy-pack consecutive j runs so one st tile / one exp covers
    several j blocks.  Each pack's total 2w must fit 1024 f32 (4KB)."""
    js = [j for j in range(NB) if plan[c][j][0] > 0]
    packs = []
    cur, cur_sz = [], 0
    for j in js:
        sz = 2 * plan[c][j][0]
        if cur and cur_sz + sz > 1024:
            packs.append(cur)
            cur, cur_sz = [], 0
        cur.append(j)
        cur_sz += sz
    if cur:
        packs.append(cur)
    return packs


def _build(plan, first_j, nmix, has_bk, has_bv, has_bo):
    nc = bass.Bass()
    qt = nc.dram_tensor("qt", [128, 4, N], BF16, kind="ExternalInput")
    kt = nc.dram_tensor("kt", [128, 4, N], BF16, kind="ExternalInput")
    vt = nc.dram_tensor("vt", [128, 4, N], BF16, kind="ExternalInput")
    wkt = nc.dram_tensor("wkt", [4, 128, 4, 128], BF16, kind="ExternalInput")
    wvt = nc.dram_tensor("wvt", [128, 4, D], BF16, kind="ExternalInput")
    wot = nc.dram_tensor("wot", [128, 4, D], BF16, kind="ExternalInput")
    bk32 = nc.dram_tensor("bk32", [128, 4], F32, kind="ExternalInput")
    bvb = nc.dram_tensor("bvb", [1, D], BF16, kind="ExternalInput")
    bob = nc.dram_tensor("bob", [1, D], BF16, kind="ExternalInput")
    mixmul = nc.dram_tensor("mixmul", [max(nmix, 1), 128, 128], BF16,
                            kind="ExternalInput")
    out = nc.dram_tensor("out", [N, D], F32, kind="ExternalOutput")

    with tile.TileContext(nc) as tc:
        with ExitStack() as ctx:
            sing = ctx.enter_context(tc.tile_pool(name="sing", bufs=1))
            expp = ctx.enter_context(tc.tile_pool(name="expp", bufs=21))
            rcp = ctx.enter_context(tc.tile_pool(name="rcp", bufs=4))
            ctp = ctx.enter_context(tc.tile_pool(name="ctp", bufs=2))
            cti = ctx.enter_context(tc.tile_pool(name="cti", bufs=4))
            outp = ctx.enter_context(tc.tile_pool(name="outp", bufs=4))
            stp = ctx.enter_context(tc.tile_pool(name="stp", bufs=2, space="PSUM"))
            avp = ctx.enter_context(tc.tile_pool(name="avp", bufs=2, space="PSUM"))
            shp = ctx.enter_context(tc.tile_pool(name="shp", bufs=2, space="PSUM"))

            # ---- input loads: critical-path first.  Attention-phase DMAs
            # go on sync; startup loads spread over scalar/sync/gpsimd.
            # kt0 on sync and wkt on scalar so the first projection's two
            # inputs stream through HWDGE back-to-back instead of serially
            kt_c, qt_c, vt_c = [], [], []
            kt_c = [None] * 4
            qt_c = [None] * 4
            vt_c = [None] * 4
            wkt_ob = []
            for ob in range(4):
                t = sing.tile([128, 4, 128], BF16, tag=f"wktob{ob}",
                              name=f"wktob{ob}")
                wkt_ob.append(t)
            # arrival order follows the iteration schedule
            # b0c0 -> b0c1 -> b1c1 -> b1c0: ch 0, 1, 3, 2
            ch_order = (0, 1, 3, 2)
            for ci, ch in enumerate(ch_order):
                csl = slice(ch * 512, ch * 512 + 512)
                t = sing.tile([128, 4, 512], BF16, tag=f"ktc{ch}",
                              name=f"ktc{ch}")
                nc.sync.dma_start(out=t, in_=kt[:, :, csl])
                kt_c[ch] = t
                if ci == 0:
                    nc.sync.dma_start(out=wkt_ob[0], in_=wkt[0, :, :, :])
                t = sing.tile([128, 4, 512], BF16, tag=f"qtc{ch}",
                              name=f"qtc{ch}")
                nc.scalar.dma_start(out=t, in_=qt[:, :, csl])
                qt_c[ch] = t
                if ci == 0:
                    for ob in range(1, 4):
                        nc.scalar.dma_start(out=wkt_ob[ob],
                                            in_=wkt[ob, :, :, :])
                    wvt_sb = sing.tile([128, 4, D], BF16)
                    nc.scalar.dma_start(out=wvt_sb, in_=wvt[:, :, :])
                else:
                    # v loads ride the scalar HWDGE queue behind the more
                    # critical kq inputs (per-engine priority order holds;
                    # Pool SWDGE DGEs would jump the shared DMA engines)
                    pch = ch_order[ci - 1]
                    t = sing.tile([128, 4, 512], BF16, tag=f"vtc{pch}",
                                  name=f"vtc{pch}")
                    nc.scalar.dma_start(out=t, in_=vt[:, :, pch * 512:pch * 512 + 512])
                    vt_c[pch] = t
            t = sing.tile([128, 4, 512], BF16, tag="vtc2f", name="vtc2f")
            nc.scalar.dma_start(out=t, in_=vt[:, :, 2 * 512:2 * 512 + 512])
            vt_c[2] = t
            mix_sb = sing.tile([128, max(nmix, 1), 128], BF16)
            nc.scalar.dma_start(out=mix_sb, in_=mixmul.rearrange("m t s -> t m s"))
            wot_sb = sing.tile([128, 4, D], BF16)
            nc.scalar.dma_start(out=wot_sb, in_=wot[:, :, :])
            bk_sb = None
            if has_bk:
                bk_sb = sing.tile([128, 4], F32)
                nc.sync.dma_start(out=bk_sb, in_=bk32[:, :])
            bvb_sb = bob_sb = ones_k1 = None
            if has_bv or has_bo:
                ones_k1 = sing.tile([1, 128], BF16)
                nc.vector.memset(ones_k1, 1.0)
            if has_bv:
                bvb_sb = sing.tile([1, D], BF16)
                nc.sync.dma_start(out=bvb_sb, in_=bvb[:, :])
            if has_bo:
                bob_sb = sing.tile([1, D], BF16)
                nc.sync.dma_start(out=bob_sb, in_=bob[:, :])

            kp_sb = sing.tile([128, 4, N], BF16)
            qp_sb = sing.tile([128, 4, N], BF16)
            vp_sb = sing.tile([128, N // 128, 520], BF16)

            # ones columns of vp (denominator trick)
            for nt in range(N // 128):
                nc.vector.memset(
                    vp_sb[:, nt, :].rearrange("p (h u) -> p h u", u=65)[:, :, 64:65],
                    1.0)

            fillers = []
            staged = []    # (pop_stamp, thunk): finals wait out their
                           # transpose latency before becoming poppable
            pop_ctr = [0]
            copy_rr = [0]

            def psum_copy(dst, src):
                # PSUM->SBUF copies on DVE; ACT stays exp-only and GPSIMD
                # cannot access PSUM
                nc.vector.tensor_copy(out=dst, in_=src)

            def kq_half(ob, ch, which):
                csl = slice(ch * 512, ch * 512 + 512)
                src = kt_c[ch] if which == "k" else qt_c[ch]
                dst = kp_sb if which == "k" else qp_sb
                ps = shp.tile([128, 512], F32, tag="sh", name=f"ps{which}")
                for db in range(4):
                    nc.tensor.matmul(
                        ps, wkt_ob[ob][:, db, :],
                        src[:, db, :], start=(db == 0), stop=(db == 3))
                if has_bk:
                    if which == "k":
                        nc.scalar.add(dst[:, ob, csl], ps, bk_sb[:, ob:ob + 1])
                    else:
                        nc.vector.tensor_scalar_add(
                            dst[:, ob, csl], ps, bk_sb[:, ob:ob + 1])
                else:
                    psum_copy(dst[:, ob, csl], ps)

            def kq_group(ob, ch):
                kq_half(ob, ch, "k")
                kq_half(ob, ch, "q")

            def v_proj(nt):
                psV = shp.tile([128, 512], F32, tag="sh")
                for db in range(4):
                    nc.tensor.matmul(
                        psV, vt_c[nt // 4][:, db, (nt % 4) * 128:(nt % 4) * 128 + 128],
                        wvt_sb[:, db, :], start=(db == 0),
                        stop=(db == 3 and not has_bv))
                if has_bv:
                    nc.tensor.matmul(psV, ones_k1, bvb_sb[0:1, :],
                                     start=False, stop=True)
                dst = vp_sb[:, nt, :].rearrange("p (h u) -> p h u", u=65)[:, :, 0:64]
                src = psV.rearrange("p (h u) -> p h u", u=64)
                psum_copy(dst, src)

            def pop_filler(k=1):
                for _ in range(k):
                    pop_ctr[0] += 1
                    while staged and staged[0][0] + 16 <= pop_ctr[0]:
                        fillers.append((("fin",), staged.pop(0)[1]))
                    if fillers:
                        fillers.pop(0)[1]()

            def need_filler(key):
                """Force-emit a specific filler now (dependency deadline)."""
                for fi, (k, thunk) in enumerate(fillers):
                    if k == key:
                        fillers.pop(fi)
                        thunk()
                        return

            ct_t_tiles = {}

            def attention_packs(b, hp, c, ex_t):
                """Thunks: scores+exp+mask, one per pack of j blocks."""
                packs = _pack_js(plan, c)
                thunks = []

                def do_pack(pack):
                    # PSUM bank rule: each matmul output must stay inside one
                    # 2KB bank.  h0 segments stack downward from col 512
                    # (bank 0), h1 segments upward from col 512 (bank 1); the
                    # exp covers the contiguous union [512-tw, 512+tw).
                    tw = sum(plan[c][j][0] for j in pack)
                    st = stp.tile([128, 1024], F32, tag="st")
                    ex = expp.tile([128, 1024], BF16, tag="ex")
                    pre = 0
                    for j in pack:
                        w, _ = plan[c][j]
                        tsl = slice(b * S + j * 128, b * S + j * 128 + 128)
                        ssl = slice(b * S + c * 512 + 512 - w,
                                    b * S + c * 512 + 512)
                        h0s = 512 - pre - w
                        h1s = 512 + pre
                        nc.tensor.matmul(st[:, h0s:h0s + w],
                                         kp_sb[0:64, hp, tsl],
                                         qp_sb[0:64, hp, ssl],
                                         start=True, stop=True)
                        nc.tensor.matmul(st[:, h1s:h1s + w],
                                         kp_sb[64:128, hp, tsl],
                                         qp_sb[64:128, hp, ssl],
                                         start=True, stop=True)
                        ex_t[j] = (ex, h0s, h1s, w)
                        pre += w
                    nc.scalar.activation(
                        ex[:, 512 - tw:512 + tw], st[:, 512 - tw:512 + tw],
                        mybir.ActivationFunctionType.Exp, scale=0.125)
                    # 0/1 mask multiply for mixed blocks (per head half)
                    for j in pack:
                        w, mixes = plan[c][j]
                        _, h0s, h1s, _ = ex_t[j]
                        for moff, pid in mixes:
                            for hs in (h0s, h1s):
                                sl = ex[:, hs + moff:hs + moff + 128]
                                nc.vector.tensor_mul(
                                    sl, sl, mix_sb[:, pid, :])

                for pack in packs:
                    import functools
                    thunks.append(functools.partial(do_pack, pack))
                return thunks

            def attention_avs(b, hp, c, ex_t, last_hp, tail=False):
                """Thunks: AV + normalize, one per query block i (swapped
                operands: ex stationary, vp moving)."""
                h0 = 2 * hp
                thunks = []

                def do_av(i):
                    js_i = []
                    for j, (ex, h0s, h1s, w) in ex_t.items():
                        i_start = 4 * c + 4 - w // 128
                        if i >= i_start:
                            o = (i - i_start) * 128
                            js_i.append((j, ex, (h0s + o, h1s + o)))
                    if not js_i:
                        return
                    for j, _, _ in js_i:
                        need_filler(("vp", b * NB + j))
                    av = avp.tile([128, 2, 65], F32, tag="av")
                    nmm = len(js_i) * 2
                    mi = 0
                    for j, ex, hss in js_i:
                        vrow = b * NB + j
                        for h in range(2):
                            # single accumulation group per av tile: PSUM
                            # zeroing is bank-granular (start marks the whole
                            # bank pending-zero; first write to each address
                            # assigns, later writes accumulate)
                            nc.tensor.matmul(
                                av[:, h, :],
                                ex[:, hss[h]:hss[h] + 128],
                                vp_sb[:, vrow,
                                      65 * (h0 + h):65 * (h0 + h) + 65],
                                start=(mi == 0), stop=(mi == nmm - 1),
                                skip_group_check=True)
                            mi += 1
                    # normalize: per-partition reciprocal + broadcast mul
                    key = (b, i)
                    if key not in ct_t_tiles:
                        ct_t_tiles[key] = ctp.tile([128, 512], BF16,
                                                   name=f"ctt{b}_{i}",
                                                   tag=f"ctt{b}_{i % 4}")
                    ct_t = ct_t_tiles[key]
                    rc = rcp.tile([128, 2], BF16, tag="rc")
                    with nc.allow_low_precision(reason="softmax recip bf16"):
                        nc.vector.reciprocal(out=rc, in_=av[:, :, 64])
                    dst = ct_t[:, 128 * hp:128 * hp + 128].rearrange(
                        "p (h w) -> p h w", h=2)
                    nc.vector.tensor_mul(
                        dst, av[:, :, 0:64],
                        rc[:, :, None].broadcast_to([128, 2, 64]))
                    if last_hp:
                        finish_block(b, i, ct_t, tail=tail)

                import functools
                for i in range(4 * c, 4 * c + 4):
                    thunks.append(functools.partial(do_av, i))
                return thunks

            def finish_block(b, i, ct_t, tail=False):
                """transpose ct_t -> feature-major, then queue out-proj."""
                ct_i = cti.tile([128, 4, 128], BF16, tag="cti")
                nc.sync.dma_start_transpose(ct_i[:, :, :], ct_t[:, :])
                del ct_t_tiles[(b, i)]

                def final(b=b, i=i, ct_i=ct_i):
                    psO = shp.tile([128, 512], F32, tag="sh")
                    for db in range(4):
                        nc.tensor.matmul(
                            psO, ct_i[:, db, :], wot_sb[:, db, :],
                            start=(db == 0), stop=(db == 3 and not has_bo))
                    if has_bo:
                        nc.tensor.matmul(psO, ones_k1, bob_sb[0:1, :],
                                         start=False, stop=True)
                    ot = outp.tile([128, 512], F32)
                    psum_copy(ot, psO)
                    row = b * S + i * 128
                    nc.sync.dma_start(out=out[row:row + 128, :], in_=ot)

                staged.append((pop_ctr[0], final))

            # ---- emission schedule (software-pipelined) ----
            # upfront: only what iteration 0's scores need; the rest of the
            # projections become ordered fillers consumed during attention.
            import functools
            # upfront: iteration 0 (b0, hp0, c0) needs only ob0/ch0
            kq_half(0, 0, "k")
            kq_half(0, 0, "q")
            # deadline-ordered fillers matching the b0c0,b0c1,b1c1,b1c0 seq;
            # keys let consumers force-emit their prerequisites in time
            def FK(ob, ch):
                fillers.append((("kq", ob, ch), functools.partial(kq_group, ob, ch)))

            def FV(nt):
                fillers.append((("vp", nt), functools.partial(v_proj, nt)))

            FK(1, 0)
            FK(2, 0)
            FV(0)
            FV(1)
            FK(3, 0)
            FV(2)
            FV(3)
            FK(0, 1)
            FK(1, 1)
            FV(4)
            FV(5)
            FK(2, 1)
            FV(6)
            FV(7)
            FK(3, 1)
            for ob in range(4):
                FK(ob, 3)
                FK(ob, 2)
            for nt in range(12, 16):
                FV(nt)
            for nt in range(8, 12):
                FV(nt)

            seq = []
            border = {0: (0, 1), 1: (1, 0)}
            for b in range(BL):
                for c in border[b % 2] if NCH == 2 else range(NCH):
                    for hp in range(4):
                        seq.append((b, hp, c))

            # iteration k's AV phase is interleaved with iteration k+2's
            # scores/exp packs (2-deep software pipeline): by the time an AV
            # runs, its exps retired during iteration k+1, so PE never waits
            # on ACT across iteration boundaries
            pend = []      # queue of AV thunk lists
            nseq = len(seq)
            for it, (b, hp, c) in enumerate(seq):
                # scores need this iteration's kq projections emitted first
                for ch in ([2 * b] if c == 0 else [2 * b, 2 * b + 1]):
                    need_filler(("kq", hp, ch))
                ex_t = {}
                packs = attention_packs(b, hp, c, ex_t)
                avs = attention_avs(b, hp, c, ex_t, last_hp=(hp == 3),
                                    tail=False)
                ready = pend.pop(0) if (len(pend) >= 2 or
                                        (pend and it == nseq - 1)) else []
                pops = 0
                # cap pops in the first (b0) half so fillers remain for the
                # ACT-bound b1c1 phase; first iterations also delay pops so
                # a not-yet-loaded filler can't head-of-line block PE
                cap = 4 if it < nseq // 2 else (6 if it < 3 * nseq // 4 else 99)
                for x in range(max(len(packs), len(ready))):
                    do_pop = (it >= 2 or x >= 2) and pops < cap
                    if x < len(packs):
                        packs[x]()
                        if do_pop:
                            pop_filler(1)
                            pops += 1
                    if x < len(ready):
                        ready[x]()
                        if do_pop and pops < cap:
                            pop_filler(1)
                            pops += 1
                pend.append(avs)
            for avs in pend:
                for av in avs:
                    av()
                    pop_filler(1)
            while fillers or staged:
                pop_filler(1)

    return nc


_prog_cache = {}


def kernel(q, k, v, mask, zero_pad, Wk, bk, Wv, bv, Wo, bo):
    global LAST_SIM_NS, LAST_EXEC_NS
    q = np.asarray(q, dtype=np.float32)
    k = np.asarray(k, dtype=np.float32)
    v = np.asarray(v, dtype=np.float32)
    Wk = np.asarray(Wk, dtype=np.float32)
    Wv = np.asarray(Wv, dtype=np.float32)
    Wo = np.asarray(Wo, dtype=np.float32)
    bk = np.asarray(bk, dtype=np.float32).reshape(D)
    bv = np.asarray(bv, dtype=np.float32).reshape(D)
    bo = np.asarray(bo, dtype=np.float32).reshape(D)
    mask2d = np.asarray(mask).reshape(S, S).astype(bool)
    zp = int(np.asarray(zero_pad))

    status, patterns = _classify_mask(mask2d)
    plan, first_j = _plan_chunks(status, patterns)
    nmix = len(patterns)
    has_bk = bool(np.any(bk))
    has_bv = bool(np.any(bv))
    has_bo = bool(np.any(bo))

    sig = (tuple(tuple(r) for r in status), nmix, has_bk, has_bv, has_bo)
    if sig not in _prog_cache:
        nc_new = _build(plan, first_j, nmix, has_bk, has_bv, has_bo)
        legalize_waits(nc_new)   # hardware-only pass (sim runs pre-legalized)
        _prog_cache[sig] = nc_new
    nc = _prog_cache[sig]

    def _sbuf_layout(wt):
        # [D, X] -> [128, 4, X]: row d = a*128+p  ->  [p, a, :]
        return np.ascontiguousarray(wt.reshape(4, 128, -1).transpose(1, 0, 2))

    # wkt grouped by ob block: [4, 128, 4, 128], wkt[ob][p, db, c] =
    # Wk.T[db*128+p, ob*128+c]
    wkt = np.ascontiguousarray(
        _sbuf_layout(Wk.T.astype(BF)).reshape(128, 4, 4, 128)
        .transpose(2, 0, 1, 3))
    wvt = _sbuf_layout(Wv.T.astype(BF))
    wot = _sbuf_layout(Wo.T.astype(BF))
    bk32 = np.ascontiguousarray(bk.reshape(4, 128).T).astype(np.float32)
    bvb = bv.reshape(1, D).astype(BF)
    bob = bo.reshape(1, D).astype(BF)
    mixmul = (np.stack(patterns) if patterns
              else np.zeros((1, 128, 128), np.float32)).astype(BF)

    common = dict(wkt=wkt, wvt=wvt, wot=wot, bk32=bk32, bvb=bvb, bob=bob,
                  mixmul=mixmul)
    in_maps = []
    for ci in range(NCORES):
        sl = slice(ci * BL, (ci + 1) * BL)
        in_maps.append(dict(
            qt=_sbuf_layout(q[sl].reshape(N, D).T.astype(BF)),
            kt=_sbuf_layout(k[sl].reshape(N, D).T.astype(BF)),
            vt=_sbuf_layout(v[sl].reshape(N, D).T.astype(BF)),
            **common))

    if os.environ.get("BASS_KERNEL_SIM_TIME"):
        from concourse.timeline_sim import TimelineSim
        LAST_SIM_NS = TimelineSim(nc).simulate()

    res = run_bass_kernel_spmd(nc, in_maps, list(range(NCORES)))
    LAST_EXEC_NS = res.exec_time_ns

    outs = [res.results[ci]["out"].reshape(BL, S, D) for ci in range(NCORES)]
    full = np.concatenate(outs, axis=0)
    if zp:
        full[:, 0, :] = bo
    return full



# revision 4
# speedup vs baseline: 1.0223x; 1.0223x over previous
"""Trainium2 Bass kernel for nn_MultiHeadAttention_47579647705431.

Multi-head attention (8 heads, dim 512, seq 1024, batch 16) with:
  - shared key/query linear (key_query_same=True: q and k both use Wk/bk)
  - causal (or arbitrary block-structured) mask
  - SimpleKT zero_pad: attention row 0 zeroed => out[:, 0, :] = bo

Sharding: data-parallel over batch across 8 NeuronCores (2 batches/core).

Per-core pipeline (all matmuls bf16, fp32 PSUM):
  1. kp/qp = Wk.T-stationary projections -> feature-major [o, n] bf16
  2. vp    = token-major projection [n, o] bf16 with interleaved ones
             columns (stride-65) providing the softmax denominator column
  3. per (b, hp, c): scores^T st [t, s] via K=64 row-packed matmuls;
     exp on ACT (scale 1/8 folded, several j-blocks packed per
     activation); causal/diagonal masking as a 0/1 multiply on DVE
  4. AV with SWAPPED operands: stationary = ex [t, s-block], moving =
     vp [t, 65] -> av PSUM [s, 2, 65] per (b, i, hp).  Cost = 65 free
     columns per (i, j, head) instead of 512 -- half the PE cycles of
     the stationary-vp form, and the denominator lands per-partition so
     normalization is a DVE reciprocal + broadcast multiply (no PE
     broadcast matmuls, no mask identity matmuls).
  5. ct_t token-major [s, 512] per (b, i) -> feature-major ct_i
     [128, 4, 128] via one XBAR dma_start_transpose
  6. out projection per (b, i) (ct_i-stationary) -> [128, 512] f32 -> DRAM

The walrus build here supports ONE sync wait per instruction; Tile emits
more. legalize_waits() hoists extra waits onto same-engine NoOps.
"""

import os
from contextlib import ExitStack

import numpy as np
import ml_dtypes

import concourse.bass as bass
import concourse.mybir as mybir
import concourse.tile as tile
from concourse.bass_utils import run_bass_kernel_spmd

F32 = mybir.dt.float32
BF16 = mybir.dt.bfloat16
BF = ml_dtypes.bfloat16

B, S, D, H, DH = 16, 1024, 512, 8, 64
NCORES = 8
BL = B // NCORES          # batches per core
N = BL * S                # tokens per core
NB = S // 128             # 128-blocks per sequence (8)
HP = H // 2               # head pairs (= o-blocks of 128)
NCH = S // 512            # 512-chunks per sequence (2)

LAST_SIM_NS = None
LAST_EXEC_NS = None


def legalize_waits(nc):
    """Split multi-wait instructions: keep one wait, hoist the rest onto
    preceding same-engine NoOps (this walrus encodes 1 wait/instruction)."""
    for f in nc.m.functions:
        for blk in f.blocks:
            il = blk.instructions
            i = 0
            while i < len(il):
                inst = il[i]
                si = inst.sync_info
                if si is not None and si.on_wait and len(si.on_wait) > 1:
                    waits = list(si.on_wait)
                    for j, w in enumerate(waits[:-1]):
                        nop = mybir.InstNoOp(
                            name=f"{inst.name}-hw{j}",
                            sync_info=mybir.SyncInfo(on_wait=[w], on_update=[]),
                            bass_nofuse=True,
                            engine=inst.engine,
                        )
                        il.insert(i, nop)
                        i += 1
                    si.on_wait = waits[-1:]
                i += 1


def _classify_mask(mask2d):
    """Classify 128x128 blocks of the [S, S] bool mask (query s, key t).

    Returns (status[j][i], patterns) in scores-transposed coords:
    j = key(t) block, i = query(s) block. status: -1 skip, -2 full,
    >=0 index into patterns (multiplicative bf16 0/1 [t, s] blocks).
    """
    status = [[-1] * NB for _ in range(NB)]
    patterns = []
    pat_idx = {}
    for j in range(NB):
        for i in range(NB):
            blk = mask2d[i * 128:(i + 1) * 128, j * 128:(j + 1) * 128]  # [s, t]
            if blk.all():
                status[j][i] = -2
            elif not blk.any():
                status[j][i] = -1
            else:
                mul = np.where(blk.T, 1.0, 0.0).astype(BF)  # [t, s]
                key = mul.tobytes()
                if key not in pat_idx:
                    pat_idx[key] = len(patterns)
                    patterns.append(mul)
                status[j][i] = pat_idx[key]
    return status, patterns


def _plan_chunks(status, patterns):
    """Per (c, j): suffix run of non-skip query blocks within chunk c.

    Returns plan[c][j] = (w, mixes) where w = run width and mixes =
    [(col_offset_in_region, pattern_id), ...] for mixed blocks. Also
    first_j[c]. Asserts the suffix-nested structure the kernel relies on.
    """
    plan = [[None] * NB for _ in range(NCH)]
    first_j = [None] * NCH
    for c in range(NCH):
        i_lo, i_hi = 4 * c, 4 * c + 4
        prev_w = None
        for j in range(NB):
            sts = [status[j][i] for i in range(i_lo, i_hi)]
            nz = [k for k, s in enumerate(sts) if s != -1]
            if not nz:
                plan[c][j] = (0, [])
                continue
            # must be a contiguous suffix of the chunk
            if nz != list(range(nz[0], 4)):
                raise NotImplementedError("mask block structure not suffix-contiguous")
            w = 128 * len(nz)
            if prev_w is not None and w > prev_w:
                raise NotImplementedError("mask runs not nested over key blocks")
            prev_w = w
            mixes = [((k - nz[0]) * 128, sts[k]) for k in nz if sts[k] >= 0]
            plan[c][j] = (w, mixes)
            if first_j[c] is None:
                first_j[c] = j
    return plan, first_j


def _pack_js(plan, c):
    """Greedy-pack consecutive j runs so one st tile / one exp covers
    several j blocks.  Each pack's total 2w must fit 1024 f32 (4KB)."""
    js = [j for j in range(NB) if plan[c][j][0] > 0]
    packs = []
    cur, cur_sz = [], 0
    for j in js:
        sz = 2 * plan[c][j][0]
        if cur and cur_sz + sz > 1024:
            packs.append(cur)
            cur, cur_sz = [], 0
        cur.append(j)
        cur_sz += sz
    if cur:
        packs.append(cur)
    return packs


def _build(plan, first_j, nmix, has_bk, has_bv, has_bo):
    nc = bass.Bass()
    # boot: startup-critical inputs merged in compute order so the first
    # projection's operands stream in a few pipelined DMAs:
    #   [wkt_ob0 (512) | ktc0 db-major (2048) | qtc0 db-major (2048)]
    boot = nc.dram_tensor("boot", [128, 4608], BF16, kind="ExternalInput")
    qt = nc.dram_tensor("qt", [128, 4, N], BF16, kind="ExternalInput")
    kt = nc.dram_tensor("kt", [128, 4, N], BF16, kind="ExternalInput")
    vt = nc.dram_tensor("vt", [128, 4, N], BF16, kind="ExternalInput")
    wkt = nc.dram_tensor("wkt", [4, 128, 4, 128], BF16, kind="ExternalInput")
    wvt = nc.dram_tensor("wvt", [128, 4, D], BF16, kind="ExternalInput")
    wot = nc.dram_tensor("wot", [128, 4, D], BF16, kind="ExternalInput")
    bk32 = nc.dram_tensor("bk32", [128, 4], F32, kind="ExternalInput")
    bvb = nc.dram_tensor("bvb", [1, D], BF16, kind="ExternalInput")
    bob = nc.dram_tensor("bob", [1, D], BF16, kind="ExternalInput")
    mixmul = nc.dram_tensor("mixmul", [max(nmix, 1), 128, 128], BF16,
                            kind="ExternalInput")
    out = nc.dram_tensor("out", [N, D], F32, kind="ExternalOutput")

    with tile.TileContext(nc) as tc:
        with ExitStack() as ctx:
            sing = ctx.enter_context(tc.tile_pool(name="sing", bufs=1))
            expp = ctx.enter_context(tc.tile_pool(name="expp", bufs=21))
            rcp = ctx.enter_context(tc.tile_pool(name="rcp", bufs=4))
            ctp = ctx.enter_context(tc.tile_pool(name="ctp", bufs=2))
            cti = ctx.enter_context(tc.tile_pool(name="cti", bufs=4))
            outp = ctx.enter_context(tc.tile_pool(name="outp", bufs=4))
            stp = ctx.enter_context(tc.tile_pool(name="stp", bufs=2, space="PSUM"))
            avp = ctx.enter_context(tc.tile_pool(name="avp", bufs=2, space="PSUM"))
            shp = ctx.enter_context(tc.tile_pool(name="shp", bufs=2, space="PSUM"))

            # ---- input loads: critical-path first.  Attention-phase DMAs
            # go on sync; startup loads spread over scalar/sync/gpsimd.
            # kt0 on sync and wkt on scalar so the first projection's two
            # inputs stream through HWDGE back-to-back instead of serially
            kt_c = [None] * 4
            qt_c = [None] * 4
            vt_c = [None] * 4
            # boot tile: wkt_ob0 + ktc0 + qtc0 in one SBUF region, loaded by
            # 4 pipelined DMAs in compute order (each unblocks the next
            # projection matmuls)
            boot_sb = sing.tile([128, 4608], BF16, name="boot_sb")
            nc.sync.dma_start(out=boot_sb[:, 0:1024], in_=boot[:, 0:1024])
            nc.sync.dma_start(out=boot_sb[:, 1024:2560], in_=boot[:, 1024:2560])
            nc.sync.dma_start(out=boot_sb[:, 2560:3584], in_=boot[:, 2560:3584])
            nc.sync.dma_start(out=boot_sb[:, 3584:4608], in_=boot[:, 3584:4608])
            wkt_ob = []
            wkt_ob.append(boot_sb[:, 0:512].rearrange("p (db c) -> p db c", db=4))
            for ob in range(1, 4):
                t = sing.tile([128, 4, 128], BF16, tag=f"wktob{ob}",
                              name=f"wktob{ob}")
                wkt_ob.append(t)
            kt_c[0] = boot_sb[:, 512:2560].rearrange("p (db t) -> p db t", db=4)
            qt_c[0] = boot_sb[:, 2560:4608].rearrange("p (db t) -> p db t", db=4)
            # arrival order follows the iteration schedule
            # b0c0 -> b0c1 -> b1c1 -> b1c0: ch 0, 1, 3, 2
            for ci, ch in enumerate((1, 3, 2)):
                csl = slice(ch * 512, ch * 512 + 512)
                t = sing.tile([128, 4, 512], BF16, tag=f"ktc{ch}",
                              name=f"ktc{ch}")
                nc.sync.dma_start(out=t, in_=kt[:, :, csl])
                kt_c[ch] = t
                t = sing.tile([128, 4, 512], BF16, tag=f"qtc{ch}",
                              name=f"qtc{ch}")
                nc.scalar.dma_start(out=t, in_=qt[:, :, csl])
                qt_c[ch] = t
                if ci == 0:
                    for ob in range(1, 4):
                        nc.scalar.dma_start(out=wkt_ob[ob],
                                            in_=wkt[ob, :, :, :])
                    wvt_sb = sing.tile([128, 4, D], BF16)
                    nc.scalar.dma_start(out=wvt_sb, in_=wvt[:, :, :])
                # v loads ride the scalar HWDGE queue behind the more
                # critical kq inputs (per-engine priority order holds;
                # Pool SWDGE DGEs would jump the shared DMA engines)
                pch = (0, 1, 3)[ci]
                t = sing.tile([128, 4, 512], BF16, tag=f"vtc{pch}",
                              name=f"vtc{pch}")
                nc.scalar.dma_start(out=t, in_=vt[:, :, pch * 512:pch * 512 + 512])
                vt_c[pch] = t
            t = sing.tile([128, 4, 512], BF16, tag="vtc2f", name="vtc2f")
            nc.scalar.dma_start(out=t, in_=vt[:, :, 2 * 512:2 * 512 + 512])
            vt_c[2] = t
            mix_sb = sing.tile([128, max(nmix, 1), 128], BF16)
            nc.scalar.dma_start(out=mix_sb, in_=mixmul.rearrange("m t s -> t m s"))
            wot_sb = sing.tile([128, 4, D], BF16)
            nc.scalar.dma_start(out=wot_sb, in_=wot[:, :, :])
            bk_sb = None
            if has_bk:
                bk_sb = sing.tile([128, 4], F32)
                nc.sync.dma_start(out=bk_sb, in_=bk32[:, :])
            bvb_sb = bob_sb = ones_k1 = None
            if has_bv or has_bo:
                ones_k1 = sing.tile([1, 128], BF16)
                nc.vector.memset(ones_k1, 1.0)
            if has_bv:
                bvb_sb = sing.tile([1, D], BF16)
                nc.sync.dma_start(out=bvb_sb, in_=bvb[:, :])
            if has_bo:
                bob_sb = sing.tile([1, D], BF16)
                nc.sync.dma_start(out=bob_sb, in_=bob[:, :])

            kp_sb = sing.tile([128, 4, N], BF16)
            qp_sb = sing.tile([128, 4, N], BF16)
            vp_sb = sing.tile([128, N // 128, 520], BF16)

            # ones columns of vp (denominator trick)
            for nt in range(N // 128):
                nc.vector.memset(
                    vp_sb[:, nt, :].rearrange("p (h u) -> p h u", u=65)[:, :, 64:65],
                    1.0)

            fillers = []
            staged = []    # (pop_stamp, thunk): finals wait out their
                           # transpose latency before becoming poppable
            pop_ctr = [0]
            copy_rr = [0]

            def psum_copy(dst, src):
                # PSUM->SBUF copies on DVE; ACT stays exp-only and GPSIMD
                # cannot access PSUM
                nc.vector.tensor_copy(out=dst, in_=src)

            def kq_half(ob, ch, which):
                csl = slice(ch * 512, ch * 512 + 512)
                src = kt_c[ch] if which == "k" else qt_c[ch]
                dst = kp_sb if which == "k" else qp_sb
                ps = shp.tile([128, 512], F32, tag="sh", name=f"ps{which}")
                for db in range(4):
                    nc.tensor.matmul(
                        ps, wkt_ob[ob][:, db, :],
                        src[:, db, :], start=(db == 0), stop=(db == 3))
                if has_bk:
                    if which == "k":
                        nc.scalar.add(dst[:, ob, csl], ps, bk_sb[:, ob:ob + 1])
                    else:
                        nc.vector.tensor_scalar_add(
                            dst[:, ob, csl], ps, bk_sb[:, ob:ob + 1])
                else:
                    psum_copy(dst[:, ob, csl], ps)

            def kq_group(ob, ch):
                kq_half(ob, ch, "k")
                kq_half(ob, ch, "q")

            def v_proj(nt):
                psV = shp.tile([128, 512], F32, tag="sh")
                for db in range(4):
                    nc.tensor.matmul(
                        psV, vt_c[nt // 4][:, db, (nt % 4) * 128:(nt % 4) * 128 + 128],
                        wvt_sb[:, db, :], start=(db == 0),
                        stop=(db == 3 and not has_bv))
                if has_bv:
                    nc.tensor.matmul(psV, ones_k1, bvb_sb[0:1, :],
                                     start=False, stop=True)
                dst = vp_sb[:, nt, :].rearrange("p (h u) -> p h u", u=65)[:, :, 0:64]
                src = psV.rearrange("p (h u) -> p h u", u=64)
                psum_copy(dst, src)

            def pop_filler(k=1):
                for _ in range(k):
                    pop_ctr[0] += 1
                    while staged and staged[0][0] + 16 <= pop_ctr[0]:
                        fillers.append((("fin",), staged.pop(0)[1]))
                    if fillers:
                        fillers.pop(0)[1]()

            def need_filler(key):
                """Force-emit a specific filler now (dependency deadline)."""
                for fi, (k, thunk) in enumerate(fillers):
                    if k == key:
                        fillers.pop(fi)
                        thunk()
                        return

            ct_t_tiles = {}

            def attention_packs(b, hp, c, ex_t):
                """Thunks: scores+exp+mask, one per pack of j blocks."""
                packs = _pack_js(plan, c)
                thunks = []

                def do_pack(pack):
                    # PSUM bank rule: each matmul output must stay inside one
                    # 2KB bank.  h0 segments stack downward from col 512
                    # (bank 0), h1 segments upward from col 512 (bank 1); the
                    # exp covers the contiguous union [512-tw, 512+tw).
                    tw = sum(plan[c][j][0] for j in pack)
                    st = stp.tile([128, 1024], F32, tag="st")
                    ex = expp.tile([128, 1024], BF16, tag="ex")
                    pre = 0
                    for j in pack:
                        w, _ = plan[c][j]
                        tsl = slice(b * S + j * 128, b * S + j * 128 + 128)
                        ssl = slice(b * S + c * 512 + 512 - w,
                                    b * S + c * 512 + 512)
                        h0s = 512 - pre - w
                        h1s = 512 + pre
                        nc.tensor.matmul(st[:, h0s:h0s + w],
                                         kp_sb[0:64, hp, tsl],
                                         qp_sb[0:64, hp, ssl],
                                         start=True, stop=True)
                        nc.tensor.matmul(st[:, h1s:h1s + w],
                                         kp_sb[64:128, hp, tsl],
                                         qp_sb[64:128, hp, ssl],
                                         start=True, stop=True)
                        ex_t[j] = (ex, h0s, h1s, w)
                        pre += w
                    nc.scalar.activation(
                        ex[:, 512 - tw:512 + tw], st[:, 512 - tw:512 + tw],
                        mybir.ActivationFunctionType.Exp, scale=0.125)
                    # 0/1 mask multiply for mixed blocks (per head half)
                    for j in pack:
                        w, mixes = plan[c][j]
                        _, h0s, h1s, _ = ex_t[j]
                        for moff, pid in mixes:
                            for hs in (h0s, h1s):
                                sl = ex[:, hs + moff:hs + moff + 128]
                                nc.vector.tensor_mul(
                                    sl, sl, mix_sb[:, pid, :])

                for pack in packs:
                    import functools
                    thunks.append(functools.partial(do_pack, pack))
                return thunks

            def attention_avs(b, hp, c, ex_t, last_hp, tail=False):
                """Thunks: AV + normalize, one per query block i (swapped
                operands: ex stationary, vp moving)."""
                h0 = 2 * hp
                thunks = []

                def do_av(i):
                    js_i = []
                    for j, (ex, h0s, h1s, w) in ex_t.items():
                        i_start = 4 * c + 4 - w // 128
                        if i >= i_start:
                            o = (i - i_start) * 128
                            js_i.append((j, ex, (h0s + o, h1s + o)))
                    if not js_i:
                        return
                    for j, _, _ in js_i:
                        need_filler(("vp", b * NB + j))
                    av = avp.tile([128, 2, 65], F32, tag="av")
                    nmm = len(js_i) * 2
                    mi = 0
                    for j, ex, hss in js_i:
                        vrow = b * NB + j
                        for h in range(2):
                            # single accumulation group per av tile: PSUM
                            # zeroing is bank-granular (start marks the whole
                            # bank pending-zero; first write to each address
                            # assigns, later writes accumulate)
                            nc.tensor.matmul(
                                av[:, h, :],
                                ex[:, hss[h]:hss[h] + 128],
                                vp_sb[:, vrow,
                                      65 * (h0 + h):65 * (h0 + h) + 65],
                                start=(mi == 0), stop=(mi == nmm - 1),
                                skip_group_check=True)
                            mi += 1
                    # normalize: per-partition reciprocal + broadcast mul
                    key = (b, i)
                    if key not in ct_t_tiles:
                        ct_t_tiles[key] = ctp.tile([128, 512], BF16,
                                                   name=f"ctt{b}_{i}",
                                                   tag=f"ctt{b}_{i % 4}")
                    ct_t = ct_t_tiles[key]
                    rc = rcp.tile([128, 2], BF16, tag="rc")
                    with nc.allow_low_precision(reason="softmax recip bf16"):
                        nc.vector.reciprocal(out=rc, in_=av[:, :, 64])
                    dst = ct_t[:, 128 * hp:128 * hp + 128].rearrange(
                        "p (h w) -> p h w", h=2)
                    nc.vector.tensor_mul(
                        dst, av[:, :, 0:64],
                        rc[:, :, None].broadcast_to([128, 2, 64]))
                    if last_hp:
                        finish_block(b, i, ct_t, tail=tail)

                import functools
                for i in range(4 * c, 4 * c + 4):
                    thunks.append(functools.partial(do_av, i))
                return thunks

            def finish_block(b, i, ct_t, tail=False):
                """transpose ct_t -> feature-major, then queue out-proj."""
                ct_i = cti.tile([128, 4, 128], BF16, tag="cti")
                nc.sync.dma_start_transpose(ct_i[:, :, :], ct_t[:, :])
                del ct_t_tiles[(b, i)]

                def final(b=b, i=i, ct_i=ct_i):
                    psO = shp.tile([128, 512], F32, tag="sh")
                    for db in range(4):
                        nc.tensor.matmul(
                            psO, ct_i[:, db, :], wot_sb[:, db, :],
                            start=(db == 0), stop=(db == 3 and not has_bo))
                    if has_bo:
                        nc.tensor.matmul(psO, ones_k1, bob_sb[0:1, :],
                                         start=False, stop=True)
                    ot = outp.tile([128, 512], F32)
                    psum_copy(ot, psO)
                    row = b * S + i * 128
                    nc.sync.dma_start(out=out[row:row + 128, :], in_=ot)

                staged.append((pop_ctr[0], final))

            # ---- emission schedule (software-pipelined) ----
            # upfront: only what iteration 0's scores need; the rest of the
            # projections become ordered fillers consumed during attention.
            import functools
            # upfront: iteration 0 (b0, hp0, c0) needs only ob0/ch0
            kq_half(0, 0, "k")
            kq_half(0, 0, "q")
            # deadline-ordered fillers matching the b0c0,b0c1,b1c1,b1c0 seq;
            # keys let consumers force-emit their prerequisites in time
            def FK(ob, ch):
                fillers.append((("kq", ob, ch), functools.partial(kq_group, ob, ch)))

            def FV(nt):
                fillers.append((("vp", nt), functools.partial(v_proj, nt)))

            FK(1, 0)
            FK(2, 0)
            FV(0)
            FV(1)
            FK(3, 0)
            FV(2)
            FV(3)
            FK(0, 1)
            FK(1, 1)
            FV(4)
            FV(5)
            FK(2, 1)
            FV(6)
            FV(7)
            FK(3, 1)
            for ob in range(4):
                FK(ob, 3)
                FK(ob, 2)
            for nt in range(12, 16):
                FV(nt)
            for nt in range(8, 12):
                FV(nt)

            seq = []
            border = {0: (0, 1), 1: (1, 0)}
            for b in range(BL):
                for c in border[b % 2] if NCH == 2 else range(NCH):
                    for hp in range(4):
                        seq.append((b, hp, c))

            # iteration k's AV phase is interleaved with iteration k+2's
            # scores/exp packs (2-deep software pipeline): by the time an AV
            # runs, its exps retired during iteration k+1, so PE never waits
            # on ACT across iteration boundaries
            pend = []      # queue of AV thunk lists
            nseq = len(seq)
            for it, (b, hp, c) in enumerate(seq):
                # scores need this iteration's kq projections emitted first
                for ch in ([2 * b] if c == 0 else [2 * b, 2 * b + 1]):
                    need_filler(("kq", hp, ch))
                ex_t = {}
                packs = attention_packs(b, hp, c, ex_t)
                avs = attention_avs(b, hp, c, ex_t, last_hp=(hp == 3),
                                    tail=False)
                ready = pend.pop(0) if (len(pend) >= 2 or
                                        (pend and it == nseq - 1)) else []
                pops = 0
                # cap pops in the first (b0) half so fillers remain for the
                # ACT-bound b1c1 phase; first iterations also delay pops so
                # a not-yet-loaded filler can't head-of-line block PE
                cap = 4 if it < nseq // 2 else (6 if it < 3 * nseq // 4 else 99)
                for x in range(max(len(packs), len(ready))):
                    do_pop = (it >= 2 or x >= 2) and pops < cap
                    if x < len(packs):
                        packs[x]()
                        if do_pop:
                            pop_filler(1)
                            pops += 1
                    if x < len(ready):
                        ready[x]()
                        if do_pop and pops < cap:
                            pop_filler(1)
                            pops += 1
                pend.append(avs)
            for avs in pend:
                for av in avs:
                    av()
                    pop_filler(1)
            while fillers or staged:
                pop_filler(1)

    return nc


_prog_cache = {}


def kernel(q, k, v, mask, zero_pad, Wk, bk, Wv, bv, Wo, bo):
    global LAST_SIM_NS, LAST_EXEC_NS
    q = np.asarray(q, dtype=np.float32)
    k = np.asarray(k, dtype=np.float32)
    v = np.asarray(v, dtype=np.float32)
    Wk = np.asarray(Wk, dtype=np.float32)
    Wv = np.asarray(Wv, dtype=np.float32)
    Wo = np.asarray(Wo, dtype=np.float32)
    bk = np.asarray(bk, dtype=np.float32).reshape(D)
    bv = np.asarray(bv, dtype=np.float32).reshape(D)
    bo = np.asarray(bo, dtype=np.float32).reshape(D)
    mask2d = np.asarray(mask).reshape(S, S).astype(bool)
    zp = int(np.asarray(zero_pad))

    status, patterns = _classify_mask(mask2d)
    plan, first_j = _plan_chunks(status, patterns)
    nmix = len(patterns)
    has_bk = bool(np.any(bk))
    has_bv = bool(np.any(bv))
    has_bo = bool(np.any(bo))

    sig = (tuple(tuple(r) for r in status), nmix, has_bk, has_bv, has_bo)
    if sig not in _prog_cache:
        nc_new = _build(plan, first_j, nmix, has_bk, has_bv, has_bo)
        legalize_waits(nc_new)   # hardware-only pass (sim runs pre-legalized)
        _prog_cache[sig] = nc_new
    nc = _prog_cache[sig]

    def _sbuf_layout(wt):
        # [D, X] -> [128, 4, X]: row d = a*128+p  ->  [p, a, :]
        return np.ascontiguousarray(wt.reshape(4, 128, -1).transpose(1, 0, 2))

    # wkt grouped by ob block: [4, 128, 4, 128], wkt[ob][p, db, c] =
    # Wk.T[db*128+p, ob*128+c]
    wkt = np.ascontiguousarray(
        _sbuf_layout(Wk.T.astype(BF)).reshape(128, 4, 4, 128)
        .transpose(2, 0, 1, 3))
    wvt = _sbuf_layout(Wv.T.astype(BF))
    wot = _sbuf_layout(Wo.T.astype(BF))
    bk32 = np.ascontiguousarray(bk.reshape(4, 128).T).astype(np.float32)
    bvb = bv.reshape(1, D).astype(BF)
    bob = bo.reshape(1, D).astype(BF)
    mixmul = (np.stack(patterns) if patterns
              else np.zeros((1, 128, 128), np.float32)).astype(BF)

    common = dict(wkt=wkt, wvt=wvt, wot=wot, bk32=bk32, bvb=bvb, bob=bob,
                  mixmul=mixmul)
    wkt_ob0_flat = wkt[0].reshape(128, 512)
    in_maps = []
    for ci in range(NCORES):
        sl = slice(ci * BL, (ci + 1) * BL)
        qt_h = _sbuf_layout(q[sl].reshape(N, D).T.astype(BF))
        kt_h = _sbuf_layout(k[sl].reshape(N, D).T.astype(BF))
        boot_h = np.ascontiguousarray(np.concatenate(
            [wkt_ob0_flat,
             kt_h[:, :, 0:512].reshape(128, 2048),
             qt_h[:, :, 0:512].reshape(128, 2048)], axis=1))
        in_maps.append(dict(
            boot=boot_h,
            qt=qt_h,
            kt=kt_h,
            vt=_sbuf_layout(v[sl].reshape(N, D).T.astype(BF)),
            **common))

    if os.environ.get("BASS_KERNEL_SIM_TIME"):
        from concourse.timeline_sim import TimelineSim
        LAST_SIM_NS = TimelineSim(nc).simulate()

    res = run_bass_kernel_spmd(nc, in_maps, list(range(NCORES)))
    LAST_EXEC_NS = res.exec_time_ns

    outs = [res.results[ci]["out"].reshape(BL, S, D) for ci in range(NCORES)]
    full = np.concatenate(outs, axis=0)
    if zp:
        full[:, 0, :] = bo
    return full



# revision 8
# speedup vs baseline: 1.0317x; 1.0091x over previous
"""Trainium2 Bass kernel for nn_MultiHeadAttention_47579647705431.

Multi-head attention (8 heads, dim 512, seq 1024, batch 16) with:
  - shared key/query linear (key_query_same=True: q and k both use Wk/bk)
  - causal (or arbitrary block-structured) mask
  - SimpleKT zero_pad: attention row 0 zeroed => out[:, 0, :] = bo

Sharding: data-parallel over batch across 8 NeuronCores (2 batches/core).

Per-core pipeline (all matmuls bf16, fp32 PSUM):
  1. kp/qp = Wk.T-stationary projections -> feature-major [o, n] bf16
  2. vp    = token-major projection [n, o] bf16 with interleaved ones
             columns (stride-65) providing the softmax denominator column
  3. per (b, hp, c): scores^T st [t, s] via K=64 row-packed matmuls;
     exp on ACT (scale 1/8 folded, several j-blocks packed per
     activation); causal/diagonal masking as a 0/1 multiply on DVE
  4. AV with SWAPPED operands: stationary = ex [t, s-block], moving =
     vp [t, 65] -> av PSUM [s, 2, 65] per (b, i, hp).  Cost = 65 free
     columns per (i, j, head) instead of 512 -- half the PE cycles of
     the stationary-vp form, and the denominator lands per-partition so
     normalization is a DVE reciprocal + broadcast multiply (no PE
     broadcast matmuls, no mask identity matmuls).
  5. ct_t token-major [s, 512] per (b, i) -> feature-major ct_i
     [128, 4, 128] via one XBAR dma_start_transpose
  6. out projection per (b, i) (ct_i-stationary) -> [128, 512] f32 -> DRAM

The walrus build here supports ONE sync wait per instruction; Tile emits
more. legalize_waits() hoists extra waits onto same-engine NoOps.
"""

import os
from contextlib import ExitStack

import numpy as np
import ml_dtypes

import concourse.bass as bass
import concourse.mybir as mybir
import concourse.tile as tile
from concourse.bass_utils import run_bass_kernel_spmd

F32 = mybir.dt.float32
BF16 = mybir.dt.bfloat16
BF = ml_dtypes.bfloat16

B, S, D, H, DH = 16, 1024, 512, 8, 64
NCORES = 8
BL = B // NCORES          # batches per core
N = BL * S                # tokens per core
NB = S // 128             # 128-blocks per sequence (8)
HP = H // 2               # head pairs (= o-blocks of 128)
NCH = S // 512            # 512-chunks per sequence (2)

LAST_SIM_NS = None
LAST_EXEC_NS = None


def legalize_waits(nc):
    """Split multi-wait instructions: keep one wait, hoist the rest onto
    preceding same-engine NoOps (this walrus encodes 1 wait/instruction)."""
    for f in nc.m.functions:
        for blk in f.blocks:
            il = blk.instructions
            i = 0
            while i < len(il):
                inst = il[i]
                si = inst.sync_info
                if si is not None and si.on_wait and len(si.on_wait) > 1:
                    waits = list(si.on_wait)
                    for j, w in enumerate(waits[:-1]):
                        nop = mybir.InstNoOp(
                            name=f"{inst.name}-hw{j}",
                            sync_info=mybir.SyncInfo(on_wait=[w], on_update=[]),
                            bass_nofuse=True,
                            engine=inst.engine,
                        )
                        il.insert(i, nop)
                        i += 1
                    si.on_wait = waits[-1:]
                i += 1


def _classify_mask(mask2d):
    """Classify 128x128 blocks of the [S, S] bool mask (query s, key t).

    Returns (status[j][i], patterns) in scores-transposed coords:
    j = key(t) block, i = query(s) block. status: -1 skip, -2 full,
    >=0 index into patterns (multiplicative bf16 0/1 [t, s] blocks).
    """
    status = [[-1] * NB for _ in range(NB)]
    patterns = []
    pat_idx = {}
    for j in range(NB):
        for i in range(NB):
            blk = mask2d[i * 128:(i + 1) * 128, j * 128:(j + 1) * 128]  # [s, t]
            if blk.all():
                status[j][i] = -2
            elif not blk.any():
                status[j][i] = -1
            else:
                mul = np.where(blk.T, 1.0, 0.0).astype(BF)  # [t, s]
                key = mul.tobytes()
                if key not in pat_idx:
                    pat_idx[key] = len(patterns)
                    patterns.append(mul)
                status[j][i] = pat_idx[key]
    return status, patterns


def _plan_chunks(status, patterns):
    """Per (c, j): suffix run of non-skip query blocks within chunk c.

    Returns plan[c][j] = (w, mixes) where w = run width and mixes =
    [(col_offset_in_region, pattern_id), ...] for mixed blocks. Also
    first_j[c]. Asserts the suffix-nested structure the kernel relies on.
    """
    plan = [[None] * NB for _ in range(NCH)]
    first_j = [None] * NCH
    for c in range(NCH):
        i_lo, i_hi = 4 * c, 4 * c + 4
        prev_w = None
        for j in range(NB):
            sts = [status[j][i] for i in range(i_lo, i_hi)]
            nz = [k for k, s in enumerate(sts) if s != -1]
            if not nz:
                plan[c][j] = (0, [])
                continue
            # must be a contiguous suffix of the chunk
            if nz != list(range(nz[0], 4)):
                raise NotImplementedError("mask block structure not suffix-contiguous")
            w = 128 * len(nz)
            if prev_w is not None and w > prev_w:
                raise NotImplementedError("mask runs not nested over key blocks")
            prev_w = w
            mixes = [((k - nz[0]) * 128, sts[k]) for k in nz if sts[k] >= 0]
            plan[c][j] = (w, mixes)
            if first_j[c] is None:
                first_j[c] = j
    return plan, first_j


def _pack_js(plan, c):
    """Greedy-pack consecutive j runs so one st tile / one exp covers
    several j blocks.  Each pack's total 2w must fit 1024 f32 (4KB)."""
    js = [j for j in range(NB) if plan[c][j][0] > 0]
    packs = []
    cur, cur_sz = [], 0
    for j in js:
        sz = 2 * plan[c][j][0]
        if cur and cur_sz + sz > 1024:
            packs.append(cur)
            cur, cur_sz = [], 0
        cur.append(j)
        cur_sz += sz
    if cur:
        packs.append(cur)
    return packs


def _build(plan, first_j, nmix, has_bk, has_bv, has_bo):
    nc = bass.Bass()
    # boot: startup-critical inputs merged in compute order so the first
    # projection's operands stream in a few pipelined DMAs:
    #   [wkt_ob0 (512) | ktc0 db-major (2048) | qtc0 db-major (2048)]
    boot = nc.dram_tensor("boot", [128, 4608], BF16, kind="ExternalInput")
    qt = nc.dram_tensor("qt", [128, 4, N], BF16, kind="ExternalInput")
    kt = nc.dram_tensor("kt", [128, 4, N], BF16, kind="ExternalInput")
    vt = nc.dram_tensor("vt", [128, 4, N], BF16, kind="ExternalInput")
    wkt = nc.dram_tensor("wkt", [4, 128, 4, 128], BF16, kind="ExternalInput")
    wvt = nc.dram_tensor("wvt", [128, 4, D], BF16, kind="ExternalInput")
    wot = nc.dram_tensor("wot", [128, 4, D], BF16, kind="ExternalInput")
    bk32 = nc.dram_tensor("bk32", [128, 4], F32, kind="ExternalInput")
    bvb = nc.dram_tensor("bvb", [1, D], BF16, kind="ExternalInput")
    bob = nc.dram_tensor("bob", [1, D], BF16, kind="ExternalInput")
    mixmul = nc.dram_tensor("mixmul", [max(nmix, 1), 128, 128], BF16,
                            kind="ExternalInput")
    out = nc.dram_tensor("out", [N, D], F32, kind="ExternalOutput")

    with tile.TileContext(nc) as tc:
        with ExitStack() as ctx:
            sing = ctx.enter_context(tc.tile_pool(name="sing", bufs=1))
            expp = ctx.enter_context(tc.tile_pool(name="expp", bufs=21))
            rcp = ctx.enter_context(tc.tile_pool(name="rcp", bufs=4))
            ctp = ctx.enter_context(tc.tile_pool(name="ctp", bufs=2))
            cti = ctx.enter_context(tc.tile_pool(name="cti", bufs=4))
            outp = ctx.enter_context(tc.tile_pool(name="outp", bufs=4))
            stp = ctx.enter_context(tc.tile_pool(name="stp", bufs=2, space="PSUM"))
            avp = ctx.enter_context(tc.tile_pool(name="avp", bufs=2, space="PSUM"))
            shp = ctx.enter_context(tc.tile_pool(name="shp", bufs=2, space="PSUM"))

            # ---- input loads: critical-path first.  Attention-phase DMAs
            # go on sync; startup loads spread over scalar/sync/gpsimd.
            # kt0 on sync and wkt on scalar so the first projection's two
            # inputs stream through HWDGE back-to-back instead of serially
            kt_c = [None] * 4
            qt_c = [None] * 4
            vt_c = [None] * 4
            # boot tile: wkt_ob0 + interleaved k/q chunk-0 db slices, loaded
            # by 5 pipelined DMAs whose arrival order matches the db-
            # interleaved first projection (each DMA unblocks the next 1-2
            # matmuls, so PE starts at ~4us and never re-stalls)
            boot_sb = sing.tile([128, 4608], BF16, name="boot_sb")
            for lo, hi in ((0, 1024), (1024, 2048), (2048, 3072),
                           (3072, 4096), (4096, 4608)):
                nc.sync.dma_start(out=boot_sb[:, lo:hi], in_=boot[:, lo:hi])
            wkt_ob = []
            wkt_ob.append(boot_sb[:, 0:512].rearrange("p (db c) -> p db c", db=4))
            for ob in range(1, 4):
                t = sing.tile([128, 4, 128], BF16, tag=f"wktob{ob}",
                              name=f"wktob{ob}")
                wkt_ob.append(t)
            kq0 = boot_sb[:, 512:4608].rearrange("p (db two t) -> p two db t",
                                                 two=2, t=512)
            kt_c[0] = kq0[:, 0]
            qt_c[0] = kq0[:, 1]
            # all remaining loads on the single sync queue in strict
            # deadline order: SP issues every ~650ns (never holding waits)
            # and DMA_ENGINES FIFO == emission order, so nothing competes
            # with the startup-critical boot DMAs
            for ob in range(1, 4):
                nc.sync.dma_start(out=wkt_ob[ob], in_=wkt[ob, :, :, :])
            wvt_sb = sing.tile([128, 4, D], BF16)
            nc.sync.dma_start(out=wvt_sb, in_=wvt[:, :, :])

            def load_chunk(which, ch):
                t = sing.tile([128, 4, 512], BF16, tag=f"{which}tc{ch}",
                              name=f"{which}tc{ch}")
                src = kt if which == "k" else (qt if which == "q" else vt)
                nc.sync.dma_start(out=t, in_=src[:, :, ch * 512:ch * 512 + 512])
                (kt_c if which == "k" else (qt_c if which == "q" else vt_c))[ch] = t

            load_chunk("v", 0)
            mix_sb = sing.tile([128, max(nmix, 1), 128], BF16)
            nc.sync.dma_start(out=mix_sb, in_=mixmul.rearrange("m t s -> t m s"))
            load_chunk("k", 1)
            load_chunk("q", 1)
            wot_sb = sing.tile([128, 4, D], BF16)
            nc.sync.dma_start(out=wot_sb, in_=wot[:, :, :])
            load_chunk("v", 1)
            load_chunk("k", 3)
            load_chunk("q", 3)
            load_chunk("v", 3)
            load_chunk("k", 2)
            load_chunk("q", 2)
            load_chunk("v", 2)
            bk_sb = None
            if has_bk:
                bk_sb = sing.tile([128, 4], F32)
                nc.sync.dma_start(out=bk_sb, in_=bk32[:, :])
            bvb_sb = bob_sb = ones_k1 = None
            if has_bv or has_bo:
                ones_k1 = sing.tile([1, 128], BF16)
                nc.vector.memset(ones_k1, 1.0)
            if has_bv:
                bvb_sb = sing.tile([1, D], BF16)
                nc.sync.dma_start(out=bvb_sb, in_=bvb[:, :])
            if has_bo:
                bob_sb = sing.tile([1, D], BF16)
                nc.sync.dma_start(out=bob_sb, in_=bob[:, :])

            kp_sb = sing.tile([128, 4, N], BF16)
            qp_sb = sing.tile([128, 4, N], BF16)
            vp_sb = sing.tile([128, N // 128, 520], BF16)

            # ones columns of vp (denominator trick)
            for nt in range(N // 128):
                nc.vector.memset(
                    vp_sb[:, nt, :].rearrange("p (h u) -> p h u", u=65)[:, :, 64:65],
                    1.0)

            fillers = []
            staged = []    # (pop_stamp, thunk): finals wait out their
                           # transpose latency before becoming poppable
            pop_ctr = [0]
            copy_rr = [0]

            def psum_copy(dst, src):
                # PSUM->SBUF copies on DVE; ACT stays exp-only and GPSIMD
                # cannot access PSUM
                nc.vector.tensor_copy(out=dst, in_=src)

            def kq_half(ob, ch, which):
                csl = slice(ch * 512, ch * 512 + 512)
                src = kt_c[ch] if which == "k" else qt_c[ch]
                dst = kp_sb if which == "k" else qp_sb
                ps = shp.tile([128, 512], F32, tag="sh", name=f"ps{which}")
                for db in range(4):
                    nc.tensor.matmul(
                        ps, wkt_ob[ob][:, db, :],
                        src[:, db, :], start=(db == 0), stop=(db == 3))
                if has_bk:
                    if which == "k":
                        nc.scalar.add(dst[:, ob, csl], ps, bk_sb[:, ob:ob + 1])
                    else:
                        nc.vector.tensor_scalar_add(
                            dst[:, ob, csl], ps, bk_sb[:, ob:ob + 1])
                else:
                    psum_copy(dst[:, ob, csl], ps)

            def kq_group(ob, ch):
                kq_half(ob, ch, "k")
                kq_half(ob, ch, "q")

            def v_proj(nt):
                psV = shp.tile([128, 512], F32, tag="sh")
                for db in range(4):
                    nc.tensor.matmul(
                        psV, vt_c[nt // 4][:, db, (nt % 4) * 128:(nt % 4) * 128 + 128],
                        wvt_sb[:, db, :], start=(db == 0),
                        stop=(db == 3 and not has_bv))
                if has_bv:
                    nc.tensor.matmul(psV, ones_k1, bvb_sb[0:1, :],
                                     start=False, stop=True)
                dst = vp_sb[:, nt, :].rearrange("p (h u) -> p h u", u=65)[:, :, 0:64]
                src = psV.rearrange("p (h u) -> p h u", u=64)
                psum_copy(dst, src)

            def pop_filler(k=1):
                for _ in range(k):
                    pop_ctr[0] += 1
                    while staged and staged[0][0] + 16 <= pop_ctr[0]:
                        fillers.append((("fin",), staged.pop(0)[1]))
                    if fillers:
                        fillers.pop(0)[1]()

            def need_filler(key):
                """Force-emit a specific filler now (dependency deadline)."""
                for fi, (k, thunk) in enumerate(fillers):
                    if k == key:
                        fillers.pop(fi)
                        thunk()
                        return

            ct_t_tiles = {}

            def attention_packs(b, hp, c, ex_t):
                """Thunks: scores+exp+mask, one per pack of j blocks."""
                packs = _pack_js(plan, c)
                thunks = []

                def do_pack(pack):
                    # PSUM bank rule: each matmul output must stay inside one
                    # 2KB bank.  h0 segments stack downward from col 512
                    # (bank 0), h1 segments upward from col 512 (bank 1); the
                    # exp covers the contiguous union [512-tw, 512+tw).
                    tw = sum(plan[c][j][0] for j in pack)
                    st = stp.tile([128, 1024], F32, tag="st")
                    ex = expp.tile([128, 1024], BF16, tag="ex")
                    pre = 0
                    for j in pack:
                        w, _ = plan[c][j]
                        tsl = slice(b * S + j * 128, b * S + j * 128 + 128)
                        ssl = slice(b * S + c * 512 + 512 - w,
                                    b * S + c * 512 + 512)
                        h0s = 512 - pre - w
                        h1s = 512 + pre
                        nc.tensor.matmul(st[:, h0s:h0s + w],
                                         kp_sb[0:64, hp, tsl],
                                         qp_sb[0:64, hp, ssl],
                                         start=True, stop=True)
                        nc.tensor.matmul(st[:, h1s:h1s + w],
                                         kp_sb[64:128, hp, tsl],
                                         qp_sb[64:128, hp, ssl],
                                         start=True, stop=True)
                        ex_t[j] = (ex, h0s, h1s, w)
                        pre += w
                    nc.scalar.activation(
                        ex[:, 512 - tw:512 + tw], st[:, 512 - tw:512 + tw],
                        mybir.ActivationFunctionType.Exp, scale=0.125)
                    # 0/1 mask multiply for mixed blocks (per head half)
                    for j in pack:
                        w, mixes = plan[c][j]
                        _, h0s, h1s, _ = ex_t[j]
                        for moff, pid in mixes:
                            for hs in (h0s, h1s):
                                sl = ex[:, hs + moff:hs + moff + 128]
                                nc.vector.tensor_mul(
                                    sl, sl, mix_sb[:, pid, :])

                for pack in packs:
                    import functools
                    thunks.append(functools.partial(do_pack, pack))
                return thunks

            def attention_avs(b, hp, c, ex_t, last_hp, tail=False):
                """Thunks: AV + normalize, one per query block i (swapped
                operands: ex stationary, vp moving)."""
                h0 = 2 * hp
                thunks = []

                def do_av(i):
                    js_i = []
                    for j, (ex, h0s, h1s, w) in ex_t.items():
                        i_start = 4 * c + 4 - w // 128
                        if i >= i_start:
                            o = (i - i_start) * 128
                            js_i.append((j, ex, (h0s + o, h1s + o)))
                    if not js_i:
                        return
                    for j, _, _ in js_i:
                        need_filler(("vp", b * NB + j))
                    av = avp.tile([128, 2, 65], F32, tag="av")
                    nmm = len(js_i) * 2
                    mi = 0
                    for j, ex, hss in js_i:
                        vrow = b * NB + j
                        for h in range(2):
                            # single accumulation group per av tile: PSUM
                            # zeroing is bank-granular (start marks the whole
                            # bank pending-zero; first write to each address
                            # assigns, later writes accumulate)
                            nc.tensor.matmul(
                                av[:, h, :],
                                ex[:, hss[h]:hss[h] + 128],
                                vp_sb[:, vrow,
                                      65 * (h0 + h):65 * (h0 + h) + 65],
                                start=(mi == 0), stop=(mi == nmm - 1),
                                skip_group_check=True)
                            mi += 1
                    # normalize: per-partition reciprocal + broadcast mul
                    key = (b, i)
                    if key not in ct_t_tiles:
                        ct_t_tiles[key] = ctp.tile([128, 512], BF16,
                                                   name=f"ctt{b}_{i}",
                                                   tag=f"ctt{b}_{i % 4}")
                    ct_t = ct_t_tiles[key]
                    rc = rcp.tile([128, 2], BF16, tag="rc")
                    with nc.allow_low_precision(reason="softmax recip bf16"):
                        nc.vector.reciprocal(out=rc, in_=av[:, :, 64])
                    dst = ct_t[:, 128 * hp:128 * hp + 128].rearrange(
                        "p (h w) -> p h w", h=2)
                    nc.vector.tensor_mul(
                        dst, av[:, :, 0:64],
                        rc[:, :, None].broadcast_to([128, 2, 64]))
                    if last_hp:
                        finish_block(b, i, ct_t, tail=tail)

                import functools
                for i in range(4 * c, 4 * c + 4):
                    thunks.append(functools.partial(do_av, i))
                return thunks

            def finish_block(b, i, ct_t, tail=False):
                """transpose ct_t -> feature-major, then queue out-proj."""
                ct_i = cti.tile([128, 4, 128], BF16, tag="cti")
                nc.sync.dma_start_transpose(ct_i[:, :, :], ct_t[:, :])
                del ct_t_tiles[(b, i)]

                def final(b=b, i=i, ct_i=ct_i):
                    psO = shp.tile([128, 512], F32, tag="sh")
                    for db in range(4):
                        nc.tensor.matmul(
                            psO, ct_i[:, db, :], wot_sb[:, db, :],
                            start=(db == 0), stop=(db == 3 and not has_bo))
                    if has_bo:
                        nc.tensor.matmul(psO, ones_k1, bob_sb[0:1, :],
                                         start=False, stop=True)
                    ot = outp.tile([128, 512], F32)
                    psum_copy(ot, psO)
                    row = b * S + i * 128
                    nc.sync.dma_start(out=out[row:row + 128, :], in_=ot)

                staged.append((pop_ctr[0], final))

            # ---- emission schedule (software-pipelined) ----
            # upfront: only what iteration 0's scores need; the rest of the
            # projections become ordered fillers consumed during attention.
            import functools
            # upfront: iteration 0 (b0, hp0, c0) needs only ob0/ch0.
            # k/q matmuls interleaved at db granularity to match the boot
            # DMA arrival order (psk/psq accumulate in separate PSUM banks)
            psk = shp.tile([128, 512], F32, tag="sh", name="psk")
            psq = shp.tile([128, 512], F32, tag="sh", name="psq")
            for db in range(4):
                nc.tensor.matmul(psk, wkt_ob[0][:, db, :], kt_c[0][:, db, :],
                                 start=(db == 0), stop=(db == 3),
                                 skip_group_check=True)
                nc.tensor.matmul(psq, wkt_ob[0][:, db, :], qt_c[0][:, db, :],
                                 start=(db == 0), stop=(db == 3),
                                 skip_group_check=True)
            if has_bk:
                nc.scalar.add(kp_sb[:, 0, 0:512], psk, bk_sb[:, 0:1])
                nc.vector.tensor_scalar_add(qp_sb[:, 0, 0:512], psq,
                                            bk_sb[:, 0:1])
            else:
                psum_copy(kp_sb[:, 0, 0:512], psk)
                psum_copy(qp_sb[:, 0, 0:512], psq)
            # deadline-ordered fillers matching the b0c0,b0c1,b1c1,b1c0 seq;
            # keys let consumers force-emit their prerequisites in time
            def FK(ob, ch):
                fillers.append((("kq", ob, ch), functools.partial(kq_group, ob, ch)))

            def FV(nt):
                fillers.append((("vp", nt), functools.partial(v_proj, nt)))

            FK(1, 0)
            FK(2, 0)
            FV(0)
            FV(1)
            FK(3, 0)
            FV(2)
            FV(3)
            FK(0, 1)
            FK(1, 1)
            FV(4)
            FV(5)
            FK(2, 1)
            FV(6)
            FV(7)
            FK(3, 1)
            for ob in range(4):
                FK(ob, 3)
                FK(ob, 2)
            for nt in range(12, 16):
                FV(nt)
            for nt in range(8, 12):
                FV(nt)

            seq = []
            border = {0: (0, 1), 1: (1, 0)}
            for b in range(BL):
                for c in border[b % 2] if NCH == 2 else range(NCH):
                    for hp in range(4):
                        seq.append((b, hp, c))

            # iteration k's AV phase is interleaved with iteration k+2's
            # scores/exp packs (2-deep software pipeline): by the time an AV
            # runs, its exps retired during iteration k+1, so PE never waits
            # on ACT across iteration boundaries
            pend = []      # queue of AV thunk lists
            nseq = len(seq)
            for it, (b, hp, c) in enumerate(seq):
                # scores need this iteration's kq projections emitted first
                for ch in ([2 * b] if c == 0 else [2 * b, 2 * b + 1]):
                    need_filler(("kq", hp, ch))
                ex_t = {}
                packs = attention_packs(b, hp, c, ex_t)
                avs = attention_avs(b, hp, c, ex_t, last_hp=(hp == 3),
                                    tail=False)
                ready = pend.pop(0) if (len(pend) >= 2 or
                                        (pend and it == nseq - 1)) else []
                pops = 0
                # cap pops in the first (b0) half so fillers remain for the
                # ACT-bound b1c1 phase; first iterations also delay pops so
                # a not-yet-loaded filler can't head-of-line block PE
                cap = 4 if it < nseq // 2 else (6 if it < 3 * nseq // 4 else 99)
                for x in range(max(len(packs), len(ready))):
                    do_pop = (it >= 2 or x >= 2) and pops < cap
                    if x < len(packs):
                        packs[x]()
                        if do_pop:
                            pop_filler(1)
                            pops += 1
                    if x < len(ready):
                        ready[x]()
                        if do_pop and pops < cap:
                            pop_filler(1)
                            pops += 1
                pend.append(avs)
            for avs in pend:
                for av in avs:
                    av()
                    pop_filler(1)
            while fillers or staged:
                pop_filler(1)

    return nc


_prog_cache = {}


def kernel(q, k, v, mask, zero_pad, Wk, bk, Wv, bv, Wo, bo):
    global LAST_SIM_NS, LAST_EXEC_NS
    q = np.asarray(q, dtype=np.float32)
    k = np.asarray(k, dtype=np.float32)
    v = np.asarray(v, dtype=np.float32)
    Wk = np.asarray(Wk, dtype=np.float32)
    Wv = np.asarray(Wv, dtype=np.float32)
    Wo = np.asarray(Wo, dtype=np.float32)
    bk = np.asarray(bk, dtype=np.float32).reshape(D)
    bv = np.asarray(bv, dtype=np.float32).reshape(D)
    bo = np.asarray(bo, dtype=np.float32).reshape(D)
    mask2d = np.asarray(mask).reshape(S, S).astype(bool)
    zp = int(np.asarray(zero_pad))

    status, patterns = _classify_mask(mask2d)
    plan, first_j = _plan_chunks(status, patterns)
    nmix = len(patterns)
    has_bk = bool(np.any(bk))
    has_bv = bool(np.any(bv))
    has_bo = bool(np.any(bo))

    sig = (tuple(tuple(r) for r in status), nmix, has_bk, has_bv, has_bo)
    if sig not in _prog_cache:
        nc_new = _build(plan, first_j, nmix, has_bk, has_bv, has_bo)
        legalize_waits(nc_new)   # hardware-only pass (sim runs pre-legalized)
        _prog_cache[sig] = nc_new
    nc = _prog_cache[sig]

    def _sbuf_layout(wt):
        # [D, X] -> [128, 4, X]: row d = a*128+p  ->  [p, a, :]
        return np.ascontiguousarray(wt.reshape(4, 128, -1).transpose(1, 0, 2))

    # wkt grouped by ob block: [4, 128, 4, 128], wkt[ob][p, db, c] =
    # Wk.T[db*128+p, ob*128+c]
    wkt = np.ascontiguousarray(
        _sbuf_layout(Wk.T.astype(BF)).reshape(128, 4, 4, 128)
        .transpose(2, 0, 1, 3))
    wvt = _sbuf_layout(Wv.T.astype(BF))
    wot = _sbuf_layout(Wo.T.astype(BF))
    bk32 = np.ascontiguousarray(bk.reshape(4, 128).T).astype(np.float32)
    bvb = bv.reshape(1, D).astype(BF)
    bob = bo.reshape(1, D).astype(BF)
    mixmul = (np.stack(patterns) if patterns
              else np.zeros((1, 128, 128), np.float32)).astype(BF)

    common = dict(wkt=wkt, wvt=wvt, wot=wot, bk32=bk32, bvb=bvb, bob=bob,
                  mixmul=mixmul)
    wkt_ob0_flat = wkt[0].reshape(128, 512)
    in_maps = []
    for ci in range(NCORES):
        sl = slice(ci * BL, (ci + 1) * BL)
        qt_h = _sbuf_layout(q[sl].reshape(N, D).T.astype(BF))
        kt_h = _sbuf_layout(k[sl].reshape(N, D).T.astype(BF))
        # [db, {k,q}, t] interleaved to match the boot DMA pipeline
        kq0_h = np.stack([kt_h[:, :, 0:512], qt_h[:, :, 0:512]],
                         axis=2).reshape(128, 4096)
        boot_h = np.ascontiguousarray(
            np.concatenate([wkt_ob0_flat, kq0_h], axis=1))
        in_maps.append(dict(
            boot=boot_h,
            qt=qt_h,
            kt=kt_h,
            vt=_sbuf_layout(v[sl].reshape(N, D).T.astype(BF)),
            **common))

    if os.environ.get("BASS_KERNEL_SIM_TIME"):
        from concourse.timeline_sim import TimelineSim
        LAST_SIM_NS = TimelineSim(nc).simulate()

    res = run_bass_kernel_spmd(nc, in_maps, list(range(NCORES)))
    LAST_EXEC_NS = res.exec_time_ns

    outs = [res.results[ci]["out"].reshape(BL, S, D) for ci in range(NCORES)]
    full = np.concatenate(outs, axis=0)
    if zp:
        full[:, 0, :] = bo
    return full



# revision 16
# speedup vs baseline: 1.0431x; 1.0110x over previous
"""Trainium2 Bass kernel for nn_MultiHeadAttention_47579647705431.

Multi-head attention (8 heads, dim 512, seq 1024, batch 16) with:
  - shared key/query linear (key_query_same=True: q and k both use Wk/bk)
  - causal (or arbitrary block-structured) mask
  - SimpleKT zero_pad: attention row 0 zeroed => out[:, 0, :] = bo

Sharding: data-parallel over batch across 8 NeuronCores (2 batches/core).

Per-core pipeline (all matmuls bf16, fp32 PSUM):
  1. kp/qp = Wk.T-stationary projections -> feature-major [o, n] bf16
  2. vp    = token-major projection [n, o] bf16 with interleaved ones
             columns (stride-65) providing the softmax denominator column
  3. per (b, hp, c): scores^T st [t, s] via K=64 row-packed matmuls;
     exp on ACT (scale 1/8 folded, several j-blocks packed per
     activation); causal/diagonal masking as a 0/1 multiply on DVE
  4. AV with SWAPPED operands: stationary = ex [t, s-block], moving =
     vp [t, 65] -> av PSUM [s, 2, 65] per (b, i, hp).  Cost = 65 free
     columns per (i, j, head) instead of 512 -- half the PE cycles of
     the stationary-vp form, and the denominator lands per-partition so
     normalization is a DVE reciprocal + broadcast multiply (no PE
     broadcast matmuls, no mask identity matmuls).
  5. ct_t token-major [s, 512] per (b, i) -> feature-major ct_i
     [128, 4, 128] via one XBAR dma_start_transpose
  6. out projection per (b, i) (ct_i-stationary) -> [128, 512] f32 -> DRAM

The walrus build here supports ONE sync wait per instruction; Tile emits
more. legalize_waits() hoists extra waits onto same-engine NoOps.
"""

import os
from contextlib import ExitStack

import numpy as np
import ml_dtypes

import concourse.bass as bass
import concourse.mybir as mybir
import concourse.tile as tile
from concourse.bass_utils import run_bass_kernel_spmd

F32 = mybir.dt.float32
BF16 = mybir.dt.bfloat16
BF = ml_dtypes.bfloat16

B, S, D, H, DH = 16, 1024, 512, 8, 64
NCORES = 8
BL = B // NCORES          # batches per core
N = BL * S                # tokens per core
NB = S // 128             # 128-blocks per sequence (8)
HP = H // 2               # head pairs (= o-blocks of 128)
NCH = S // 512            # 512-chunks per sequence (2)

LAST_SIM_NS = None
LAST_EXEC_NS = None


def legalize_waits(nc):
    """Split multi-wait instructions: keep one wait, hoist the rest onto
    preceding same-engine NoOps (this walrus encodes 1 wait/instruction)."""
    for f in nc.m.functions:
        for blk in f.blocks:
            il = blk.instructions
            i = 0
            while i < len(il):
                inst = il[i]
                si = inst.sync_info
                if si is not None and si.on_wait and len(si.on_wait) > 1:
                    waits = list(si.on_wait)
                    for j, w in enumerate(waits[:-1]):
                        nop = mybir.InstNoOp(
                            name=f"{inst.name}-hw{j}",
                            sync_info=mybir.SyncInfo(on_wait=[w], on_update=[]),
                            bass_nofuse=True,
                            engine=inst.engine,
                        )
                        il.insert(i, nop)
                        i += 1
                    si.on_wait = waits[-1:]
                i += 1


def _classify_mask(mask2d):
    """Classify 128x128 blocks of the [S, S] bool mask (query s, key t).

    Returns (status[j][i], patterns) in scores-transposed coords:
    j = key(t) block, i = query(s) block. status: -1 skip, -2 full,
    >=0 index into patterns (multiplicative bf16 0/1 [t, s] blocks).
    """
    status = [[-1] * NB for _ in range(NB)]
    patterns = []
    pat_idx = {}
    for j in range(NB):
        for i in range(NB):
            blk = mask2d[i * 128:(i + 1) * 128, j * 128:(j + 1) * 128]  # [s, t]
            if blk.all():
                status[j][i] = -2
            elif not blk.any():
                status[j][i] = -1
            else:
                mul = np.where(blk.T, 1.0, 0.0).astype(BF)  # [t, s]
                key = mul.tobytes()
                if key not in pat_idx:
                    pat_idx[key] = len(patterns)
                    patterns.append(mul)
                status[j][i] = pat_idx[key]
    return status, patterns


def _plan_chunks(status, patterns):
    """Per (c, j): suffix run of non-skip query blocks within chunk c.

    Returns plan[c][j] = (w, mixes) where w = run width and mixes =
    [(col_offset_in_region, pattern_id), ...] for mixed blocks. Also
    first_j[c]. Asserts the suffix-nested structure the kernel relies on.
    """
    plan = [[None] * NB for _ in range(NCH)]
    first_j = [None] * NCH
    for c in range(NCH):
        i_lo, i_hi = 4 * c, 4 * c + 4
        prev_w = None
        for j in range(NB):
            sts = [status[j][i] for i in range(i_lo, i_hi)]
            nz = [k for k, s in enumerate(sts) if s != -1]
            if not nz:
                plan[c][j] = (0, [])
                continue
            # must be a contiguous suffix of the chunk
            if nz != list(range(nz[0], 4)):
                raise NotImplementedError("mask block structure not suffix-contiguous")
            w = 128 * len(nz)
            if prev_w is not None and w > prev_w:
                raise NotImplementedError("mask runs not nested over key blocks")
            prev_w = w
            mixes = [((k - nz[0]) * 128, sts[k]) for k in nz if sts[k] >= 0]
            plan[c][j] = (w, mixes)
            if first_j[c] is None:
                first_j[c] = j
    return plan, first_j


def _pack_js(plan, c):
    """Greedy-pack consecutive j runs so one st tile / one exp covers
    several j blocks.  Each pack's total 2w must fit 1024 f32 (4KB)."""
    js = [j for j in range(NB) if plan[c][j][0] > 0]
    packs = []
    cur, cur_sz = [], 0
    for j in js:
        sz = 2 * plan[c][j][0]
        if cur and cur_sz + sz > 1024:
            packs.append(cur)
            cur, cur_sz = [], 0
        cur.append(j)
        cur_sz += sz
    if cur:
        packs.append(cur)
    return packs


def _build(plan, first_j, nmix, has_bk, has_bv, has_bo):
    nc = bass.Bass()
    # boot: startup-critical inputs merged in compute order so the first
    # projection's operands stream in a few pipelined DMAs:
    #   [wkt_ob0 (512) | ktc0 db-major (2048) | qtc0 db-major (2048)]
    boot = nc.dram_tensor("boot", [128, 4608], BF16, kind="ExternalInput")
    qt = nc.dram_tensor("qt", [128, 4, N], BF16, kind="ExternalInput")
    kt = nc.dram_tensor("kt", [128, 4, N], BF16, kind="ExternalInput")
    vt = nc.dram_tensor("vt", [128, 4, N], BF16, kind="ExternalInput")
    wkt = nc.dram_tensor("wkt", [4, 128, 4, 128], BF16, kind="ExternalInput")
    wvt = nc.dram_tensor("wvt", [128, 4, D], BF16, kind="ExternalInput")
    wot = nc.dram_tensor("wot", [128, 4, D], BF16, kind="ExternalInput")
    bk32 = nc.dram_tensor("bk32", [128, 4], F32, kind="ExternalInput")
    bvb = nc.dram_tensor("bvb", [1, D], BF16, kind="ExternalInput")
    bob = nc.dram_tensor("bob", [1, D], BF16, kind="ExternalInput")
    mixmul = nc.dram_tensor("mixmul", [max(nmix, 1), 128, 128], BF16,
                            kind="ExternalInput")
    ident = nc.dram_tensor("ident", [128, 128], BF16, kind="ExternalInput")
    # bf16 output: halves output DMA transfer time; host converts to f32.
    # Rounding cost ~2^-9 relative, well inside the error budget.
    out = nc.dram_tensor("out", [N, D], BF16, kind="ExternalOutput")

    with tile.TileContext(nc) as tc:
        with ExitStack() as ctx:
            sing = ctx.enter_context(tc.tile_pool(name="sing", bufs=1))
            expp = ctx.enter_context(tc.tile_pool(name="expp", bufs=21))
            rcp = ctx.enter_context(tc.tile_pool(name="rcp", bufs=4))
            ctp = ctx.enter_context(tc.tile_pool(name="ctp", bufs=2))
            cti = ctx.enter_context(tc.tile_pool(name="cti", bufs=4))
            outp = ctx.enter_context(tc.tile_pool(name="outp", bufs=4))
            stp = ctx.enter_context(tc.tile_pool(name="stp", bufs=2, space="PSUM"))
            avp = ctx.enter_context(tc.tile_pool(name="avp", bufs=2, space="PSUM"))
            shp = ctx.enter_context(tc.tile_pool(name="shp", bufs=2, space="PSUM"))

            # ---- input loads: critical-path first.  Attention-phase DMAs
            # go on sync; startup loads spread over scalar/sync/gpsimd.
            # kt0 on sync and wkt on scalar so the first projection's two
            # inputs stream through HWDGE back-to-back instead of serially
            kt_c = [None] * 4
            qt_c = [None] * 4
            vt_c = [None] * 4
            # boot tile: wkt_ob0 + interleaved k/q chunk-0 db slices, loaded
            # by 5 pipelined DMAs whose arrival order matches the db-
            # interleaved first projection (each DMA unblocks the next 1-2
            # matmuls, so PE starts at ~4us and never re-stalls)
            boot_sb = sing.tile([128, 4608], BF16, name="boot_sb")
            for lo, hi in ((0, 1024), (1024, 2048), (2048, 3072),
                           (3072, 4096), (4096, 4608)):
                nc.sync.dma_start(out=boot_sb[:, lo:hi], in_=boot[:, lo:hi])
            wkt_ob = []
            wkt_ob.append(boot_sb[:, 0:512].rearrange("p (db c) -> p db c", db=4))
            for ob in range(1, 4):
                t = sing.tile([128, 4, 128], BF16, tag=f"wktob{ob}",
                              name=f"wktob{ob}")
                wkt_ob.append(t)
            kq0 = boot_sb[:, 512:4608].rearrange("p (db two t) -> p two db t",
                                                 two=2, t=512)
            kt_c[0] = kq0[:, 0]
            qt_c[0] = kq0[:, 1]
            # all remaining loads on the single sync queue in strict
            # deadline order: SP issues every ~650ns (never holding waits)
            # and DMA_ENGINES FIFO == emission order, so nothing competes
            # with the startup-critical boot DMAs
            for ob in range(1, 4):
                nc.sync.dma_start(out=wkt_ob[ob], in_=wkt[ob, :, :, :])
            wvt_sb = sing.tile([128, 4, D], BF16)
            nc.sync.dma_start(out=wvt_sb, in_=wvt[:, :, :])

            def load_chunk(which, ch):
                t = sing.tile([128, 4, 512], BF16, tag=f"{which}tc{ch}",
                              name=f"{which}tc{ch}")
                src = kt if which == "k" else (qt if which == "q" else vt)
                nc.sync.dma_start(out=t, in_=src[:, :, ch * 512:ch * 512 + 512])
                (kt_c if which == "k" else (qt_c if which == "q" else vt_c))[ch] = t

            load_chunk("v", 0)
            mix_sb = sing.tile([128, max(nmix, 1), 128], BF16)
            nc.sync.dma_start(out=mix_sb, in_=mixmul.rearrange("m t s -> t m s"))
            load_chunk("k", 1)
            load_chunk("q", 1)
            wot_sb = sing.tile([128, 4, D], BF16)
            nc.sync.dma_start(out=wot_sb, in_=wot[:, :, :])
            load_chunk("v", 1)
            load_chunk("k", 3)
            load_chunk("q", 3)
            load_chunk("v", 3)
            load_chunk("k", 2)
            load_chunk("q", 2)
            load_chunk("v", 2)
            ident_sb = sing.tile([128, 128], BF16)
            nc.sync.dma_start(out=ident_sb, in_=ident[:, :])
            bk_sb = None
            if has_bk:
                bk_sb = sing.tile([128, 4], F32)
                nc.sync.dma_start(out=bk_sb, in_=bk32[:, :])
            bvb_sb = bob_sb = ones_k1 = None
            if has_bv or has_bo:
                ones_k1 = sing.tile([1, 128], BF16)
                nc.vector.memset(ones_k1, 1.0)
            if has_bv:
                bvb_sb = sing.tile([1, D], BF16)
                nc.sync.dma_start(out=bvb_sb, in_=bvb[:, :])
            if has_bo:
                bob_sb = sing.tile([1, D], BF16)
                nc.sync.dma_start(out=bob_sb, in_=bob[:, :])

            kp_sb = sing.tile([128, 4, N], BF16)
            qp_sb = sing.tile([128, 4, N], BF16)
            vp_sb = sing.tile([128, N // 128, 520], BF16)

            # ones columns of vp (denominator trick)
            for nt in range(N // 128):
                nc.vector.memset(
                    vp_sb[:, nt, :].rearrange("p (h u) -> p h u", u=65)[:, :, 64:65],
                    1.0)

            fillers = []
            staged = []    # (pop_stamp, thunk): finals wait out their
                           # transpose latency before becoming poppable
            pop_ctr = [0]
            copy_rr = [0]

            def psum_copy(dst, src):
                # PSUM->SBUF copies on DVE; ACT stays exp-only and GPSIMD
                # cannot access PSUM
                nc.vector.tensor_copy(out=dst, in_=src)

            def kq_half(ob, ch, which):
                csl = slice(ch * 512, ch * 512 + 512)
                src = kt_c[ch] if which == "k" else qt_c[ch]
                dst = kp_sb if which == "k" else qp_sb
                ps = shp.tile([128, 512], F32, tag="sh", name=f"ps{which}")
                for db in range(4):
                    nc.tensor.matmul(
                        ps, wkt_ob[ob][:, db, :],
                        src[:, db, :], start=(db == 0), stop=(db == 3))
                if has_bk:
                    if which == "k":
                        nc.scalar.add(dst[:, ob, csl], ps, bk_sb[:, ob:ob + 1])
                    else:
                        nc.vector.tensor_scalar_add(
                            dst[:, ob, csl], ps, bk_sb[:, ob:ob + 1])
                else:
                    psum_copy(dst[:, ob, csl], ps)

            def kq_group(ob, ch):
                kq_half(ob, ch, "k")
                kq_half(ob, ch, "q")

            def v_proj(nt):
                psV = shp.tile([128, 512], F32, tag="sh")
                for db in range(4):
                    nc.tensor.matmul(
                        psV, vt_c[nt // 4][:, db, (nt % 4) * 128:(nt % 4) * 128 + 128],
                        wvt_sb[:, db, :], start=(db == 0),
                        stop=(db == 3 and not has_bv))
                if has_bv:
                    nc.tensor.matmul(psV, ones_k1, bvb_sb[0:1, :],
                                     start=False, stop=True)
                dst = vp_sb[:, nt, :].rearrange("p (h u) -> p h u", u=65)[:, :, 0:64]
                src = psV.rearrange("p (h u) -> p h u", u=64)
                psum_copy(dst, src)

            def pop_filler(k=1):
                for _ in range(k):
                    pop_ctr[0] += 1
                    while staged and staged[0][0] + 16 <= pop_ctr[0]:
                        fillers.append((("fin",), staged.pop(0)[1]))
                    if fillers:
                        fillers.pop(0)[1]()

            def need_filler(key):
                """Force-emit a specific filler now (dependency deadline)."""
                for fi, (k, thunk) in enumerate(fillers):
                    if k == key:
                        fillers.pop(fi)
                        thunk()
                        return

            ct_t_tiles = {}

            def attention_packs(b, hp, c, ex_t):
                """Thunks: scores+exp+mask, one per pack of j blocks."""
                packs = _pack_js(plan, c)
                thunks = []

                def do_pack(pack):
                    # PSUM bank rule: each matmul output must stay inside one
                    # 2KB bank.  h0 segments stack downward from col 512
                    # (bank 0), h1 segments upward from col 512 (bank 1); the
                    # exp covers the contiguous union [512-tw, 512+tw).
                    tw = sum(plan[c][j][0] for j in pack)
                    st = stp.tile([128, 1024], F32, tag="st")
                    ex = expp.tile([128, 1024], BF16, tag="ex")
                    pre = 0
                    for j in pack:
                        w, _ = plan[c][j]
                        tsl = slice(b * S + j * 128, b * S + j * 128 + 128)
                        ssl = slice(b * S + c * 512 + 512 - w,
                                    b * S + c * 512 + 512)
                        h0s = 512 - pre - w
                        h1s = 512 + pre
                        nc.tensor.matmul(st[:, h0s:h0s + w],
                                         kp_sb[0:64, hp, tsl],
                                         qp_sb[0:64, hp, ssl],
                                         start=True, stop=True)
                        nc.tensor.matmul(st[:, h1s:h1s + w],
                                         kp_sb[64:128, hp, tsl],
                                         qp_sb[64:128, hp, ssl],
                                         start=True, stop=True)
                        ex_t[j] = (ex, h0s, h1s, w)
                        pre += w
                    nc.scalar.activation(
                        ex[:, 512 - tw:512 + tw], st[:, 512 - tw:512 + tw],
                        mybir.ActivationFunctionType.Exp, scale=0.125)
                    # 0/1 mask multiply for mixed blocks (per head half)
                    for j in pack:
                        w, mixes = plan[c][j]
                        _, h0s, h1s, _ = ex_t[j]
                        for moff, pid in mixes:
                            for hs in (h0s, h1s):
                                sl = ex[:, hs + moff:hs + moff + 128]
                                nc.vector.tensor_mul(
                                    sl, sl, mix_sb[:, pid, :])

                for pack in packs:
                    import functools
                    thunks.append(functools.partial(do_pack, pack))
                return thunks

            def attention_avs(b, hp, c, ex_t, last_hp, tail=False):
                """Thunks: AV + normalize, one per query block i (swapped
                operands: ex stationary, vp moving)."""
                h0 = 2 * hp
                thunks = []

                def do_av(i):
                    js_i = []
                    for j, (ex, h0s, h1s, w) in ex_t.items():
                        i_start = 4 * c + 4 - w // 128
                        if i >= i_start:
                            o = (i - i_start) * 128
                            js_i.append((j, ex, (h0s + o, h1s + o)))
                    if not js_i:
                        return
                    for j, _, _ in js_i:
                        need_filler(("vp", b * NB + j))
                    av = avp.tile([128, 2, 65], F32, tag="av")
                    nmm = len(js_i) * 2
                    mi = 0
                    for j, ex, hss in js_i:
                        vrow = b * NB + j
                        for h in range(2):
                            # single accumulation group per av tile: PSUM
                            # zeroing is bank-granular (start marks the whole
                            # bank pending-zero; first write to each address
                            # assigns, later writes accumulate)
                            nc.tensor.matmul(
                                av[:, h, :],
                                ex[:, hss[h]:hss[h] + 128],
                                vp_sb[:, vrow,
                                      65 * (h0 + h):65 * (h0 + h) + 65],
                                start=(mi == 0), stop=(mi == nmm - 1),
                                skip_group_check=True)
                            mi += 1
                    # normalize: per-partition reciprocal + broadcast mul
                    key = (b, i)
                    if key not in ct_t_tiles:
                        ct_t_tiles[key] = ctp.tile([128, 512], BF16,
                                                   name=f"ctt{b}_{i}",
                                                   tag=f"ctt{b}_{i % 4}")
                    ct_t = ct_t_tiles[key]
                    rc = rcp.tile([128, 2], BF16, tag="rc")
                    with nc.allow_low_precision(reason="softmax recip bf16"):
                        nc.vector.reciprocal(out=rc, in_=av[:, :, 64])
                    dst = ct_t[:, 128 * hp:128 * hp + 128].rearrange(
                        "p (h w) -> p h w", h=2)
                    nc.vector.tensor_mul(
                        dst, av[:, :, 0:64],
                        rc[:, :, None].broadcast_to([128, 2, 64]))
                    if last_hp:
                        finish_block(b, i, ct_t, tail=tail)

                import functools
                for i in range(4 * c, 4 * c + 4):
                    thunks.append(functools.partial(do_av, i))
                return thunks

            def finish_block(b, i, ct_t, tail=False):
                """transpose ct_t -> feature-major, then queue out-proj.

                tail=True blocks (the last c-group) use PE transposes + an
                ACT copy instead of a DMA transpose: ~1us latency instead of
                ~3.3us of DMA issue+transfer+sem-prop on the critical tail.
                """
                ct_i = cti.tile([128, 4, 128], BF16, tag="cti")
                if tail:
                    stq = stp.tile([128, 1024], F32, tag="st",
                                   name=f"pst{b}_{i}")
                    psT = stq.bitcast(BF16)
                    for q4 in range(4):
                        nc.tensor.transpose(
                            psT[:, q4 * 128:q4 * 128 + 128],
                            ct_t[:, q4 * 128:q4 * 128 + 128], ident_sb)
                    nc.scalar.copy(ct_i.rearrange("p a c -> p (a c)"),
                                   psT[:, 0:512])
                else:
                    nc.sync.dma_start_transpose(ct_i[:, :, :], ct_t[:, :])
                del ct_t_tiles[(b, i)]

                def final(b=b, i=i, ct_i=ct_i, tail=tail):
                    psO = shp.tile([128, 512], F32, tag="sh")
                    for db in range(4):
                        nc.tensor.matmul(
                            psO, ct_i[:, db, :], wot_sb[:, db, :],
                            start=(db == 0), stop=(db == 3 and not has_bo))
                    if has_bo:
                        nc.tensor.matmul(psO, ones_k1, bob_sb[0:1, :],
                                         start=False, stop=True)
                    ot = outp.tile([128, 512], BF16)
                    row = b * S + i * 128
                    if tail:
                        # alternate copy engine and DMA queue so the last
                        # finals' copies and issues don't serialize
                        if i % 2 == 0:
                            nc.scalar.copy(ot, psO)
                        else:
                            psum_copy(ot, psO)
                        eng = nc.sync if i % 2 == 0 else nc.scalar
                        eng.dma_start(out=out[row:row + 128, :], in_=ot)
                    else:
                        psum_copy(ot, psO)
                        nc.sync.dma_start(out=out[row:row + 128, :], in_=ot)

                staged.append((pop_ctr[0] if not tail else pop_ctr[0] - 8,
                               final))

            # ---- emission schedule (software-pipelined) ----
            # upfront: only what iteration 0's scores need; the rest of the
            # projections become ordered fillers consumed during attention.
            import functools
            # upfront: iteration 0 (b0, hp0, c0) needs only ob0/ch0.
            # k/q matmuls interleaved at db granularity to match the boot
            # DMA arrival order (psk/psq accumulate in separate PSUM banks)
            psk = shp.tile([128, 512], F32, tag="sh", name="psk")
            psq = shp.tile([128, 512], F32, tag="sh", name="psq")
            for db in range(4):
                nc.tensor.matmul(psk, wkt_ob[0][:, db, :], kt_c[0][:, db, :],
                                 start=(db == 0), stop=(db == 3),
                                 skip_group_check=True)
                nc.tensor.matmul(psq, wkt_ob[0][:, db, :], qt_c[0][:, db, :],
                                 start=(db == 0), stop=(db == 3),
                                 skip_group_check=True)
            if has_bk:
                nc.scalar.add(kp_sb[:, 0, 0:512], psk, bk_sb[:, 0:1])
                nc.vector.tensor_scalar_add(qp_sb[:, 0, 0:512], psq,
                                            bk_sb[:, 0:1])
            else:
                psum_copy(kp_sb[:, 0, 0:512], psk)
                psum_copy(qp_sb[:, 0, 0:512], psq)
            # deadline-ordered fillers matching the b0c0,b0c1,b1c1,b1c0 seq;
            # keys let consumers force-emit their prerequisites in time
            def FK(ob, ch):
                fillers.append((("kq", ob, ch), functools.partial(kq_group, ob, ch)))

            def FV(nt):
                fillers.append((("vp", nt), functools.partial(v_proj, nt)))

            FK(1, 0)
            FK(2, 0)
            FV(0)
            FV(1)
            FK(3, 0)
            FV(2)
            FV(3)
            FK(0, 1)
            FK(1, 1)
            FV(4)
            FV(5)
            FK(2, 1)
            FV(6)
            FV(7)
            FK(3, 1)
            for ob in range(4):
                FK(ob, 3)
                FK(ob, 2)
            for nt in range(12, 16):
                FV(nt)
            for nt in range(8, 12):
                FV(nt)

            seq = []
            border = {0: (0, 1), 1: (1, 0)}
            for b in range(BL):
                for c in border[b % 2] if NCH == 2 else range(NCH):
                    for hp in range(4):
                        seq.append((b, hp, c))

            # iteration k's AV phase is interleaved with iteration k+2's
            # scores/exp packs (2-deep software pipeline): by the time an AV
            # runs, its exps retired during iteration k+1, so PE never waits
            # on ACT across iteration boundaries
            pend = []      # queue of AV thunk lists
            nseq = len(seq)
            for it, (b, hp, c) in enumerate(seq):
                # scores need this iteration's kq projections emitted first
                for ch in ([2 * b] if c == 0 else [2 * b, 2 * b + 1]):
                    need_filler(("kq", hp, ch))
                ex_t = {}
                packs = attention_packs(b, hp, c, ex_t)
                avs = attention_avs(b, hp, c, ex_t, last_hp=(hp == 3),
                                    tail=(it >= nseq - 4))
                ready = pend.pop(0) if (len(pend) >= 2 or
                                        (pend and it == nseq - 1)) else []
                pops = 0
                # cap pops in the first (b0) half so fillers remain for the
                # ACT-bound b1c1 phase; first iterations also delay pops so
                # a not-yet-loaded filler can't head-of-line block PE
                cap = 4 if it < nseq // 2 else (6 if it < 3 * nseq // 4 else 99)
                for x in range(max(len(packs), len(ready))):
                    do_pop = (it >= 2 or x >= 2) and pops < cap
                    if x < len(packs):
                        packs[x]()
                        if do_pop:
                            pop_filler(1)
                            pops += 1
                    if x < len(ready):
                        ready[x]()
                        if do_pop and pops < cap:
                            pop_filler(1)
                            pops += 1
                pend.append(avs)
            for avs in pend:
                for av in avs:
                    av()
                    pop_filler(1)
            while fillers or staged:
                pop_filler(1)

    return nc


_prog_cache = {}


def kernel(q, k, v, mask, zero_pad, Wk, bk, Wv, bv, Wo, bo):
    global LAST_SIM_NS, LAST_EXEC_NS
    q = np.asarray(q, dtype=np.float32)
    k = np.asarray(k, dtype=np.float32)
    v = np.asarray(v, dtype=np.float32)
    Wk = np.asarray(Wk, dtype=np.float32)
    Wv = np.asarray(Wv, dtype=np.float32)
    Wo = np.asarray(Wo, dtype=np.float32)
    bk = np.asarray(bk, dtype=np.float32).reshape(D)
    bv = np.asarray(bv, dtype=np.float32).reshape(D)
    bo = np.asarray(bo, dtype=np.float32).reshape(D)
    mask2d = np.asarray(mask).reshape(S, S).astype(bool)
    zp = int(np.asarray(zero_pad))

    status, patterns = _classify_mask(mask2d)
    plan, first_j = _plan_chunks(status, patterns)
    nmix = len(patterns)
    has_bk = bool(np.any(bk))
    has_bv = bool(np.any(bv))
    has_bo = bool(np.any(bo))

    sig = (tuple(tuple(r) for r in status), nmix, has_bk, has_bv, has_bo)
    if sig not in _prog_cache:
        nc_new = _build(plan, first_j, nmix, has_bk, has_bv, has_bo)
        legalize_waits(nc_new)   # hardware-only pass (sim runs pre-legalized)
        _prog_cache[sig] = nc_new
    nc = _prog_cache[sig]

    def _sbuf_layout(wt):
        # [D, X] -> [128, 4, X]: row d = a*128+p  ->  [p, a, :]
        return np.ascontiguousarray(wt.reshape(4, 128, -1).transpose(1, 0, 2))

    # wkt grouped by ob block: [4, 128, 4, 128], wkt[ob][p, db, c] =
    # Wk.T[db*128+p, ob*128+c]
    wkt = np.ascontiguousarray(
        _sbuf_layout(Wk.T.astype(BF)).reshape(128, 4, 4, 128)
        .transpose(2, 0, 1, 3))
    wvt = _sbuf_layout(Wv.T.astype(BF))
    wot = _sbuf_layout(Wo.T.astype(BF))
    bk32 = np.ascontiguousarray(bk.reshape(4, 128).T).astype(np.float32)
    bvb = bv.reshape(1, D).astype(BF)
    bob = bo.reshape(1, D).astype(BF)
    mixmul = (np.stack(patterns) if patterns
              else np.zeros((1, 128, 128), np.float32)).astype(BF)

    common = dict(wkt=wkt, wvt=wvt, wot=wot, bk32=bk32, bvb=bvb, bob=bob,
                  mixmul=mixmul, ident=np.eye(128, dtype=BF))
    wkt_ob0_flat = wkt[0].reshape(128, 512)
    in_maps = []
    for ci in range(NCORES):
        sl = slice(ci * BL, (ci + 1) * BL)
        qt_h = _sbuf_layout(q[sl].reshape(N, D).T.astype(BF))
        kt_h = _sbuf_layout(k[sl].reshape(N, D).T.astype(BF))
        # [db, {k,q}, t] interleaved to match the boot DMA pipeline
        kq0_h = np.stack([kt_h[:, :, 0:512], qt_h[:, :, 0:512]],
                         axis=2).reshape(128, 4096)
        boot_h = np.ascontiguousarray(
            np.concatenate([wkt_ob0_flat, kq0_h], axis=1))
        in_maps.append(dict(
            boot=boot_h,
            qt=qt_h,
            kt=kt_h,
            vt=_sbuf_layout(v[sl].reshape(N, D).T.astype(BF)),
            **common))

    if os.environ.get("BASS_KERNEL_SIM_TIME"):
        from concourse.timeline_sim import TimelineSim
        LAST_SIM_NS = TimelineSim(nc).simulate()

    res = run_bass_kernel_spmd(nc, in_maps, list(range(NCORES)))
    LAST_EXEC_NS = res.exec_time_ns

    outs = [np.asarray(res.results[ci]["out"], dtype=np.float32)
            .reshape(BL, S, D) for ci in range(NCORES)]
    full = np.concatenate(outs, axis=0)
    if zp:
        full[:, 0, :] = bo
    return full



# revision 22
# speedup vs baseline: 1.0623x; 1.0184x over previous
"""Trainium2 Bass kernel for nn_MultiHeadAttention_47579647705431.

Multi-head attention (8 heads, dim 512, seq 1024, batch 16) with:
  - shared key/query linear (key_query_same=True: q and k both use Wk/bk)
  - causal (or arbitrary block-structured) mask
  - SimpleKT zero_pad: attention row 0 zeroed => out[:, 0, :] = bo

Sharding: data-parallel over batch across 8 NeuronCores (2 batches/core).

Per-core pipeline (all matmuls bf16, fp32 PSUM):
  1. kp/qp = Wk.T-stationary projections -> feature-major [o, n] bf16
  2. vp    = token-major projection [n, o] bf16 with interleaved ones
             columns (stride-65) providing the softmax denominator column
  3. per (b, hp, c): scores^T st [t, s] via K=64 row-packed matmuls;
     exp on ACT (scale 1/8 folded, several j-blocks packed per
     activation); causal/diagonal masking as a 0/1 multiply on DVE
  4. AV with SWAPPED operands: stationary = ex [t, s-block], moving =
     vp [t, 65] -> av PSUM [s, 2, 65] per (b, i, hp).  Cost = 65 free
     columns per (i, j, head) instead of 512 -- half the PE cycles of
     the stationary-vp form, and the denominator lands per-partition so
     normalization is a DVE reciprocal + broadcast multiply (no PE
     broadcast matmuls, no mask identity matmuls).
  5. ct_t token-major [s, 512] per (b, i) -> feature-major ct_i
     [128, 4, 128] via one XBAR dma_start_transpose
  6. out projection per (b, i) (ct_i-stationary) -> [128, 512] f32 -> DRAM

The walrus build here supports ONE sync wait per instruction; Tile emits
more. legalize_waits() hoists extra waits onto same-engine NoOps.
"""

import os
from contextlib import ExitStack

import numpy as np
import ml_dtypes

import concourse.bass as bass
import concourse.mybir as mybir
import concourse.tile as tile
from concourse.bass_utils import run_bass_kernel_spmd

F32 = mybir.dt.float32
BF16 = mybir.dt.bfloat16
BF = ml_dtypes.bfloat16

B, S, D, H, DH = 16, 1024, 512, 8, 64
NCORES = 8
BL = B // NCORES          # batches per core
N = BL * S                # tokens per core
NB = S // 128             # 128-blocks per sequence (8)
HP = H // 2               # head pairs (= o-blocks of 128)
NCH = S // 512            # 512-chunks per sequence (2)

LAST_SIM_NS = None
LAST_EXEC_NS = None

# schedule tuning knobs (sweepable via TimelineSim)
TUNE = dict(
    cap1=4,        # pop cap while it < nseq*capfrac1
    cap2=6,        # pop cap while it < nseq*capfrac2
    capfrac1=0.5,
    capfrac2=0.75,
    stage_delay=16,   # pops a staged final waits before becoming poppable
    tail_stage_delay=8,   # same for tail (PE-transpose) finals
    tail_n=4,      # trailing iterations whose finals use the PE-transpose path
    b1_order=None,
    mask_on_pool=True,  # diag-mask multiplies on the idle GPSIMD engine
)


def legalize_waits(nc):
    """Split multi-wait instructions: keep one wait, hoist the rest onto
    preceding same-engine NoOps (this walrus encodes 1 wait/instruction)."""
    for f in nc.m.functions:
        for blk in f.blocks:
            il = blk.instructions
            i = 0
            while i < len(il):
                inst = il[i]
                si = inst.sync_info
                if si is not None and si.on_wait and len(si.on_wait) > 1:
                    waits = list(si.on_wait)
                    for j, w in enumerate(waits[:-1]):
                        nop = mybir.InstNoOp(
                            name=f"{inst.name}-hw{j}",
                            sync_info=mybir.SyncInfo(on_wait=[w], on_update=[]),
                            bass_nofuse=True,
                            engine=inst.engine,
                        )
                        il.insert(i, nop)
                        i += 1
                    si.on_wait = waits[-1:]
                i += 1


def _classify_mask(mask2d):
    """Classify 128x128 blocks of the [S, S] bool mask (query s, key t).

    Returns (status[j][i], patterns) in scores-transposed coords:
    j = key(t) block, i = query(s) block. status: -1 skip, -2 full,
    >=0 index into patterns (multiplicative bf16 0/1 [t, s] blocks).
    """
    status = [[-1] * NB for _ in range(NB)]
    patterns = []
    pat_idx = {}
    for j in range(NB):
        for i in range(NB):
            blk = mask2d[i * 128:(i + 1) * 128, j * 128:(j + 1) * 128]  # [s, t]
            if blk.all():
                status[j][i] = -2
            elif not blk.any():
                status[j][i] = -1
            else:
                mul = np.where(blk.T, 1.0, 0.0).astype(BF)  # [t, s]
                key = mul.tobytes()
                if key not in pat_idx:
                    pat_idx[key] = len(patterns)
                    patterns.append(mul)
                status[j][i] = pat_idx[key]
    return status, patterns


def _plan_chunks(status, patterns):
    """Per (c, j): suffix run of non-skip query blocks within chunk c.

    Returns plan[c][j] = (w, mixes) where w = run width and mixes =
    [(col_offset_in_region, pattern_id), ...] for mixed blocks. Also
    first_j[c]. Asserts the suffix-nested structure the kernel relies on.
    """
    plan = [[None] * NB for _ in range(NCH)]
    first_j = [None] * NCH
    for c in range(NCH):
        i_lo, i_hi = 4 * c, 4 * c + 4
        prev_w = None
        for j in range(NB):
            sts = [status[j][i] for i in range(i_lo, i_hi)]
            nz = [k for k, s in enumerate(sts) if s != -1]
            if not nz:
                plan[c][j] = (0, [])
                continue
            # must be a contiguous suffix of the chunk
            if nz != list(range(nz[0], 4)):
                raise NotImplementedError("mask block structure not suffix-contiguous")
            w = 128 * len(nz)
            if prev_w is not None and w > prev_w:
                raise NotImplementedError("mask runs not nested over key blocks")
            prev_w = w
            mixes = [((k - nz[0]) * 128, sts[k]) for k in nz if sts[k] >= 0]
            plan[c][j] = (w, mixes)
            if first_j[c] is None:
                first_j[c] = j
    return plan, first_j


def _pack_js(plan, c):
    """Greedy-pack consecutive j runs so one st tile / one exp covers
    several j blocks.  Each pack's total 2w must fit 1024 f32 (4KB)."""
    js = [j for j in range(NB) if plan[c][j][0] > 0]
    packs = []
    cur, cur_sz = [], 0
    for j in js:
        sz = 2 * plan[c][j][0]
        if cur and cur_sz + sz > 1024:
            packs.append(cur)
            cur, cur_sz = [], 0
        cur.append(j)
        cur_sz += sz
    if cur:
        packs.append(cur)
    return packs


def _build(plan, first_j, nmix, has_bk, has_bv, has_bo):
    nc = bass.Bass()
    # boot: startup-critical inputs merged in compute order so the first
    # projection's operands stream in a few pipelined DMAs:
    #   [wkt_ob0 (512) | ktc0 db-major (2048) | qtc0 db-major (2048)]
    boot = nc.dram_tensor("boot", [128, 4608], BF16, kind="ExternalInput")
    qt = nc.dram_tensor("qt", [128, 4, N], BF16, kind="ExternalInput")
    kt = nc.dram_tensor("kt", [128, 4, N], BF16, kind="ExternalInput")
    vt = nc.dram_tensor("vt", [128, 4, N], BF16, kind="ExternalInput")
    wkt = nc.dram_tensor("wkt", [4, 128, 4, 128], BF16, kind="ExternalInput")
    wvt = nc.dram_tensor("wvt", [128, 4, D], BF16, kind="ExternalInput")
    wot = nc.dram_tensor("wot", [128, 4, D], BF16, kind="ExternalInput")
    bk32 = nc.dram_tensor("bk32", [128, 4], F32, kind="ExternalInput")
    bvb = nc.dram_tensor("bvb", [1, D], BF16, kind="ExternalInput")
    bob = nc.dram_tensor("bob", [1, D], BF16, kind="ExternalInput")
    mixmul = nc.dram_tensor("mixmul", [max(nmix, 1), 128, 128], BF16,
                            kind="ExternalInput")
    ident = nc.dram_tensor("ident", [128, 128], BF16, kind="ExternalInput")
    # bf16 output: halves output DMA transfer time; host converts to f32.
    # Rounding cost ~2^-9 relative, well inside the error budget.
    out = nc.dram_tensor("out", [N, D], BF16, kind="ExternalOutput")

    with tile.TileContext(nc) as tc:
        with ExitStack() as ctx:
            sing = ctx.enter_context(tc.tile_pool(name="sing", bufs=1))
            expp = ctx.enter_context(tc.tile_pool(name="expp", bufs=21))
            rcp = ctx.enter_context(tc.tile_pool(name="rcp", bufs=4))
            ctp = ctx.enter_context(tc.tile_pool(name="ctp", bufs=2))
            cti = ctx.enter_context(tc.tile_pool(name="cti", bufs=4))
            outp = ctx.enter_context(tc.tile_pool(name="outp", bufs=4))
            stp = ctx.enter_context(tc.tile_pool(name="stp", bufs=2, space="PSUM"))
            avp = ctx.enter_context(tc.tile_pool(name="avp", bufs=2, space="PSUM"))
            shp = ctx.enter_context(tc.tile_pool(name="shp", bufs=2, space="PSUM"))

            # ---- input loads: critical-path first.  Attention-phase DMAs
            # go on sync; startup loads spread over scalar/sync/gpsimd.
            # kt0 on sync and wkt on scalar so the first projection's two
            # inputs stream through HWDGE back-to-back instead of serially
            kt_c = [None] * 4
            qt_c = [None] * 4
            vt_c = [None] * 4
            # boot tile: wkt_ob0 + interleaved k/q chunk-0 db slices, loaded
            # by 5 pipelined DMAs whose arrival order matches the db-
            # interleaved first projection (each DMA unblocks the next 1-2
            # matmuls, so PE starts at ~4us and never re-stalls)
            boot_sb = sing.tile([128, 4608], BF16, name="boot_sb")
            for lo, hi in ((0, 1024), (1024, 2048), (2048, 3072),
                           (3072, 4096), (4096, 4608)):
                nc.sync.dma_start(out=boot_sb[:, lo:hi], in_=boot[:, lo:hi])
            wkt_ob = []
            wkt_ob.append(boot_sb[:, 0:512].rearrange("p (db c) -> p db c", db=4))
            for ob in range(1, 4):
                t = sing.tile([128, 4, 128], BF16, tag=f"wktob{ob}",
                              name=f"wktob{ob}")
                wkt_ob.append(t)
            kq0 = boot_sb[:, 512:4608].rearrange("p (db two t) -> p two db t",
                                                 two=2, t=512)
            kt_c[0] = kq0[:, 0]
            qt_c[0] = kq0[:, 1]
            # all remaining loads on the single sync queue in strict
            # deadline order: SP issues every ~650ns (never holding waits)
            # and DMA_ENGINES FIFO == emission order, so nothing competes
            # with the startup-critical boot DMAs
            for ob in range(1, 4):
                nc.sync.dma_start(out=wkt_ob[ob], in_=wkt[ob, :, :, :])
            wvt_sb = sing.tile([128, 4, D], BF16)
            nc.sync.dma_start(out=wvt_sb, in_=wvt[:, :, :])

            def load_chunk(which, ch):
                t = sing.tile([128, 4, 512], BF16, tag=f"{which}tc{ch}",
                              name=f"{which}tc{ch}")
                src = kt if which == "k" else (qt if which == "q" else vt)
                nc.sync.dma_start(out=t, in_=src[:, :, ch * 512:ch * 512 + 512])
                (kt_c if which == "k" else (qt_c if which == "q" else vt_c))[ch] = t

            load_chunk("v", 0)
            mix_sb = sing.tile([128, max(nmix, 1), 128], BF16)
            nc.sync.dma_start(out=mix_sb, in_=mixmul.rearrange("m t s -> t m s"))
            load_chunk("k", 1)
            load_chunk("q", 1)
            wot_sb = sing.tile([128, 4, D], BF16)
            nc.sync.dma_start(out=wot_sb, in_=wot[:, :, :])
            load_chunk("v", 1)
            load_chunk("k", 3)
            load_chunk("q", 3)
            load_chunk("v", 3)
            load_chunk("k", 2)
            load_chunk("q", 2)
            load_chunk("v", 2)
            ident_sb = sing.tile([128, 128], BF16)
            nc.sync.dma_start(out=ident_sb, in_=ident[:, :])
            bk_sb = None
            if has_bk:
                bk_sb = sing.tile([128, 4], F32)
                nc.sync.dma_start(out=bk_sb, in_=bk32[:, :])
            bvb_sb = bob_sb = ones_k1 = None
            if has_bv or has_bo:
                ones_k1 = sing.tile([1, 128], BF16)
                nc.vector.memset(ones_k1, 1.0)
            if has_bv:
                bvb_sb = sing.tile([1, D], BF16)
                nc.sync.dma_start(out=bvb_sb, in_=bvb[:, :])
            if has_bo:
                bob_sb = sing.tile([1, D], BF16)
                nc.sync.dma_start(out=bob_sb, in_=bob[:, :])

            kp_sb = sing.tile([128, 4, N], BF16)
            qp_sb = sing.tile([128, 4, N], BF16)
            vp_sb = sing.tile([128, N // 128, 520], BF16)

            # ones columns of vp (denominator trick)
            for nt in range(N // 128):
                nc.vector.memset(
                    vp_sb[:, nt, :].rearrange("p (h u) -> p h u", u=65)[:, :, 64:65],
                    1.0)

            fillers = []
            staged = []    # (pop_stamp, thunk): finals wait out their
                           # transpose latency before becoming poppable
            pop_ctr = [0]
            copy_rr = [0]

            def psum_copy(dst, src):
                # PSUM->SBUF copies on DVE; ACT stays exp-only and GPSIMD
                # cannot access PSUM
                nc.vector.tensor_copy(out=dst, in_=src)

            def kq_half(ob, ch, which):
                csl = slice(ch * 512, ch * 512 + 512)
                src = kt_c[ch] if which == "k" else qt_c[ch]
                dst = kp_sb if which == "k" else qp_sb
                ps = shp.tile([128, 512], F32, tag="sh", name=f"ps{which}")
                for db in range(4):
                    nc.tensor.matmul(
                        ps, wkt_ob[ob][:, db, :],
                        src[:, db, :], start=(db == 0), stop=(db == 3))
                if has_bk:
                    if which == "k":
                        nc.scalar.add(dst[:, ob, csl], ps, bk_sb[:, ob:ob + 1])
                    else:
                        nc.vector.tensor_scalar_add(
                            dst[:, ob, csl], ps, bk_sb[:, ob:ob + 1])
                else:
                    psum_copy(dst[:, ob, csl], ps)

            def kq_group(ob, ch):
                kq_half(ob, ch, "k")
                kq_half(ob, ch, "q")

            def v_proj(nt):
                psV = shp.tile([128, 512], F32, tag="sh")
                for db in range(4):
                    nc.tensor.matmul(
                        psV, vt_c[nt // 4][:, db, (nt % 4) * 128:(nt % 4) * 128 + 128],
                        wvt_sb[:, db, :], start=(db == 0),
                        stop=(db == 3 and not has_bv))
                if has_bv:
                    nc.tensor.matmul(psV, ones_k1, bvb_sb[0:1, :],
                                     start=False, stop=True)
                dst = vp_sb[:, nt, :].rearrange("p (h u) -> p h u", u=65)[:, :, 0:64]
                src = psV.rearrange("p (h u) -> p h u", u=64)
                psum_copy(dst, src)

            def pop_filler(k=1):
                for _ in range(k):
                    pop_ctr[0] += 1
                    while staged and staged[0][0] + TUNE["stage_delay"] <= pop_ctr[0]:
                        fillers.append((("fin",), staged.pop(0)[1]))
                    if fillers:
                        fillers.pop(0)[1]()

            def need_filler(key):
                """Force-emit a specific filler now (dependency deadline)."""
                for fi, (k, thunk) in enumerate(fillers):
                    if k == key:
                        fillers.pop(fi)
                        thunk()
                        return

            ct_t_tiles = {}

            def attention_packs(b, hp, c, ex_t):
                """Thunks: scores+exp+mask, one per pack of j blocks."""
                packs = _pack_js(plan, c)
                thunks = []

                def do_pack(pack):
                    # PSUM bank rule: each matmul output must stay inside one
                    # 2KB bank.  h0 segments stack downward from col 512
                    # (bank 0), h1 segments upward from col 512 (bank 1); the
                    # exp covers the contiguous union [512-tw, 512+tw).
                    tw = sum(plan[c][j][0] for j in pack)
                    st = stp.tile([128, 1024], F32, tag="st")
                    ex = expp.tile([128, 1024], BF16, tag="ex")
                    pre = 0
                    for j in pack:
                        w, _ = plan[c][j]
                        tsl = slice(b * S + j * 128, b * S + j * 128 + 128)
                        ssl = slice(b * S + c * 512 + 512 - w,
                                    b * S + c * 512 + 512)
                        h0s = 512 - pre - w
                        h1s = 512 + pre
                        nc.tensor.matmul(st[:, h0s:h0s + w],
                                         kp_sb[0:64, hp, tsl],
                                         qp_sb[0:64, hp, ssl],
                                         start=True, stop=True)
                        nc.tensor.matmul(st[:, h1s:h1s + w],
                                         kp_sb[64:128, hp, tsl],
                                         qp_sb[64:128, hp, ssl],
                                         start=True, stop=True)
                        ex_t[j] = (ex, h0s, h1s, w)
                        pre += w
                    nc.scalar.activation(
                        ex[:, 512 - tw:512 + tw], st[:, 512 - tw:512 + tw],
                        mybir.ActivationFunctionType.Exp, scale=0.125)
                    # 0/1 mask multiply for mixed blocks (per head half)
                    mask_eng = (nc.gpsimd if TUNE.get("mask_on_pool")
                                else nc.vector)
                    for j in pack:
                        w, mixes = plan[c][j]
                        _, h0s, h1s, _ = ex_t[j]
                        for moff, pid in mixes:
                            for hs in (h0s, h1s):
                                sl = ex[:, hs + moff:hs + moff + 128]
                                mask_eng.tensor_mul(
                                    sl, sl, mix_sb[:, pid, :])

                for pack in packs:
                    import functools
                    thunks.append(functools.partial(do_pack, pack))
                return thunks

            def attention_avs(b, hp, c, ex_t, last_hp, tail=False):
                """Thunks: AV + normalize, one per query block i (swapped
                operands: ex stationary, vp moving)."""
                h0 = 2 * hp
                thunks = []

                def do_av(i):
                    js_i = []
                    for j, (ex, h0s, h1s, w) in ex_t.items():
                        i_start = 4 * c + 4 - w // 128
                        if i >= i_start:
                            o = (i - i_start) * 128
                            js_i.append((j, ex, (h0s + o, h1s + o)))
                    if not js_i:
                        return
                    for j, _, _ in js_i:
                        need_filler(("vp", b * NB + j))
                    av = avp.tile([128, 2, 65], F32, tag="av")
                    nmm = len(js_i) * 2
                    mi = 0
                    for j, ex, hss in js_i:
                        vrow = b * NB + j
                        for h in range(2):
                            # single accumulation group per av tile: PSUM
                            # zeroing is bank-granular (start marks the whole
                            # bank pending-zero; first write to each address
                            # assigns, later writes accumulate)
                            nc.tensor.matmul(
                                av[:, h, :],
                                ex[:, hss[h]:hss[h] + 128],
                                vp_sb[:, vrow,
                                      65 * (h0 + h):65 * (h0 + h) + 65],
                                start=(mi == 0), stop=(mi == nmm - 1),
                                skip_group_check=True)
                            mi += 1
                    # normalize: per-partition reciprocal + broadcast mul
                    key = (b, i)
                    if key not in ct_t_tiles:
                        ct_t_tiles[key] = ctp.tile([128, 512], BF16,
                                                   name=f"ctt{b}_{i}",
                                                   tag=f"ctt{b}_{i % 4}")
                    ct_t = ct_t_tiles[key]
                    rc = rcp.tile([128, 2], BF16, tag="rc")
                    with nc.allow_low_precision(reason="softmax recip bf16"):
                        nc.vector.reciprocal(out=rc, in_=av[:, :, 64])
                    dst = ct_t[:, 128 * hp:128 * hp + 128].rearrange(
                        "p (h w) -> p h w", h=2)
                    nc.vector.tensor_mul(
                        dst, av[:, :, 0:64],
                        rc[:, :, None].broadcast_to([128, 2, 64]))
                    if last_hp:
                        finish_block(b, i, ct_t, tail=tail)

                import functools
                for i in range(4 * c, 4 * c + 4):
                    thunks.append(functools.partial(do_av, i))
                return thunks

            def finish_block(b, i, ct_t, tail=False):
                """transpose ct_t -> feature-major, then queue out-proj.

                tail=True blocks (the last c-group) use PE transposes + an
                ACT copy instead of a DMA transpose: ~1us latency instead of
                ~3.3us of DMA issue+transfer+sem-prop on the critical tail.
                """
                ct_i = cti.tile([128, 4, 128], BF16, tag="cti")
                if tail:
                    stq = stp.tile([128, 1024], F32, tag="st",
                                   name=f"pst{b}_{i}")
                    psT = stq.bitcast(BF16)
                    for q4 in range(4):
                        nc.tensor.transpose(
                            psT[:, q4 * 128:q4 * 128 + 128],
                            ct_t[:, q4 * 128:q4 * 128 + 128], ident_sb)
                    # DVE, not ACT: ACT is still draining exps here (in-order
                    # engine => a copy there queues behind them), and the
                    # bf16->bf16 copy gets DVE's 2x mode
                    nc.vector.tensor_copy(
                        out=ct_i.rearrange("p a c -> p (a c)"),
                        in_=psT[:, 0:512])
                else:
                    nc.sync.dma_start_transpose(ct_i[:, :, :], ct_t[:, :])
                del ct_t_tiles[(b, i)]

                def final(b=b, i=i, ct_i=ct_i, tail=tail):
                    psO = shp.tile([128, 512], F32, tag="sh")
                    for db in range(4):
                        nc.tensor.matmul(
                            psO, ct_i[:, db, :], wot_sb[:, db, :],
                            start=(db == 0), stop=(db == 3 and not has_bo))
                    if has_bo:
                        nc.tensor.matmul(psO, ones_k1, bob_sb[0:1, :],
                                         start=False, stop=True)
                    ot = outp.tile([128, 512], BF16)
                    row = b * S + i * 128
                    if tail:
                        # alternate copy engine and DMA queue so the last
                        # finals' copies and issues don't serialize
                        if i % 2 == 0:
                            nc.scalar.copy(ot, psO)
                        else:
                            psum_copy(ot, psO)
                        eng = nc.sync if i % 2 == 0 else nc.scalar
                        eng.dma_start(out=out[row:row + 128, :], in_=ot)
                    else:
                        psum_copy(ot, psO)
                        nc.sync.dma_start(out=out[row:row + 128, :], in_=ot)

                staged.append((pop_ctr[0] if not tail else
                               pop_ctr[0] - TUNE["stage_delay"]
                               + TUNE["tail_stage_delay"], final))

            # ---- emission schedule (software-pipelined) ----
            # upfront: only what iteration 0's scores need; the rest of the
            # projections become ordered fillers consumed during attention.
            import functools
            # upfront: iteration 0 (b0, hp0, c0) needs only ob0/ch0.
            # k/q matmuls interleaved at db granularity to match the boot
            # DMA arrival order (psk/psq accumulate in separate PSUM banks)
            psk = shp.tile([128, 512], F32, tag="sh", name="psk")
            psq = shp.tile([128, 512], F32, tag="sh", name="psq")
            for db in range(4):
                nc.tensor.matmul(psk, wkt_ob[0][:, db, :], kt_c[0][:, db, :],
                                 start=(db == 0), stop=(db == 3),
                                 skip_group_check=True)
                nc.tensor.matmul(psq, wkt_ob[0][:, db, :], qt_c[0][:, db, :],
                                 start=(db == 0), stop=(db == 3),
                                 skip_group_check=True)
            if has_bk:
                nc.scalar.add(kp_sb[:, 0, 0:512], psk, bk_sb[:, 0:1])
                nc.vector.tensor_scalar_add(qp_sb[:, 0, 0:512], psq,
                                            bk_sb[:, 0:1])
            else:
                psum_copy(kp_sb[:, 0, 0:512], psk)
                psum_copy(qp_sb[:, 0, 0:512], psq)
            # deadline-ordered fillers matching the b0c0,b0c1,b1c1,b1c0 seq;
            # keys let consumers force-emit their prerequisites in time
            def FK(ob, ch):
                fillers.append((("kq", ob, ch), functools.partial(kq_group, ob, ch)))

            def FV(nt):
                fillers.append((("vp", nt), functools.partial(v_proj, nt)))

            FK(1, 0)
            FK(2, 0)
            FV(0)
            FV(1)
            FK(3, 0)
            FV(2)
            FV(3)
            FK(0, 1)
            FK(1, 1)
            FV(4)
            FV(5)
            FK(2, 1)
            FV(6)
            FV(7)
            FK(3, 1)
            for ob in range(4):
                FK(ob, 3)
                FK(ob, 2)
            for nt in range(12, 16):
                FV(nt)
            for nt in range(8, 12):
                FV(nt)

            seq = []
            border = {0: (0, 1), 1: (1, 0)}
            for b in range(BL):
                if NCH == 2 and b % 2 == 1 and TUNE.get("b1_order"):
                    # interleave the light c0 iterations among the heavy c1
                    # ones so the tail's exp backlog on ACT is smaller
                    for hp, c in TUNE["b1_order"]:
                        seq.append((b, hp, c))
                else:
                    for c in border[b % 2] if NCH == 2 else range(NCH):
                        for hp in range(4):
                            seq.append((b, hp, c))

            # iteration k's AV phase is interleaved with iteration k+2's
            # scores/exp packs (2-deep software pipeline): by the time an AV
            # runs, its exps retired during iteration k+1, so PE never waits
            # on ACT across iteration boundaries
            pend = []      # queue of AV thunk lists
            nseq = len(seq)
            for it, (b, hp, c) in enumerate(seq):
                # scores need this iteration's kq projections emitted first
                for ch in ([2 * b] if c == 0 else [2 * b, 2 * b + 1]):
                    need_filler(("kq", hp, ch))
                ex_t = {}
                packs = attention_packs(b, hp, c, ex_t)
                avs = attention_avs(b, hp, c, ex_t, last_hp=(hp == 3),
                                    tail=(it >= nseq - TUNE["tail_n"]))
                ready = pend.pop(0) if (len(pend) >= 2 or
                                        (pend and it == nseq - 1)) else []
                pops = 0
                # cap pops in the first (b0) half so fillers remain for the
                # ACT-bound b1c1 phase; first iterations also delay pops so
                # a not-yet-loaded filler can't head-of-line block PE
                cap = (TUNE["cap1"] if it < nseq * TUNE["capfrac1"] else
                       (TUNE["cap2"] if it < nseq * TUNE["capfrac2"] else 99))
                for x in range(max(len(packs), len(ready))):
                    do_pop = (it >= 2 or x >= 2) and pops < cap
                    if x < len(packs):
                        packs[x]()
                        if do_pop:
                            pop_filler(1)
                            pops += 1
                    if x < len(ready):
                        ready[x]()
                        if do_pop and pops < cap:
                            pop_filler(1)
                            pops += 1
                pend.append(avs)
            for avs in pend:
                for av in avs:
                    av()
                    pop_filler(1)
            while fillers or staged:
                pop_filler(1)

    return nc


_prog_cache = {}


def kernel(q, k, v, mask, zero_pad, Wk, bk, Wv, bv, Wo, bo):
    global LAST_SIM_NS, LAST_EXEC_NS
    q = np.asarray(q, dtype=np.float32)
    k = np.asarray(k, dtype=np.float32)
    v = np.asarray(v, dtype=np.float32)
    Wk = np.asarray(Wk, dtype=np.float32)
    Wv = np.asarray(Wv, dtype=np.float32)
    Wo = np.asarray(Wo, dtype=np.float32)
    bk = np.asarray(bk, dtype=np.float32).reshape(D)
    bv = np.asarray(bv, dtype=np.float32).reshape(D)
    bo = np.asarray(bo, dtype=np.float32).reshape(D)
    mask2d = np.asarray(mask).reshape(S, S).astype(bool)
    zp = int(np.asarray(zero_pad))

    status, patterns = _classify_mask(mask2d)
    plan, first_j = _plan_chunks(status, patterns)
    nmix = len(patterns)
    has_bk = bool(np.any(bk))
    has_bv = bool(np.any(bv))
    has_bo = bool(np.any(bo))

    sig = (tuple(tuple(r) for r in status), nmix, has_bk, has_bv, has_bo)
    if sig not in _prog_cache:
        nc_new = _build(plan, first_j, nmix, has_bk, has_bv, has_bo)
        legalize_waits(nc_new)   # hardware-only pass (sim runs pre-legalized)
        _prog_cache[sig] = nc_new
    nc = _prog_cache[sig]

    def _sbuf_layout(wt):
        # [D, X] -> [128, 4, X]: row d = a*128+p  ->  [p, a, :]
        return np.ascontiguousarray(wt.reshape(4, 128, -1).transpose(1, 0, 2))

    # wkt grouped by ob block: [4, 128, 4, 128], wkt[ob][p, db, c] =
    # Wk.T[db*128+p, ob*128+c]
    wkt = np.ascontiguousarray(
        _sbuf_layout(Wk.T.astype(BF)).reshape(128, 4, 4, 128)
        .transpose(2, 0, 1, 3))
    wvt = _sbuf_layout(Wv.T.astype(BF))
    wot = _sbuf_layout(Wo.T.astype(BF))
    bk32 = np.ascontiguousarray(bk.reshape(4, 128).T).astype(np.float32)
    bvb = bv.reshape(1, D).astype(BF)
    bob = bo.reshape(1, D).astype(BF)
    mixmul = (np.stack(patterns) if patterns
              else np.zeros((1, 128, 128), np.float32)).astype(BF)

    common = dict(wkt=wkt, wvt=wvt, wot=wot, bk32=bk32, bvb=bvb, bob=bob,
                  mixmul=mixmul, ident=np.eye(128, dtype=BF))
    wkt_ob0_flat = wkt[0].reshape(128, 512)
    in_maps = []
    for ci in range(NCORES):
        sl = slice(ci * BL, (ci + 1) * BL)
        qt_h = _sbuf_layout(q[sl].reshape(N, D).T.astype(BF))
        kt_h = _sbuf_layout(k[sl].reshape(N, D).T.astype(BF))
        # [db, {k,q}, t] interleaved to match the boot DMA pipeline
        kq0_h = np.stack([kt_h[:, :, 0:512], qt_h[:, :, 0:512]],
                         axis=2).reshape(128, 4096)
        boot_h = np.ascontiguousarray(
            np.concatenate([wkt_ob0_flat, kq0_h], axis=1))
        in_maps.append(dict(
            boot=boot_h,
            qt=qt_h,
            kt=kt_h,
            vt=_sbuf_layout(v[sl].reshape(N, D).T.astype(BF)),
            **common))

    if os.environ.get("BASS_KERNEL_SIM_TIME"):
        from concourse.timeline_sim import TimelineSim
        LAST_SIM_NS = TimelineSim(nc).simulate()

    res = run_bass_kernel_spmd(nc, in_maps, list(range(NCORES)))
    LAST_EXEC_NS = res.exec_time_ns

    outs = [np.asarray(res.results[ci]["out"], dtype=np.float32)
            .reshape(BL, S, D) for ci in range(NCORES)]
    full = np.concatenate(outs, axis=0)
    if zp:
        full[:, 0, :] = bo
    return full



# revision 27
# speedup vs baseline: 1.0754x; 1.0124x over previous
"""Trainium2 Bass kernel for nn_MultiHeadAttention_47579647705431.

Multi-head attention (8 heads, dim 512, seq 1024, batch 16) with:
  - shared key/query linear (key_query_same=True: q and k both use Wk/bk)
  - causal (or arbitrary block-structured) mask
  - SimpleKT zero_pad: attention row 0 zeroed => out[:, 0, :] = bo

Sharding: data-parallel over batch across 8 NeuronCores (2 batches/core).

Per-core pipeline (all matmuls bf16, fp32 PSUM):
  1. kp/qp = Wk.T-stationary projections -> feature-major [o, n] bf16
  2. vp    = token-major projection [n, o] bf16 with interleaved ones
             columns (stride-65) providing the softmax denominator column
  3. per (b, hp, c): scores^T st [t, s] via K=64 row-packed matmuls;
     exp on ACT (scale 1/8 folded, several j-blocks packed per
     activation); causal/diagonal masking as a 0/1 multiply on DVE
  4. AV with SWAPPED operands: stationary = ex [t, s-block], moving =
     vp [t, 65] -> av PSUM [s, 2, 65] per (b, i, hp).  Cost = 65 free
     columns per (i, j, head) instead of 512 -- half the PE cycles of
     the stationary-vp form, and the denominator lands per-partition so
     normalization is a DVE reciprocal + broadcast multiply (no PE
     broadcast matmuls, no mask identity matmuls).
  5. ct_t token-major [s, 512] per (b, i) -> feature-major ct_i
     [128, 4, 128] via one XBAR dma_start_transpose
  6. out projection per (b, i) (ct_i-stationary) -> [128, 512] f32 -> DRAM

The walrus build here supports ONE sync wait per instruction; Tile emits
more. legalize_waits() hoists extra waits onto same-engine NoOps.
"""

import os
from contextlib import ExitStack

import numpy as np
import ml_dtypes

import concourse.bass as bass
import concourse.mybir as mybir
import concourse.tile as tile
from concourse.bass_utils import run_bass_kernel_spmd

F32 = mybir.dt.float32
BF16 = mybir.dt.bfloat16
BF = ml_dtypes.bfloat16

B, S, D, H, DH = 16, 1024, 512, 8, 64
NCORES = 8
BL = B // NCORES          # batches per core
N = BL * S                # tokens per core
NB = S // 128             # 128-blocks per sequence (8)
HP = H // 2               # head pairs (= o-blocks of 128)
NCH = S // 512            # 512-chunks per sequence (2)

LAST_SIM_NS = None
LAST_EXEC_NS = None

# schedule tuning knobs (sweepable via TimelineSim)
TUNE = dict(
    cap1=4,        # pop cap while it < nseq*capfrac1
    cap2=6,        # pop cap while it < nseq*capfrac2
    capfrac1=0.5,
    capfrac2=0.75,
    stage_delay=16,   # pops a staged final waits before becoming poppable
    tail_stage_delay=8,   # same for tail (PE-transpose) finals
    tail_n=4,      # trailing iterations whose finals use the PE-transpose path
    b1_order=None,
    mask_on_pool=True,  # diag-mask multiplies on the idle GPSIMD engine
)


def legalize_waits(nc):
    """Split multi-wait instructions: keep one wait, hoist the rest onto
    preceding same-engine NoOps (this walrus encodes 1 wait/instruction)."""
    for f in nc.m.functions:
        for blk in f.blocks:
            il = blk.instructions
            i = 0
            while i < len(il):
                inst = il[i]
                si = inst.sync_info
                if si is not None and si.on_wait and len(si.on_wait) > 1:
                    waits = list(si.on_wait)
                    for j, w in enumerate(waits[:-1]):
                        nop = mybir.InstNoOp(
                            name=f"{inst.name}-hw{j}",
                            sync_info=mybir.SyncInfo(on_wait=[w], on_update=[]),
                            bass_nofuse=True,
                            engine=inst.engine,
                        )
                        il.insert(i, nop)
                        i += 1
                    si.on_wait = waits[-1:]
                i += 1


def _classify_mask(mask2d):
    """Classify 128x128 blocks of the [S, S] bool mask (query s, key t).

    Returns (status[j][i], patterns) in scores-transposed coords:
    j = key(t) block, i = query(s) block. status: -1 skip, -2 full,
    >=0 index into patterns (multiplicative bf16 0/1 [t, s] blocks).
    """
    status = [[-1] * NB for _ in range(NB)]
    patterns = []
    pat_idx = {}
    for j in range(NB):
        for i in range(NB):
            blk = mask2d[i * 128:(i + 1) * 128, j * 128:(j + 1) * 128]  # [s, t]
            if blk.all():
                status[j][i] = -2
            elif not blk.any():
                status[j][i] = -1
            else:
                mul = np.where(blk.T, 1.0, 0.0).astype(BF)  # [t, s]
                key = mul.tobytes()
                if key not in pat_idx:
                    pat_idx[key] = len(patterns)
                    patterns.append(mul)
                status[j][i] = pat_idx[key]
    return status, patterns


def _plan_chunks(status, patterns):
    """Per (c, j): suffix run of non-skip query blocks within chunk c.

    Returns plan[c][j] = (w, mixes) where w = run width and mixes =
    [(col_offset_in_region, pattern_id), ...] for mixed blocks. Also
    first_j[c]. Asserts the suffix-nested structure the kernel relies on.
    """
    plan = [[None] * NB for _ in range(NCH)]
    first_j = [None] * NCH
    for c in range(NCH):
        i_lo, i_hi = 4 * c, 4 * c + 4
        prev_w = None
        for j in range(NB):
            sts = [status[j][i] for i in range(i_lo, i_hi)]
            nz = [k for k, s in enumerate(sts) if s != -1]
            if not nz:
                plan[c][j] = (0, [])
                continue
            # must be a contiguous suffix of the chunk
            if nz != list(range(nz[0], 4)):
                raise NotImplementedError("mask block structure not suffix-contiguous")
            w = 128 * len(nz)
            if prev_w is not None and w > prev_w:
                raise NotImplementedError("mask runs not nested over key blocks")
            prev_w = w
            mixes = [((k - nz[0]) * 128, sts[k]) for k in nz if sts[k] >= 0]
            plan[c][j] = (w, mixes)
            if first_j[c] is None:
                first_j[c] = j
    return plan, first_j


def _pack_js(plan, c):
    """Greedy-pack consecutive j runs so one st tile / one exp covers
    several j blocks.  Each pack's total 2w must fit 1024 f32 (4KB)."""
    js = [j for j in range(NB) if plan[c][j][0] > 0]
    packs = []
    cur, cur_sz = [], 0
    for j in js:
        sz = 2 * plan[c][j][0]
        if cur and cur_sz + sz > 1024:
            packs.append(cur)
            cur, cur_sz = [], 0
        cur.append(j)
        cur_sz += sz
    if cur:
        packs.append(cur)
    return packs


def _build(plan, first_j, nmix, has_bk, has_bv, has_bo):
    nc = bass.Bass()
    # boot: startup-critical inputs merged in compute order so the first
    # projection's operands stream in a few pipelined DMAs:
    #   [wkt_ob0 (512) | ktc0 db-major (2048) | qtc0 db-major (2048)]
    boot = nc.dram_tensor("boot", [128, 4608], BF16, kind="ExternalInput")
    qt = nc.dram_tensor("qt", [128, 4, N], BF16, kind="ExternalInput")
    kt = nc.dram_tensor("kt", [128, 4, N], BF16, kind="ExternalInput")
    vt = nc.dram_tensor("vt", [128, 4, N], BF16, kind="ExternalInput")
    wkt = nc.dram_tensor("wkt", [4, 128, 4, 128], BF16, kind="ExternalInput")
    wvt = nc.dram_tensor("wvt", [128, 4, D], BF16, kind="ExternalInput")
    wot = nc.dram_tensor("wot", [128, 4, D], BF16, kind="ExternalInput")
    bk32 = nc.dram_tensor("bk32", [128, 4], F32, kind="ExternalInput")
    bvb = nc.dram_tensor("bvb", [1, D], BF16, kind="ExternalInput")
    bob = nc.dram_tensor("bob", [1, D], BF16, kind="ExternalInput")
    mixmul = nc.dram_tensor("mixmul", [max(nmix, 1), 128, 128], BF16,
                            kind="ExternalInput")
    ident = nc.dram_tensor("ident", [128, 128], BF16, kind="ExternalInput")
    # bf16 output: halves output DMA transfer time; host converts to f32.
    # Rounding cost ~2^-9 relative, well inside the error budget.
    out = nc.dram_tensor("out", [N, D], BF16, kind="ExternalOutput")

    with tile.TileContext(nc) as tc:
        with ExitStack() as ctx:
            sing = ctx.enter_context(tc.tile_pool(name="sing", bufs=1))
            expp = ctx.enter_context(tc.tile_pool(name="expp", bufs=21))
            rcp = ctx.enter_context(tc.tile_pool(name="rcp", bufs=4))
            ctp = ctx.enter_context(tc.tile_pool(name="ctp", bufs=2))
            cti = ctx.enter_context(tc.tile_pool(name="cti", bufs=4))
            outp = ctx.enter_context(tc.tile_pool(name="outp", bufs=4))
            stp = ctx.enter_context(tc.tile_pool(name="stp", bufs=2, space="PSUM"))
            avp = ctx.enter_context(tc.tile_pool(name="avp", bufs=2, space="PSUM"))
            shp = ctx.enter_context(tc.tile_pool(name="shp", bufs=2, space="PSUM"))

            # ---- input loads: critical-path first.  Attention-phase DMAs
            # go on sync; startup loads spread over scalar/sync/gpsimd.
            # kt0 on sync and wkt on scalar so the first projection's two
            # inputs stream through HWDGE back-to-back instead of serially
            kt_c = [None] * 4
            qt_c = [None] * 4
            vt_c = [None] * 4
            # boot tile: wkt_ob0 + interleaved k/q chunk-0 db slices, loaded
            # by 5 pipelined DMAs whose arrival order matches the db-
            # interleaved first projection (each DMA unblocks the next 1-2
            # matmuls, so PE starts at ~4us and never re-stalls)
            # each engine's SEQ is held through the ~625ns HWDGE acquire, so
            # one queue caps issue at ~1.3us/DMA -- slower than the boot
            # chunks' 728ns transfers.  A few early loads go on the scalar
            # (ACT) queue to double the issue rate, but only ones that clear
            # ACT.SEQ before the first exp (~10us); the rest stay on sync so
            # DMA issues never delay exp decode.
            scalar_set = {1, 3, 7, 9, 11}   # boot2, boot4, wkt2, wvt, qtc1
            qrr = [0]

            def ldq():
                i = qrr[0]
                qrr[0] += 1
                return nc.scalar if i in scalar_set else nc.sync

            boot_sb = sing.tile([128, 4608], BF16, name="boot_sb")
            for lo, hi in ((0, 1024), (1024, 2048), (2048, 3072),
                           (3072, 4096), (4096, 4608)):
                ldq().dma_start(out=boot_sb[:, lo:hi], in_=boot[:, lo:hi])
            wkt_ob = []
            wkt_ob.append(boot_sb[:, 0:512].rearrange("p (db c) -> p db c", db=4))
            for ob in range(1, 4):
                t = sing.tile([128, 4, 128], BF16, tag=f"wktob{ob}",
                              name=f"wktob{ob}")
                wkt_ob.append(t)
            kq0 = boot_sb[:, 512:4608].rearrange("p (db two t) -> p two db t",
                                                 two=2, t=512)
            kt_c[0] = kq0[:, 0]
            qt_c[0] = kq0[:, 1]
            # all remaining loads on the single sync queue in strict
            # deadline order: SP issues every ~650ns (never holding waits)
            # and DMA_ENGINES FIFO == emission order, so nothing competes
            # with the startup-critical boot DMAs
            for ob in range(1, 4):
                ldq().dma_start(out=wkt_ob[ob], in_=wkt[ob, :, :, :])
            wvt_sb = sing.tile([128, 4, D], BF16)
            ldq().dma_start(out=wvt_sb, in_=wvt[:, :, :])

            def load_chunk(which, ch):
                t = sing.tile([128, 4, 512], BF16, tag=f"{which}tc{ch}",
                              name=f"{which}tc{ch}")
                src = kt if which == "k" else (qt if which == "q" else vt)
                ldq().dma_start(out=t, in_=src[:, :, ch * 512:ch * 512 + 512])
                (kt_c if which == "k" else (qt_c if which == "q" else vt_c))[ch] = t

            load_chunk("v", 0)
            mix_sb = sing.tile([128, max(nmix, 1), 128], BF16)
            ldq().dma_start(out=mix_sb, in_=mixmul.rearrange("m t s -> t m s"))
            load_chunk("k", 1)
            load_chunk("q", 1)
            wot_sb = sing.tile([128, 4, D], BF16)
            ldq().dma_start(out=wot_sb, in_=wot[:, :, :])
            load_chunk("v", 1)
            load_chunk("k", 3)
            load_chunk("q", 3)
            load_chunk("v", 3)
            load_chunk("k", 2)
            load_chunk("q", 2)
            load_chunk("v", 2)
            ident_sb = sing.tile([128, 128], BF16)
            ldq().dma_start(out=ident_sb, in_=ident[:, :])
            bk_sb = None
            if has_bk:
                bk_sb = sing.tile([128, 4], F32)
                nc.sync.dma_start(out=bk_sb, in_=bk32[:, :])
            bvb_sb = bob_sb = ones_k1 = None
            if has_bv or has_bo:
                ones_k1 = sing.tile([1, 128], BF16)
                nc.vector.memset(ones_k1, 1.0)
            if has_bv:
                bvb_sb = sing.tile([1, D], BF16)
                nc.sync.dma_start(out=bvb_sb, in_=bvb[:, :])
            if has_bo:
                bob_sb = sing.tile([1, D], BF16)
                nc.sync.dma_start(out=bob_sb, in_=bob[:, :])

            kp_sb = sing.tile([128, 4, N], BF16)
            qp_sb = sing.tile([128, 4, N], BF16)
            vp_sb = sing.tile([128, N // 128, 520], BF16)

            # PE p-state warmup: dummy matmuls on memset scratch from ~1.4us
            # so the clock has ramped by the time the first input DMA lands
            # (PE runs at 0.65/1.2GHz for the first 3us of a busy stretch)
            nwarm = TUNE.get("nwarm", 10)
            if nwarm:
                warm_a = sing.tile([128, 128], BF16, name="warm_a")
                warm_b = sing.tile([128, 130], BF16, name="warm_b")
                nc.vector.memset(warm_a, 0.0)
                nc.vector.memset(warm_b, 0.0)
                warm_ps = avp.tile([128, 2, 65], F32, tag="av", name="warm_ps")
                for _ in range(nwarm):
                    nc.tensor.matmul(warm_ps.rearrange("p a b -> p (a b)"),
                                     warm_a, warm_b, start=True, stop=True,
                                     skip_group_check=True)

            # ones columns of vp (denominator trick)
            for nt in range(N // 128):
                nc.vector.memset(
                    vp_sb[:, nt, :].rearrange("p (h u) -> p h u", u=65)[:, :, 64:65],
                    1.0)

            fillers = []
            staged = []    # (pop_stamp, thunk): finals wait out their
                           # transpose latency before becoming poppable
            pop_ctr = [0]
            copy_rr = [0]

            def psum_copy(dst, src):
                # PSUM->SBUF copies on DVE; ACT stays exp-only and GPSIMD
                # cannot access PSUM
                nc.vector.tensor_copy(out=dst, in_=src)

            def kq_half(ob, ch, which):
                csl = slice(ch * 512, ch * 512 + 512)
                src = kt_c[ch] if which == "k" else qt_c[ch]
                dst = kp_sb if which == "k" else qp_sb
                ps = shp.tile([128, 512], F32, tag="sh", name=f"ps{which}")
                for db in range(4):
                    nc.tensor.matmul(
                        ps, wkt_ob[ob][:, db, :],
                        src[:, db, :], start=(db == 0), stop=(db == 3))
                if has_bk:
                    if which == "k":
                        nc.scalar.add(dst[:, ob, csl], ps, bk_sb[:, ob:ob + 1])
                    else:
                        nc.vector.tensor_scalar_add(
                            dst[:, ob, csl], ps, bk_sb[:, ob:ob + 1])
                else:
                    psum_copy(dst[:, ob, csl], ps)

            def kq_group(ob, ch):
                kq_half(ob, ch, "k")
                kq_half(ob, ch, "q")

            def v_proj(nt):
                psV = shp.tile([128, 512], F32, tag="sh")
                for db in range(4):
                    nc.tensor.matmul(
                        psV, vt_c[nt // 4][:, db, (nt % 4) * 128:(nt % 4) * 128 + 128],
                        wvt_sb[:, db, :], start=(db == 0),
                        stop=(db == 3 and not has_bv))
                if has_bv:
                    nc.tensor.matmul(psV, ones_k1, bvb_sb[0:1, :],
                                     start=False, stop=True)
                dst = vp_sb[:, nt, :].rearrange("p (h u) -> p h u", u=65)[:, :, 0:64]
                src = psV.rearrange("p (h u) -> p h u", u=64)
                psum_copy(dst, src)

            def pop_filler(k=1):
                for _ in range(k):
                    pop_ctr[0] += 1
                    while staged and staged[0][0] + TUNE["stage_delay"] <= pop_ctr[0]:
                        fillers.append((("fin",), staged.pop(0)[1]))
                    if fillers:
                        fillers.pop(0)[1]()

            def need_filler(key):
                """Force-emit a specific filler now (dependency deadline)."""
                for fi, (k, thunk) in enumerate(fillers):
                    if k == key:
                        fillers.pop(fi)
                        thunk()
                        return

            ct_t_tiles = {}

            def attention_packs(b, hp, c, ex_t):
                """Thunks: scores+exp+mask, one per pack of j blocks."""
                packs = _pack_js(plan, c)
                thunks = []

                def do_pack(pack):
                    # PSUM bank rule: each matmul output must stay inside one
                    # 2KB bank.  h0 segments stack downward from col 512
                    # (bank 0), h1 segments upward from col 512 (bank 1); the
                    # exp covers the contiguous union [512-tw, 512+tw).
                    tw = sum(plan[c][j][0] for j in pack)
                    st = stp.tile([128, 1024], F32, tag="st")
                    ex = expp.tile([128, 1024], BF16, tag="ex")
                    pre = 0
                    for j in pack:
                        w, _ = plan[c][j]
                        tsl = slice(b * S + j * 128, b * S + j * 128 + 128)
                        ssl = slice(b * S + c * 512 + 512 - w,
                                    b * S + c * 512 + 512)
                        h0s = 512 - pre - w
                        h1s = 512 + pre
                        nc.tensor.matmul(st[:, h0s:h0s + w],
                                         kp_sb[0:64, hp, tsl],
                                         qp_sb[0:64, hp, ssl],
                                         start=True, stop=True)
                        nc.tensor.matmul(st[:, h1s:h1s + w],
                                         kp_sb[64:128, hp, tsl],
                                         qp_sb[64:128, hp, ssl],
                                         start=True, stop=True)
                        ex_t[j] = (ex, h0s, h1s, w)
                        pre += w
                    nc.scalar.activation(
                        ex[:, 512 - tw:512 + tw], st[:, 512 - tw:512 + tw],
                        mybir.ActivationFunctionType.Exp, scale=0.125)
                    # 0/1 mask multiply for mixed blocks (per head half)
                    mask_eng = (nc.gpsimd if TUNE.get("mask_on_pool")
                                else nc.vector)
                    for j in pack:
                        w, mixes = plan[c][j]
                        _, h0s, h1s, _ = ex_t[j]
                        for moff, pid in mixes:
                            for hs in (h0s, h1s):
                                sl = ex[:, hs + moff:hs + moff + 128]
                                mask_eng.tensor_mul(
                                    sl, sl, mix_sb[:, pid, :])

                for pack in packs:
                    import functools
                    thunks.append(functools.partial(do_pack, pack))
                return thunks

            def attention_avs(b, hp, c, ex_t, last_hp, tail=False):
                """Thunks: AV + normalize, one per query block i (swapped
                operands: ex stationary, vp moving)."""
                h0 = 2 * hp
                thunks = []

                def do_av(i):
                    js_i = []
                    for j, (ex, h0s, h1s, w) in ex_t.items():
                        i_start = 4 * c + 4 - w // 128
                        if i >= i_start:
                            o = (i - i_start) * 128
                            js_i.append((j, ex, (h0s + o, h1s + o)))
                    if not js_i:
                        return
                    for j, _, _ in js_i:
                        need_filler(("vp", b * NB + j))
                    av = avp.tile([128, 2, 65], F32, tag="av")
                    nmm = len(js_i) * 2
                    mi = 0
                    for j, ex, hss in js_i:
                        vrow = b * NB + j
                        for h in range(2):
                            # single accumulation group per av tile: PSUM
                            # zeroing is bank-granular (start marks the whole
                            # bank pending-zero; first write to each address
                            # assigns, later writes accumulate)
                            nc.tensor.matmul(
                                av[:, h, :],
                                ex[:, hss[h]:hss[h] + 128],
                                vp_sb[:, vrow,
                                      65 * (h0 + h):65 * (h0 + h) + 65],
                                start=(mi == 0), stop=(mi == nmm - 1),
                                skip_group_check=True)
                            mi += 1
                    # normalize: per-partition reciprocal + broadcast mul
                    key = (b, i)
                    if key not in ct_t_tiles:
                        ct_t_tiles[key] = ctp.tile([128, 512], BF16,
                                                   name=f"ctt{b}_{i}",
                                                   tag=f"ctt{b}_{i % 4}")
                    ct_t = ct_t_tiles[key]
                    rc = rcp.tile([128, 2], BF16, tag="rc")
                    with nc.allow_low_precision(reason="softmax recip bf16"):
                        nc.vector.reciprocal(out=rc, in_=av[:, :, 64])
                    dst = ct_t[:, 128 * hp:128 * hp + 128].rearrange(
                        "p (h w) -> p h w", h=2)
                    nc.vector.tensor_mul(
                        dst, av[:, :, 0:64],
                        rc[:, :, None].broadcast_to([128, 2, 64]))
                    if last_hp:
                        finish_block(b, i, ct_t, tail=tail)

                import functools
                for i in range(4 * c, 4 * c + 4):
                    thunks.append(functools.partial(do_av, i))
                return thunks

            def finish_block(b, i, ct_t, tail=False):
                """transpose ct_t -> feature-major, then queue out-proj.

                tail=True blocks (the last c-group) use PE transposes + an
                ACT copy instead of a DMA transpose: ~1us latency instead of
                ~3.3us of DMA issue+transfer+sem-prop on the critical tail.
                """
                ct_i = cti.tile([128, 4, 128], BF16, tag="cti")
                if tail:
                    stq = stp.tile([128, 1024], F32, tag="st",
                                   name=f"pst{b}_{i}")
                    psT = stq.bitcast(BF16)
                    for q4 in range(4):
                        nc.tensor.transpose(
                            psT[:, q4 * 128:q4 * 128 + 128],
                            ct_t[:, q4 * 128:q4 * 128 + 128], ident_sb)
                    # DVE, not ACT: ACT is still draining exps here (in-order
                    # engine => a copy there queues behind them), and the
                    # bf16->bf16 copy gets DVE's 2x mode
                    nc.vector.tensor_copy(
                        out=ct_i.rearrange("p a c -> p (a c)"),
                        in_=psT[:, 0:512])
                else:
                    nc.sync.dma_start_transpose(ct_i[:, :, :], ct_t[:, :])
                del ct_t_tiles[(b, i)]

                def final(b=b, i=i, ct_i=ct_i, tail=tail):
                    psO = shp.tile([128, 512], F32, tag="sh")
                    for db in range(4):
                        nc.tensor.matmul(
                            psO, ct_i[:, db, :], wot_sb[:, db, :],
                            start=(db == 0), stop=(db == 3 and not has_bo))
                    if has_bo:
                        nc.tensor.matmul(psO, ones_k1, bob_sb[0:1, :],
                                         start=False, stop=True)
                    ot = outp.tile([128, 512], BF16)
                    row = b * S + i * 128
                    if tail:
                        # alternate copy engine and DMA queue so the last
                        # finals' copies and issues don't serialize
                        if i % 2 == 0:
                            nc.scalar.copy(ot, psO)
                        else:
                            psum_copy(ot, psO)
                        eng = nc.sync if i % 2 == 0 else nc.scalar
                        eng.dma_start(out=out[row:row + 128, :], in_=ot)
                    else:
                        psum_copy(ot, psO)
                        nc.sync.dma_start(out=out[row:row + 128, :], in_=ot)

                staged.append((pop_ctr[0] if not tail else
                               pop_ctr[0] - TUNE["stage_delay"]
                               + TUNE["tail_stage_delay"], final))

            # ---- emission schedule (software-pipelined) ----
            # upfront: only what iteration 0's scores need; the rest of the
            # projections become ordered fillers consumed during attention.
            import functools
            # upfront: iteration 0 (b0, hp0, c0) needs only ob0/ch0.
            # k/q matmuls interleaved at db granularity to match the boot
            # DMA arrival order (psk/psq accumulate in separate PSUM banks)
            psk = shp.tile([128, 512], F32, tag="sh", name="psk")
            psq = shp.tile([128, 512], F32, tag="sh", name="psq")
            for db in range(4):
                nc.tensor.matmul(psk, wkt_ob[0][:, db, :], kt_c[0][:, db, :],
                                 start=(db == 0), stop=(db == 3),
                                 skip_group_check=True)
                nc.tensor.matmul(psq, wkt_ob[0][:, db, :], qt_c[0][:, db, :],
                                 start=(db == 0), stop=(db == 3),
                                 skip_group_check=True)
            if has_bk:
                nc.scalar.add(kp_sb[:, 0, 0:512], psk, bk_sb[:, 0:1])
                nc.vector.tensor_scalar_add(qp_sb[:, 0, 0:512], psq,
                                            bk_sb[:, 0:1])
            else:
                psum_copy(kp_sb[:, 0, 0:512], psk)
                psum_copy(qp_sb[:, 0, 0:512], psq)
            # deadline-ordered fillers matching the b0c0,b0c1,b1c1,b1c0 seq;
            # keys let consumers force-emit their prerequisites in time
            def FK(ob, ch):
                fillers.append((("kq", ob, ch), functools.partial(kq_group, ob, ch)))

            def FV(nt):
                fillers.append((("vp", nt), functools.partial(v_proj, nt)))

            FK(1, 0)
            FK(2, 0)
            FV(0)
            FV(1)
            FK(3, 0)
            FV(2)
            FV(3)
            FK(0, 1)
            FK(1, 1)
            FV(4)
            FV(5)
            FK(2, 1)
            FV(6)
            FV(7)
            FK(3, 1)
            for ob in range(4):
                FK(ob, 3)
                FK(ob, 2)
            for nt in range(12, 16):
                FV(nt)
            for nt in range(8, 12):
                FV(nt)

            seq = []
            border = {0: (0, 1), 1: (1, 0)}
            for b in range(BL):
                if NCH == 2 and b % 2 == 1 and TUNE.get("b1_order"):
                    # interleave the light c0 iterations among the heavy c1
                    # ones so the tail's exp backlog on ACT is smaller
                    for hp, c in TUNE["b1_order"]:
                        seq.append((b, hp, c))
                else:
                    for c in border[b % 2] if NCH == 2 else range(NCH):
                        for hp in range(4):
                            seq.append((b, hp, c))

            # iteration k's AV phase is interleaved with iteration k+2's
            # scores/exp packs (2-deep software pipeline): by the time an AV
            # runs, its exps retired during iteration k+1, so PE never waits
            # on ACT across iteration boundaries
            pend = []      # queue of AV thunk lists
            nseq = len(seq)
            for it, (b, hp, c) in enumerate(seq):
                # scores need this iteration's kq projections emitted first
                for ch in ([2 * b] if c == 0 else [2 * b, 2 * b + 1]):
                    need_filler(("kq", hp, ch))
                ex_t = {}
                packs = attention_packs(b, hp, c, ex_t)
                avs = attention_avs(b, hp, c, ex_t, last_hp=(hp == 3),
                                    tail=(it >= nseq - TUNE["tail_n"]))
                ready = pend.pop(0) if (len(pend) >= 2 or
                                        (pend and it == nseq - 1)) else []
                pops = 0
                # cap pops in the first (b0) half so fillers remain for the
                # ACT-bound b1c1 phase; first iterations also delay pops so
                # a not-yet-loaded filler can't head-of-line block PE
                cap = (TUNE["cap1"] if it < nseq * TUNE["capfrac1"] else
                       (TUNE["cap2"] if it < nseq * TUNE["capfrac2"] else 99))
                for x in range(max(len(packs), len(ready))):
                    do_pop = (it >= 2 or x >= 2) and pops < cap
                    if x < len(packs):
                        packs[x]()
                        if do_pop:
                            pop_filler(1)
                            pops += 1
                    if x < len(ready):
                        ready[x]()
                        if do_pop and pops < cap:
                            pop_filler(1)
                            pops += 1
                pend.append(avs)
            for avs in pend:
                for av in avs:
                    av()
                    pop_filler(1)
            while fillers or staged:
                pop_filler(1)

    return nc


_prog_cache = {}


def kernel(q, k, v, mask, zero_pad, Wk, bk, Wv, bv, Wo, bo):
    global LAST_SIM_NS, LAST_EXEC_NS
    q = np.asarray(q, dtype=np.float32)
    k = np.asarray(k, dtype=np.float32)
    v = np.asarray(v, dtype=np.float32)
    Wk = np.asarray(Wk, dtype=np.float32)
    Wv = np.asarray(Wv, dtype=np.float32)
    Wo = np.asarray(Wo, dtype=np.float32)
    bk = np.asarray(bk, dtype=np.float32).reshape(D)
    bv = np.asarray(bv, dtype=np.float32).reshape(D)
    bo = np.asarray(bo, dtype=np.float32).reshape(D)
    mask2d = np.asarray(mask).reshape(S, S).astype(bool)
    zp = int(np.asarray(zero_pad))

    status, patterns = _classify_mask(mask2d)
    plan, first_j = _plan_chunks(status, patterns)
    nmix = len(patterns)
    has_bk = bool(np.any(bk))
    has_bv = bool(np.any(bv))
    has_bo = bool(np.any(bo))

    sig = (tuple(tuple(r) for r in status), nmix, has_bk, has_bv, has_bo)
    if sig not in _prog_cache:
        nc_new = _build(plan, first_j, nmix, has_bk, has_bv, has_bo)
        legalize_waits(nc_new)   # hardware-only pass (sim runs pre-legalized)
        _prog_cache[sig] = nc_new
    nc = _prog_cache[sig]

    def _sbuf_layout(wt):
        # [D, X] -> [128, 4, X]: row d = a*128+p  ->  [p, a, :]
        return np.ascontiguousarray(wt.reshape(4, 128, -1).transpose(1, 0, 2))

    # wkt grouped by ob block: [4, 128, 4, 128], wkt[ob][p, db, c] =
    # Wk.T[db*128+p, ob*128+c]
    wkt = np.ascontiguousarray(
        _sbuf_layout(Wk.T.astype(BF)).reshape(128, 4, 4, 128)
        .transpose(2, 0, 1, 3))
    wvt = _sbuf_layout(Wv.T.astype(BF))
    wot = _sbuf_layout(Wo.T.astype(BF))
    bk32 = np.ascontiguousarray(bk.reshape(4, 128).T).astype(np.float32)
    bvb = bv.reshape(1, D).astype(BF)
    bob = bo.reshape(1, D).astype(BF)
    mixmul = (np.stack(patterns) if patterns
              else np.zeros((1, 128, 128), np.float32)).astype(BF)

    common = dict(wkt=wkt, wvt=wvt, wot=wot, bk32=bk32, bvb=bvb, bob=bob,
                  mixmul=mixmul, ident=np.eye(128, dtype=BF))
    wkt_ob0_flat = wkt[0].reshape(128, 512)
    in_maps = []
    for ci in range(NCORES):
        sl = slice(ci * BL, (ci + 1) * BL)
        qt_h = _sbuf_layout(q[sl].reshape(N, D).T.astype(BF))
        kt_h = _sbuf_layout(k[sl].reshape(N, D).T.astype(BF))
        # [db, {k,q}, t] interleaved to match the boot DMA pipeline
        kq0_h = np.stack([kt_h[:, :, 0:512], qt_h[:, :, 0:512]],
                         axis=2).reshape(128, 4096)
        boot_h = np.ascontiguousarray(
            np.concatenate([wkt_ob0_flat, kq0_h], axis=1))
        in_maps.append(dict(
            boot=boot_h,
            qt=qt_h,
            kt=kt_h,
            vt=_sbuf_layout(v[sl].reshape(N, D).T.astype(BF)),
            **common))

    if os.environ.get("BASS_KERNEL_SIM_TIME"):
        from concourse.timeline_sim import TimelineSim
        LAST_SIM_NS = TimelineSim(nc).simulate()

    res = run_bass_kernel_spmd(nc, in_maps, list(range(NCORES)))
    LAST_EXEC_NS = res.exec_time_ns

    outs = [np.asarray(res.results[ci]["out"], dtype=np.float32)
            .reshape(BL, S, D) for ci in range(NCORES)]
    full = np.concatenate(outs, axis=0)
    if zp:
        full[:, 0, :] = bo
    return full



# revision 40
# speedup vs baseline: 1.0820x; 1.0062x over previous
"""Trainium2 Bass kernel for nn_MultiHeadAttention_47579647705431.

Multi-head attention (8 heads, dim 512, seq 1024, batch 16) with:
  - shared key/query linear (key_query_same=True: q and k both use Wk/bk)
  - causal (or arbitrary block-structured) mask
  - SimpleKT zero_pad: attention row 0 zeroed => out[:, 0, :] = bo

Sharding: data-parallel over batch across 8 NeuronCores (2 batches/core).

Per-core pipeline (all matmuls bf16, fp32 PSUM):
  1. kp/qp = Wk.T-stationary projections -> feature-major [o, n] bf16
  2. vp    = token-major projection [n, o] bf16 with interleaved ones
             columns (stride-65) providing the softmax denominator column
  3. per (b, hp, c): scores^T st [t, s] via K=64 row-packed matmuls;
     exp on ACT (scale 1/8 folded, several j-blocks packed per
     activation); causal/diagonal masking as a 0/1 multiply on DVE
  4. AV with SWAPPED operands: stationary = ex [t, s-block], moving =
     vp [t, 65] -> av PSUM [s, 2, 65] per (b, i, hp).  Cost = 65 free
     columns per (i, j, head) instead of 512 -- half the PE cycles of
     the stationary-vp form, and the denominator lands per-partition so
     normalization is a DVE reciprocal + broadcast multiply (no PE
     broadcast matmuls, no mask identity matmuls).
  5. ct_t token-major [s, 512] per (b, i) -> feature-major ct_i
     [128, 4, 128] via one XBAR dma_start_transpose
  6. out projection per (b, i) (ct_i-stationary) -> [128, 512] f32 -> DRAM

The walrus build here supports ONE sync wait per instruction; Tile emits
more. legalize_waits() hoists extra waits onto same-engine NoOps.
"""

import os
from contextlib import ExitStack

import numpy as np
import ml_dtypes

import concourse.bass as bass
import concourse.mybir as mybir
import concourse.tile as tile
from concourse.bass_utils import run_bass_kernel_spmd

F32 = mybir.dt.float32
BF16 = mybir.dt.bfloat16
BF = ml_dtypes.bfloat16

B, S, D, H, DH = 16, 1024, 512, 8, 64
NCORES = 8
BL = B // NCORES          # batches per core
N = BL * S                # tokens per core
NB = S // 128             # 128-blocks per sequence (8)
HP = H // 2               # head pairs (= o-blocks of 128)
NCH = S // 512            # 512-chunks per sequence (2)

LAST_SIM_NS = None
LAST_EXEC_NS = None

# schedule tuning knobs (sweepable via TimelineSim)
TUNE = dict(
    cap1=4,        # pop cap while it < nseq*capfrac1
    cap2=5,        # pop cap while it < nseq*capfrac2
    capfrac1=0.55,
    capfrac2=0.85,
    stage_delay=17,   # pops a staged final waits before becoming poppable
    tail_stage_delay=4,   # same for tail (PE-transpose) finals
    tail_n=4,      # trailing iterations whose finals use the PE-transpose path
    b1_order=None,
    mask_on_pool=True,  # diag-mask multiplies on the idle GPSIMD engine
)


def legalize_waits(nc):
    """Split multi-wait instructions: keep one wait, hoist the rest onto
    preceding same-engine NoOps (this walrus encodes 1 wait/instruction)."""
    for f in nc.m.functions:
        for blk in f.blocks:
            il = blk.instructions
            i = 0
            while i < len(il):
                inst = il[i]
                si = inst.sync_info
                if si is not None and si.on_wait and len(si.on_wait) > 1:
                    waits = list(si.on_wait)
                    for j, w in enumerate(waits[:-1]):
                        nop = mybir.InstNoOp(
                            name=f"{inst.name}-hw{j}",
                            sync_info=mybir.SyncInfo(on_wait=[w], on_update=[]),
                            bass_nofuse=True,
                            engine=inst.engine,
                        )
                        il.insert(i, nop)
                        i += 1
                    si.on_wait = waits[-1:]
                i += 1


def _classify_mask(mask2d):
    """Classify 128x128 blocks of the [S, S] bool mask (query s, key t).

    Returns (status[j][i], patterns) in scores-transposed coords:
    j = key(t) block, i = query(s) block. status: -1 skip, -2 full,
    >=0 index into patterns (multiplicative bf16 0/1 [t, s] blocks).
    """
    status = [[-1] * NB for _ in range(NB)]
    patterns = []
    pat_idx = {}
    for j in range(NB):
        for i in range(NB):
            blk = mask2d[i * 128:(i + 1) * 128, j * 128:(j + 1) * 128]  # [s, t]
            if blk.all():
                status[j][i] = -2
            elif not blk.any():
                status[j][i] = -1
            else:
                mul = np.where(blk.T, 1.0, 0.0).astype(BF)  # [t, s]
                key = mul.tobytes()
                if key not in pat_idx:
                    pat_idx[key] = len(patterns)
                    patterns.append(mul)
                status[j][i] = pat_idx[key]
    return status, patterns


def _plan_chunks(status, patterns):
    """Per (c, j): suffix run of non-skip query blocks within chunk c.

    Returns plan[c][j] = (w, mixes) where w = run width and mixes =
    [(col_offset_in_region, pattern_id), ...] for mixed blocks. Also
    first_j[c]. Asserts the suffix-nested structure the kernel relies on.
    """
    plan = [[None] * NB for _ in range(NCH)]
    first_j = [None] * NCH
    for c in range(NCH):
        i_lo, i_hi = 4 * c, 4 * c + 4
        prev_w = None
        for j in range(NB):
            sts = [status[j][i] for i in range(i_lo, i_hi)]
            nz = [k for k, s in enumerate(sts) if s != -1]
            if not nz:
                plan[c][j] = (0, [])
                continue
            # must be a contiguous suffix of the chunk
            if nz != list(range(nz[0], 4)):
                raise NotImplementedError("mask block structure not suffix-contiguous")
            w = 128 * len(nz)
            if prev_w is not None and w > prev_w:
                raise NotImplementedError("mask runs not nested over key blocks")
            prev_w = w
            mixes = [((k - nz[0]) * 128, sts[k]) for k in nz if sts[k] >= 0]
            plan[c][j] = (w, mixes)
            if first_j[c] is None:
                first_j[c] = j
    return plan, first_j


def _pack_js(plan, c):
    """Greedy-pack consecutive j runs so one st tile / one exp covers
    several j blocks.  Each pack's total 2w must fit 1024 f32 (4KB)."""
    js = [j for j in range(NB) if plan[c][j][0] > 0]
    packs = []
    cur, cur_sz = [], 0
    for j in js:
        sz = 2 * plan[c][j][0]
        if cur and cur_sz + sz > 1024:
            packs.append(cur)
            cur, cur_sz = [], 0
        cur.append(j)
        cur_sz += sz
    if cur:
        packs.append(cur)
    return packs


def _build(plan, first_j, nmix, has_bk, has_bv, has_bo):
    nc = bass.Bass()
    # boot: startup-critical inputs merged in compute order so the first
    # projection's operands stream in a few pipelined DMAs:
    #   [wkt_ob0 (512) | ktc0 db-major (2048) | qtc0 db-major (2048)]
    boot = nc.dram_tensor("boot", [128, 4608], BF16, kind="ExternalInput")
    qt = nc.dram_tensor("qt", [128, 4, N], BF16, kind="ExternalInput")
    kt = nc.dram_tensor("kt", [128, 4, N], BF16, kind="ExternalInput")
    vt = nc.dram_tensor("vt", [128, 4, N], BF16, kind="ExternalInput")
    wkt = nc.dram_tensor("wkt", [4, 128, 4, 128], BF16, kind="ExternalInput")
    wvt = nc.dram_tensor("wvt", [128, 4, D], BF16, kind="ExternalInput")
    wot = nc.dram_tensor("wot", [128, 4, D], BF16, kind="ExternalInput")
    bk32 = nc.dram_tensor("bk32", [128, 4], F32, kind="ExternalInput")
    bvb = nc.dram_tensor("bvb", [1, D], BF16, kind="ExternalInput")
    bob = nc.dram_tensor("bob", [1, D], BF16, kind="ExternalInput")
    mixmul = nc.dram_tensor("mixmul", [max(nmix, 1), 128, 128], BF16,
                            kind="ExternalInput")
    ident = nc.dram_tensor("ident", [128, 128], BF16, kind="ExternalInput")
    # bf16 output: halves output DMA transfer time; host converts to f32.
    # Rounding cost ~2^-9 relative, well inside the error budget.
    out = nc.dram_tensor("out", [N, D], BF16, kind="ExternalOutput")

    with tile.TileContext(nc) as tc:
        with ExitStack() as ctx:
            sing = ctx.enter_context(tc.tile_pool(name="sing", bufs=1))
            expp = ctx.enter_context(tc.tile_pool(name="expp", bufs=21))
            rcp = ctx.enter_context(tc.tile_pool(name="rcp", bufs=4))
            ctp = ctx.enter_context(tc.tile_pool(name="ctp", bufs=2))
            cti = ctx.enter_context(tc.tile_pool(name="cti", bufs=4))
            outp = ctx.enter_context(tc.tile_pool(name="outp", bufs=4))
            stp = ctx.enter_context(tc.tile_pool(name="stp", bufs=2, space="PSUM"))
            avp = ctx.enter_context(tc.tile_pool(name="avp", bufs=2, space="PSUM"))
            shp = ctx.enter_context(tc.tile_pool(name="shp", bufs=2, space="PSUM"))

            # ---- input loads: critical-path first.  Attention-phase DMAs
            # go on sync; startup loads spread over scalar/sync/gpsimd.
            # kt0 on sync and wkt on scalar so the first projection's two
            # inputs stream through HWDGE back-to-back instead of serially
            kt_c = [None] * 4
            qt_c = [None] * 4
            vt_c = [None] * 4
            # boot tile: wkt_ob0 + interleaved k/q chunk-0 db slices, loaded
            # by 5 pipelined DMAs whose arrival order matches the db-
            # interleaved first projection (each DMA unblocks the next 1-2
            # matmuls, so PE starts at ~4us and never re-stalls)
            # each engine's SEQ is held through the ~625ns HWDGE acquire, so
            # one queue caps issue at ~1.3us/DMA -- slower than the boot
            # chunks' 728ns transfers.  A few early loads go on the scalar
            # (ACT) queue to double the issue rate, but only ones that clear
            # ACT.SEQ before the first exp (~10us); the rest stay on sync so
            # DMA issues never delay exp decode.
            scalar_set = {1, 3, 7, 9, 11}   # boot2, boot4, wkt2, wvt, qtc1
            qrr = [0]

            def ldq():
                i = qrr[0]
                qrr[0] += 1
                return nc.scalar if i in scalar_set else nc.sync

            boot_sb = sing.tile([128, 4608], BF16, name="boot_sb")
            for lo, hi in ((0, 1024), (1024, 2048), (2048, 3072),
                           (3072, 4096), (4096, 4608)):
                ldq().dma_start(out=boot_sb[:, lo:hi], in_=boot[:, lo:hi])
            wkt_ob = []
            wkt_ob.append(boot_sb[:, 0:512].rearrange("p (db c) -> p db c", db=4))
            for ob in range(1, 4):
                t = sing.tile([128, 4, 128], BF16, tag=f"wktob{ob}",
                              name=f"wktob{ob}")
                wkt_ob.append(t)
            kq0 = boot_sb[:, 512:4608].rearrange("p (db two t) -> p two db t",
                                                 two=2, t=512)
            kt_c[0] = kq0[:, 0]
            qt_c[0] = kq0[:, 1]
            # all remaining loads on the single sync queue in strict
            # deadline order: SP issues every ~650ns (never holding waits)
            # and DMA_ENGINES FIFO == emission order, so nothing competes
            # with the startup-critical boot DMAs
            for ob in range(1, 4):
                ldq().dma_start(out=wkt_ob[ob], in_=wkt[ob, :, :, :])
            wvt_sb = sing.tile([128, 4, D], BF16)
            ldq().dma_start(out=wvt_sb, in_=wvt[:, :, :])

            def load_chunk(which, ch):
                t = sing.tile([128, 4, 512], BF16, tag=f"{which}tc{ch}",
                              name=f"{which}tc{ch}")
                src = kt if which == "k" else (qt if which == "q" else vt)
                ldq().dma_start(out=t, in_=src[:, :, ch * 512:ch * 512 + 512])
                (kt_c if which == "k" else (qt_c if which == "q" else vt_c))[ch] = t

            load_chunk("v", 0)
            mix_sb = sing.tile([128, max(nmix, 1), 128], BF16)
            ldq().dma_start(out=mix_sb, in_=mixmul.rearrange("m t s -> t m s"))
            load_chunk("k", 1)
            load_chunk("q", 1)
            wot_sb = sing.tile([128, 4, D], BF16)
            ldq().dma_start(out=wot_sb, in_=wot[:, :, :])
            load_chunk("v", 1)
            load_chunk("k", 3)
            load_chunk("q", 3)
            load_chunk("v", 3)
            load_chunk("k", 2)
            load_chunk("q", 2)
            load_chunk("v", 2)
            ident_sb = sing.tile([128, 128], BF16)
            ldq().dma_start(out=ident_sb, in_=ident[:, :])
            bk_sb = None
            if has_bk:
                bk_sb = sing.tile([128, 4], F32)
                nc.sync.dma_start(out=bk_sb, in_=bk32[:, :])
            bvb_sb = bob_sb = ones_k1 = None
            if has_bv or has_bo:
                ones_k1 = sing.tile([1, 128], BF16)
                nc.vector.memset(ones_k1, 1.0)
            if has_bv:
                bvb_sb = sing.tile([1, D], BF16)
                nc.sync.dma_start(out=bvb_sb, in_=bvb[:, :])
            if has_bo:
                bob_sb = sing.tile([1, D], BF16)
                nc.sync.dma_start(out=bob_sb, in_=bob[:, :])

            kp_sb = sing.tile([128, 4, N], BF16)
            qp_sb = sing.tile([128, 4, N], BF16)
            vp_sb = sing.tile([128, N // 128, 520], BF16)

            # PE p-state warmup: dummy matmuls on memset scratch from ~1.4us
            # so the clock has ramped by the time the first input DMA lands
            # (PE runs at 0.65/1.2GHz for the first 3us of a busy stretch)
            nwarm = TUNE.get("nwarm", 10)
            if nwarm:
                warm_a = sing.tile([128, 128], BF16, name="warm_a")
                warm_b = sing.tile([128, 130], BF16, name="warm_b")
                nc.vector.memset(warm_a, 0.0)
                nc.vector.memset(warm_b, 0.0)
                warm_ps = avp.tile([128, 2, 65], F32, tag="av", name="warm_ps")
                for _ in range(nwarm):
                    nc.tensor.matmul(warm_ps.rearrange("p a b -> p (a b)"),
                                     warm_a, warm_b, start=True, stop=True,
                                     skip_group_check=True)

            # ones columns of vp (denominator trick)
            for nt in range(N // 128):
                nc.vector.memset(
                    vp_sb[:, nt, :].rearrange("p (h u) -> p h u", u=65)[:, :, 64:65],
                    1.0)

            fillers = []
            staged = []    # (pop_stamp, thunk): finals wait out their
                           # transpose latency before becoming poppable
            pop_ctr = [0]
            copy_rr = [0]

            def psum_copy(dst, src):
                # PSUM->SBUF copies on DVE; ACT stays exp-only and GPSIMD
                # cannot access PSUM
                nc.vector.tensor_copy(out=dst, in_=src)

            def kq_half(ob, ch, which):
                csl = slice(ch * 512, ch * 512 + 512)
                src = kt_c[ch] if which == "k" else qt_c[ch]
                dst = kp_sb if which == "k" else qp_sb
                ps = shp.tile([128, 512], F32, tag="sh", name=f"ps{which}")
                for db in range(4):
                    nc.tensor.matmul(
                        ps, wkt_ob[ob][:, db, :],
                        src[:, db, :], start=(db == 0), stop=(db == 3))
                if has_bk:
                    if which == "k":
                        nc.scalar.add(dst[:, ob, csl], ps, bk_sb[:, ob:ob + 1])
                    else:
                        nc.vector.tensor_scalar_add(
                            dst[:, ob, csl], ps, bk_sb[:, ob:ob + 1])
                else:
                    psum_copy(dst[:, ob, csl], ps)

            def kq_group(ob, ch):
                kq_half(ob, ch, "k")
                kq_half(ob, ch, "q")

            def v_proj(nt):
                psV = shp.tile([128, 512], F32, tag="sh")
                for db in range(4):
                    nc.tensor.matmul(
                        psV, vt_c[nt // 4][:, db, (nt % 4) * 128:(nt % 4) * 128 + 128],
                        wvt_sb[:, db, :], start=(db == 0),
                        stop=(db == 3 and not has_bv))
                if has_bv:
                    nc.tensor.matmul(psV, ones_k1, bvb_sb[0:1, :],
                                     start=False, stop=True)
                dst = vp_sb[:, nt, :].rearrange("p (h u) -> p h u", u=65)[:, :, 0:64]
                src = psV.rearrange("p (h u) -> p h u", u=64)
                psum_copy(dst, src)

            def pop_filler(k=1):
                for _ in range(k):
                    pop_ctr[0] += 1
                    while staged and staged[0][0] + TUNE["stage_delay"] <= pop_ctr[0]:
                        fillers.append((("fin",), staged.pop(0)[1]))
                    if fillers:
                        fillers.pop(0)[1]()

            def need_filler(key):
                """Force-emit a specific filler now (dependency deadline)."""
                for fi, (k, thunk) in enumerate(fillers):
                    if k == key:
                        fillers.pop(fi)
                        thunk()
                        return

            ct_t_tiles = {}

            def attention_packs(b, hp, c, ex_t):
                """Thunks: scores+exp+mask, one per pack of j blocks."""
                packs = _pack_js(plan, c)
                thunks = []

                def do_pack(pack):
                    # PSUM bank rule: each matmul output must stay inside one
                    # 2KB bank.  h0 segments stack downward from col 512
                    # (bank 0), h1 segments upward from col 512 (bank 1); the
                    # exp covers the contiguous union [512-tw, 512+tw).
                    tw = sum(plan[c][j][0] for j in pack)
                    st = stp.tile([128, 1024], F32, tag="st")
                    ex = expp.tile([128, 1024], BF16, tag="ex")
                    pre = 0
                    for j in pack:
                        w, _ = plan[c][j]
                        tsl = slice(b * S + j * 128, b * S + j * 128 + 128)
                        ssl = slice(b * S + c * 512 + 512 - w,
                                    b * S + c * 512 + 512)
                        h0s = 512 - pre - w
                        h1s = 512 + pre
                        nc.tensor.matmul(st[:, h0s:h0s + w],
                                         kp_sb[0:64, hp, tsl],
                                         qp_sb[0:64, hp, ssl],
                                         start=True, stop=True,
                                         skip_group_check=True)
                        nc.tensor.matmul(st[:, h1s:h1s + w],
                                         kp_sb[64:128, hp, tsl],
                                         qp_sb[64:128, hp, ssl],
                                         start=True, stop=True,
                                         skip_group_check=True)
                        ex_t[j] = (ex, h0s, h1s, w)
                        pre += w
                    nc.scalar.activation(
                        ex[:, 512 - tw:512 + tw], st[:, 512 - tw:512 + tw],
                        mybir.ActivationFunctionType.Exp, scale=0.125)
                    # 0/1 mask multiply for mixed blocks (per head half)
                    mask_eng = (nc.gpsimd if TUNE.get("mask_on_pool")
                                else nc.vector)
                    for j in pack:
                        w, mixes = plan[c][j]
                        _, h0s, h1s, _ = ex_t[j]
                        for moff, pid in mixes:
                            for hs in (h0s, h1s):
                                sl = ex[:, hs + moff:hs + moff + 128]
                                mask_eng.tensor_mul(
                                    sl, sl, mix_sb[:, pid, :])

                for pack in packs:
                    import functools
                    thunks.append(functools.partial(do_pack, pack))
                return thunks

            def attention_avs(b, hp, c, ex_t, last_hp, tail=False):
                """Thunks: AV + normalize, one per query block i (swapped
                operands: ex stationary, vp moving)."""
                h0 = 2 * hp
                thunks = []

                def do_av(i):
                    js_i = []
                    for j, (ex, h0s, h1s, w) in ex_t.items():
                        i_start = 4 * c + 4 - w // 128
                        if i >= i_start:
                            o = (i - i_start) * 128
                            js_i.append((j, ex, (h0s + o, h1s + o)))
                    if not js_i:
                        return
                    for j, _, _ in js_i:
                        need_filler(("vp", b * NB + j))
                    av = avp.tile([128, 2, 65], F32, tag="av")
                    nmm = len(js_i) * 2
                    mi = 0
                    for j, ex, hss in js_i:
                        vrow = b * NB + j
                        for h in range(2):
                            # single accumulation group per av tile: PSUM
                            # zeroing is bank-granular (start marks the whole
                            # bank pending-zero; first write to each address
                            # assigns, later writes accumulate)
                            nc.tensor.matmul(
                                av[:, h, :],
                                ex[:, hss[h]:hss[h] + 128],
                                vp_sb[:, vrow,
                                      65 * (h0 + h):65 * (h0 + h) + 65],
                                start=(mi == 0), stop=(mi == nmm - 1),
                                skip_group_check=True)
                            mi += 1
                    # normalize: per-partition reciprocal + broadcast mul
                    key = (b, i)
                    if key not in ct_t_tiles:
                        ct_t_tiles[key] = ctp.tile([128, 512], BF16,
                                                   name=f"ctt{b}_{i}",
                                                   tag=f"ctt{b}_{i % 4}")
                    ct_t = ct_t_tiles[key]
                    rc = rcp.tile([128, 2], BF16, tag="rc")
                    with nc.allow_low_precision(reason="softmax recip bf16"):
                        nc.vector.reciprocal(out=rc, in_=av[:, :, 64])
                    dst = ct_t[:, 128 * hp:128 * hp + 128].rearrange(
                        "p (h w) -> p h w", h=2)
                    nc.vector.tensor_mul(
                        dst, av[:, :, 0:64],
                        rc[:, :, None].broadcast_to([128, 2, 64]))
                    if last_hp:
                        finish_block(b, i, ct_t, tail=tail)

                import functools
                for i in range(4 * c, 4 * c + 4):
                    thunks.append(functools.partial(do_av, i))
                return thunks

            def finish_block(b, i, ct_t, tail=False):
                """transpose ct_t -> feature-major, then queue out-proj.

                tail=True blocks (the last c-group) use PE transposes + an
                ACT copy instead of a DMA transpose: ~1us latency instead of
                ~3.3us of DMA issue+transfer+sem-prop on the critical tail.
                """
                ct_i = cti.tile([128, 4, 128], BF16, tag="cti")
                if tail:
                    stq = stp.tile([128, 1024], F32, tag="st",
                                   name=f"pst{b}_{i}")
                    psT = stq.bitcast(BF16)
                    for q4 in range(4):
                        nc.tensor.transpose(
                            psT[:, q4 * 128:q4 * 128 + 128],
                            ct_t[:, q4 * 128:q4 * 128 + 128], ident_sb)
                    # DVE, not ACT: ACT is still draining exps here (in-order
                    # engine => a copy there queues behind them), and the
                    # bf16->bf16 copy gets DVE's 2x mode
                    nc.vector.tensor_copy(
                        out=ct_i.rearrange("p a c -> p (a c)"),
                        in_=psT[:, 0:512])
                else:
                    nc.sync.dma_start_transpose(ct_i[:, :, :], ct_t[:, :])
                del ct_t_tiles[(b, i)]

                def final(b=b, i=i, ct_i=ct_i, tail=tail):
                    psO = shp.tile([128, 512], F32, tag="sh")
                    for db in range(4):
                        nc.tensor.matmul(
                            psO, ct_i[:, db, :], wot_sb[:, db, :],
                            start=(db == 0), stop=(db == 3 and not has_bo))
                    if has_bo:
                        nc.tensor.matmul(psO, ones_k1, bob_sb[0:1, :],
                                         start=False, stop=True)
                    ot = outp.tile([128, 512], BF16)
                    row = b * S + i * 128
                    if tail and i == 3 and TUNE.get("split_last"):
                        # very last block: split copy+DMA into halves across
                        # DVE/ACT and sync/scalar so the end chain pipelines
                        nc.vector.tensor_copy(out=ot[:, 0:256],
                                              in_=psO[:, 0:256])
                        nc.scalar.copy(ot[:, 256:512], psO[:, 256:512])
                        nc.sync.dma_start(out=out[row:row + 128, 0:256],
                                          in_=ot[:, 0:256])
                        nc.scalar.dma_start(out=out[row:row + 128, 256:512],
                                            in_=ot[:, 256:512])
                    elif tail:
                        # alternate copy engine and DMA queue so the last
                        # finals' copies and issues don't serialize
                        if i % 2 == 0:
                            nc.scalar.copy(ot, psO)
                        else:
                            psum_copy(ot, psO)
                        eng = nc.sync if i % 2 == 0 else nc.scalar
                        eng.dma_start(out=out[row:row + 128, :], in_=ot)
                    else:
                        psum_copy(ot, psO)
                        nc.sync.dma_start(out=out[row:row + 128, :], in_=ot)

                staged.append((pop_ctr[0] if not tail else
                               pop_ctr[0] - TUNE["stage_delay"]
                               + TUNE["tail_stage_delay"], final))

            # ---- emission schedule (software-pipelined) ----
            # upfront: only what iteration 0's scores need; the rest of the
            # projections become ordered fillers consumed during attention.
            import functools
            # upfront: iteration 0 (b0, hp0, c0) needs only ob0/ch0.
            # k/q matmuls interleaved at db granularity to match the boot
            # DMA arrival order (psk/psq accumulate in separate PSUM banks)
            psk = shp.tile([128, 512], F32, tag="sh", name="psk")
            psq = shp.tile([128, 512], F32, tag="sh", name="psq")
            for db in range(4):
                nc.tensor.matmul(psk, wkt_ob[0][:, db, :], kt_c[0][:, db, :],
                                 start=(db == 0), stop=(db == 3),
                                 skip_group_check=True)
                nc.tensor.matmul(psq, wkt_ob[0][:, db, :], qt_c[0][:, db, :],
                                 start=(db == 0), stop=(db == 3),
                                 skip_group_check=True)
            if has_bk:
                nc.scalar.add(kp_sb[:, 0, 0:512], psk, bk_sb[:, 0:1])
                nc.vector.tensor_scalar_add(qp_sb[:, 0, 0:512], psq,
                                            bk_sb[:, 0:1])
            else:
                psum_copy(kp_sb[:, 0, 0:512], psk)
                psum_copy(qp_sb[:, 0, 0:512], psq)
            # deadline-ordered fillers matching the b0c0,b0c1,b1c1,b1c0 seq;
            # keys let consumers force-emit their prerequisites in time
            def FK(ob, ch):
                fillers.append((("kq", ob, ch), functools.partial(kq_group, ob, ch)))

            def FV(nt):
                fillers.append((("vp", nt), functools.partial(v_proj, nt)))

            FK(1, 0)
            FK(2, 0)
            FV(0)
            FV(1)
            FK(3, 0)
            FV(2)
            FV(3)
            FK(0, 1)
            FK(1, 1)
            FV(4)
            FV(5)
            FK(2, 1)
            FV(6)
            FV(7)
            FK(3, 1)
            for ob in range(4):
                FK(ob, 3)
                FK(ob, 2)
            for nt in range(12, 16):
                FV(nt)
            for nt in range(8, 12):
                FV(nt)

            seq = []
            border = {0: (0, 1), 1: (1, 0)}
            for b in range(BL):
                if NCH == 2 and b % 2 == 1 and TUNE.get("b1_order"):
                    # interleave the light c0 iterations among the heavy c1
                    # ones so the tail's exp backlog on ACT is smaller
                    for hp, c in TUNE["b1_order"]:
                        seq.append((b, hp, c))
                else:
                    for c in border[b % 2] if NCH == 2 else range(NCH):
                        for hp in range(4):
                            seq.append((b, hp, c))

            # iteration k's AV phase is interleaved with iteration k+2's
            # scores/exp packs (2-deep software pipeline): by the time an AV
            # runs, its exps retired during iteration k+1, so PE never waits
            # on ACT across iteration boundaries
            pend = []      # queue of AV thunk lists
            nseq = len(seq)
            for it, (b, hp, c) in enumerate(seq):
                # scores need this iteration's kq projections emitted first
                for ch in ([2 * b] if c == 0 else [2 * b, 2 * b + 1]):
                    need_filler(("kq", hp, ch))
                ex_t = {}
                packs = attention_packs(b, hp, c, ex_t)
                avs = attention_avs(b, hp, c, ex_t, last_hp=(hp == 3),
                                    tail=(it >= nseq - TUNE["tail_n"]))
                ready = pend.pop(0) if (len(pend) >= 2 or
                                        (pend and it == nseq - 1)) else []
                pops = 0
                # cap pops in the first (b0) half so fillers remain for the
                # ACT-bound b1c1 phase; first iterations also delay pops so
                # a not-yet-loaded filler can't head-of-line block PE
                cap = (TUNE["cap1"] if it < nseq * TUNE["capfrac1"] else
                       (TUNE["cap2"] if it < nseq * TUNE["capfrac2"] else 99))
                for x in range(max(len(packs), len(ready))):
                    do_pop = (it >= 2 or x >= 2) and pops < cap
                    if x < len(packs):
                        packs[x]()
                        if do_pop:
                            pop_filler(1)
                            pops += 1
                    if x < len(ready):
                        ready[x]()
                        if do_pop and pops < cap:
                            pop_filler(1)
                            pops += 1
                pend.append(avs)
            for avs in pend:
                for av in avs:
                    av()
                    pop_filler(1)
            while fillers or staged:
                pop_filler(1)

    return nc


_prog_cache = {}


def kernel(q, k, v, mask, zero_pad, Wk, bk, Wv, bv, Wo, bo):
    global LAST_SIM_NS, LAST_EXEC_NS
    q = np.asarray(q, dtype=np.float32)
    k = np.asarray(k, dtype=np.float32)
    v = np.asarray(v, dtype=np.float32)
    Wk = np.asarray(Wk, dtype=np.float32)
    Wv = np.asarray(Wv, dtype=np.float32)
    Wo = np.asarray(Wo, dtype=np.float32)
    bk = np.asarray(bk, dtype=np.float32).reshape(D)
    bv = np.asarray(bv, dtype=np.float32).reshape(D)
    bo = np.asarray(bo, dtype=np.float32).reshape(D)
    mask2d = np.asarray(mask).reshape(S, S).astype(bool)
    zp = int(np.asarray(zero_pad))

    status, patterns = _classify_mask(mask2d)
    plan, first_j = _plan_chunks(status, patterns)
    nmix = len(patterns)
    has_bk = bool(np.any(bk))
    has_bv = bool(np.any(bv))
    has_bo = bool(np.any(bo))

    sig = (tuple(tuple(r) for r in status), nmix, has_bk, has_bv, has_bo)
    if sig not in _prog_cache:
        nc_new = _build(plan, first_j, nmix, has_bk, has_bv, has_bo)
        legalize_waits(nc_new)   # hardware-only pass (sim runs pre-legalized)
        _prog_cache[sig] = nc_new
    nc = _prog_cache[sig]

    def _sbuf_layout(wt):
        # [D, X] -> [128, 4, X]: row d = a*128+p  ->  [p, a, :]
        return np.ascontiguousarray(wt.reshape(4, 128, -1).transpose(1, 0, 2))

    # wkt grouped by ob block: [4, 128, 4, 128], wkt[ob][p, db, c] =
    # Wk.T[db*128+p, ob*128+c]
    wkt = np.ascontiguousarray(
        _sbuf_layout(Wk.T.astype(BF)).reshape(128, 4, 4, 128)
        .transpose(2, 0, 1, 3))
    wvt = _sbuf_layout(Wv.T.astype(BF))
    wot = _sbuf_layout(Wo.T.astype(BF))
    bk32 = np.ascontiguousarray(bk.reshape(4, 128).T).astype(np.float32)
    bvb = bv.reshape(1, D).astype(BF)
    bob = bo.reshape(1, D).astype(BF)
    mixmul = (np.stack(patterns) if patterns
              else np.zeros((1, 128, 128), np.float32)).astype(BF)

    common = dict(wkt=wkt, wvt=wvt, wot=wot, bk32=bk32, bvb=bvb, bob=bob,
                  mixmul=mixmul, ident=np.eye(128, dtype=BF))
    wkt_ob0_flat = wkt[0].reshape(128, 512)
    in_maps = []
    for ci in range(NCORES):
        sl = slice(ci * BL, (ci + 1) * BL)
        qt_h = _sbuf_layout(q[sl].reshape(N, D).T.astype(BF))
        kt_h = _sbuf_layout(k[sl].reshape(N, D).T.astype(BF))
        # [db, {k,q}, t] interleaved to match the boot DMA pipeline
        kq0_h = np.stack([kt_h[:, :, 0:512], qt_h[:, :, 0:512]],
                         axis=2).reshape(128, 4096)
        boot_h = np.ascontiguousarray(
            np.concatenate([wkt_ob0_flat, kq0_h], axis=1))
        in_maps.append(dict(
            boot=boot_h,
            qt=qt_h,
            kt=kt_h,
            vt=_sbuf_layout(v[sl].reshape(N, D).T.astype(BF)),
            **common))

    if os.environ.get("BASS_KERNEL_SIM_TIME"):
        from concourse.timeline_sim import TimelineSim
        LAST_SIM_NS = TimelineSim(nc).simulate()

    res = run_bass_kernel_spmd(nc, in_maps, list(range(NCORES)))
    LAST_EXEC_NS = res.exec_time_ns

    outs = [np.asarray(res.results[ci]["out"], dtype=np.float32)
            .reshape(BL, S, D) for ci in range(NCORES)]
    full = np.concatenate(outs, axis=0)
    if zp:
        full[:, 0, :] = bo
    return full



# revision 46
# speedup vs baseline: 1.0848x; 1.0025x over previous
"""Trainium2 Bass kernel for nn_MultiHeadAttention_47579647705431.

Multi-head attention (8 heads, dim 512, seq 1024, batch 16) with:
  - shared key/query linear (key_query_same=True: q and k both use Wk/bk)
  - causal (or arbitrary block-structured) mask
  - SimpleKT zero_pad: attention row 0 zeroed => out[:, 0, :] = bo

Sharding: data-parallel over batch across 8 NeuronCores (2 batches/core).

Per-core pipeline (all matmuls bf16, fp32 PSUM):
  1. kp/qp = Wk.T-stationary projections -> feature-major [o, n] bf16
  2. vp    = token-major projection [n, o] bf16 with interleaved ones
             columns (stride-65) providing the softmax denominator column
  3. per (b, hp, c): scores^T st [t, s] via K=64 row-packed matmuls;
     exp on ACT (scale 1/8 folded, several j-blocks packed per
     activation); causal/diagonal masking as a 0/1 multiply on DVE
  4. AV with SWAPPED operands: stationary = ex [t, s-block], moving =
     vp [t, 65] -> av PSUM [s, 2, 65] per (b, i, hp).  Cost = 65 free
     columns per (i, j, head) instead of 512 -- half the PE cycles of
     the stationary-vp form, and the denominator lands per-partition so
     normalization is a DVE reciprocal + broadcast multiply (no PE
     broadcast matmuls, no mask identity matmuls).
  5. ct_t token-major [s, 512] per (b, i) -> feature-major ct_i
     [128, 4, 128] via one XBAR dma_start_transpose
  6. out projection per (b, i) (ct_i-stationary) -> [128, 512] f32 -> DRAM

The walrus build here supports ONE sync wait per instruction; Tile emits
more. legalize_waits() hoists extra waits onto same-engine NoOps.
"""

import os
from contextlib import ExitStack

import numpy as np
import ml_dtypes

import concourse.bass as bass
import concourse.mybir as mybir
import concourse.tile as tile
from concourse.bass_utils import run_bass_kernel_spmd

F32 = mybir.dt.float32
BF16 = mybir.dt.bfloat16
BF = ml_dtypes.bfloat16

B, S, D, H, DH = 16, 1024, 512, 8, 64
NCORES = 8
BL = B // NCORES          # batches per core
N = BL * S                # tokens per core
NB = S // 128             # 128-blocks per sequence (8)
HP = H // 2               # head pairs (= o-blocks of 128)
NCH = S // 512            # 512-chunks per sequence (2)

LAST_SIM_NS = None
LAST_EXEC_NS = None

# schedule tuning knobs (sweepable via TimelineSim)
TUNE = dict(
    cap1=4,        # pop cap while it < nseq*capfrac1
    cap2=5,        # pop cap while it < nseq*capfrac2
    capfrac1=0.55,
    capfrac2=0.85,
    stage_delay=17,   # pops a staged final waits before becoming poppable
    tail_stage_delay=4,   # same for tail (PE-transpose) finals
    tail_n=4,      # trailing iterations whose finals use the PE-transpose path
    b1_order=None,
    mask_on_pool=True,  # diag-mask multiplies on the idle GPSIMD engine
    split_mm_last=True,  # column-split the very last out-proj + copies
    packs_first=True,    # from packs_first_it on, emit all score packs
    packs_first_it=12,   # before AVs (feeds ACT sooner when saturated)
)


def legalize_waits(nc):
    """Split multi-wait instructions: keep one wait, hoist the rest onto
    preceding same-engine NoOps (this walrus encodes 1 wait/instruction)."""
    for f in nc.m.functions:
        for blk in f.blocks:
            il = blk.instructions
            i = 0
            while i < len(il):
                inst = il[i]
                si = inst.sync_info
                if si is not None and si.on_wait and len(si.on_wait) > 1:
                    waits = list(si.on_wait)
                    for j, w in enumerate(waits[:-1]):
                        nop = mybir.InstNoOp(
                            name=f"{inst.name}-hw{j}",
                            sync_info=mybir.SyncInfo(on_wait=[w], on_update=[]),
                            bass_nofuse=True,
                            engine=inst.engine,
                        )
                        il.insert(i, nop)
                        i += 1
                    si.on_wait = waits[-1:]
                i += 1


def _classify_mask(mask2d):
    """Classify 128x128 blocks of the [S, S] bool mask (query s, key t).

    Returns (status[j][i], patterns) in scores-transposed coords:
    j = key(t) block, i = query(s) block. status: -1 skip, -2 full,
    >=0 index into patterns (multiplicative bf16 0/1 [t, s] blocks).
    """
    status = [[-1] * NB for _ in range(NB)]
    patterns = []
    pat_idx = {}
    for j in range(NB):
        for i in range(NB):
            blk = mask2d[i * 128:(i + 1) * 128, j * 128:(j + 1) * 128]  # [s, t]
            if blk.all():
                status[j][i] = -2
            elif not blk.any():
                status[j][i] = -1
            else:
                mul = np.where(blk.T, 1.0, 0.0).astype(BF)  # [t, s]
                key = mul.tobytes()
                if key not in pat_idx:
                    pat_idx[key] = len(patterns)
                    patterns.append(mul)
                status[j][i] = pat_idx[key]
    return status, patterns


def _plan_chunks(status, patterns):
    """Per (c, j): suffix run of non-skip query blocks within chunk c.

    Returns plan[c][j] = (w, mixes) where w = run width and mixes =
    [(col_offset_in_region, pattern_id), ...] for mixed blocks. Also
    first_j[c]. Asserts the suffix-nested structure the kernel relies on.
    """
    plan = [[None] * NB for _ in range(NCH)]
    first_j = [None] * NCH
    for c in range(NCH):
        i_lo, i_hi = 4 * c, 4 * c + 4
        prev_w = None
        for j in range(NB):
            sts = [status[j][i] for i in range(i_lo, i_hi)]
            nz = [k for k, s in enumerate(sts) if s != -1]
            if not nz:
                plan[c][j] = (0, [])
                continue
            # must be a contiguous suffix of the chunk
            if nz != list(range(nz[0], 4)):
                raise NotImplementedError("mask block structure not suffix-contiguous")
            w = 128 * len(nz)
            if prev_w is not None and w > prev_w:
                raise NotImplementedError("mask runs not nested over key blocks")
            prev_w = w
            mixes = [((k - nz[0]) * 128, sts[k]) for k in nz if sts[k] >= 0]
            plan[c][j] = (w, mixes)
            if first_j[c] is None:
                first_j[c] = j
    return plan, first_j


def _pack_js(plan, c):
    """Greedy-pack consecutive j runs so one st tile / one exp covers
    several j blocks.  Each pack's total 2w must fit 1024 f32 (4KB)."""
    js = [j for j in range(NB) if plan[c][j][0] > 0]
    packs = []
    cur, cur_sz = [], 0
    for j in js:
        sz = 2 * plan[c][j][0]
        if cur and cur_sz + sz > 1024:
            packs.append(cur)
            cur, cur_sz = [], 0
        cur.append(j)
        cur_sz += sz
    if cur:
        packs.append(cur)
    return packs


def _build(plan, first_j, nmix, has_bk, has_bv, has_bo):
    nc = bass.Bass()
    # boot: startup-critical inputs merged in compute order so the first
    # projection's operands stream in a few pipelined DMAs:
    #   [wkt_ob0 (512) | ktc0 db-major (2048) | qtc0 db-major (2048)]
    boot = nc.dram_tensor("boot", [128, 4608], BF16, kind="ExternalInput")
    qt = nc.dram_tensor("qt", [128, 4, N], BF16, kind="ExternalInput")
    kt = nc.dram_tensor("kt", [128, 4, N], BF16, kind="ExternalInput")
    vt = nc.dram_tensor("vt", [128, 4, N], BF16, kind="ExternalInput")
    wkt = nc.dram_tensor("wkt", [4, 128, 4, 128], BF16, kind="ExternalInput")
    wvt = nc.dram_tensor("wvt", [128, 4, D], BF16, kind="ExternalInput")
    wot = nc.dram_tensor("wot", [128, 4, D], BF16, kind="ExternalInput")
    bk32 = nc.dram_tensor("bk32", [128, 4], F32, kind="ExternalInput")
    bvb = nc.dram_tensor("bvb", [1, D], BF16, kind="ExternalInput")
    bob = nc.dram_tensor("bob", [1, D], BF16, kind="ExternalInput")
    mixmul = nc.dram_tensor("mixmul", [max(nmix, 1), 128, 128], BF16,
                            kind="ExternalInput")
    ident = nc.dram_tensor("ident", [128, 128], BF16, kind="ExternalInput")
    # bf16 output: halves output DMA transfer time; host converts to f32.
    # Rounding cost ~2^-9 relative, well inside the error budget.
    out = nc.dram_tensor("out", [N, D], BF16, kind="ExternalOutput")

    with tile.TileContext(nc) as tc:
        with ExitStack() as ctx:
            sing = ctx.enter_context(tc.tile_pool(name="sing", bufs=1))
            expp = ctx.enter_context(tc.tile_pool(name="expp", bufs=21))
            rcp = ctx.enter_context(tc.tile_pool(name="rcp", bufs=4))
            ctp = ctx.enter_context(tc.tile_pool(name="ctp", bufs=2))
            cti = ctx.enter_context(tc.tile_pool(name="cti", bufs=4))
            outp = ctx.enter_context(tc.tile_pool(name="outp", bufs=4))
            stp = ctx.enter_context(tc.tile_pool(name="stp", bufs=2, space="PSUM"))
            avp = ctx.enter_context(tc.tile_pool(name="avp", bufs=2, space="PSUM"))
            shp = ctx.enter_context(tc.tile_pool(name="shp", bufs=2, space="PSUM"))

            # ---- input loads: critical-path first.  Attention-phase DMAs
            # go on sync; startup loads spread over scalar/sync/gpsimd.
            # kt0 on sync and wkt on scalar so the first projection's two
            # inputs stream through HWDGE back-to-back instead of serially
            kt_c = [None] * 4
            qt_c = [None] * 4
            vt_c = [None] * 4
            # boot tile: wkt_ob0 + interleaved k/q chunk-0 db slices, loaded
            # by 5 pipelined DMAs whose arrival order matches the db-
            # interleaved first projection (each DMA unblocks the next 1-2
            # matmuls, so PE starts at ~4us and never re-stalls)
            # each engine's SEQ is held through the ~625ns HWDGE acquire, so
            # one queue caps issue at ~1.3us/DMA -- slower than the boot
            # chunks' 728ns transfers.  A few early loads go on the scalar
            # (ACT) queue to double the issue rate, but only ones that clear
            # ACT.SEQ before the first exp (~10us); the rest stay on sync so
            # DMA issues never delay exp decode.
            scalar_set = {1, 3, 7, 9, 11}   # boot2, boot4, wkt2, wvt, qtc1
            qrr = [0]

            def ldq():
                i = qrr[0]
                qrr[0] += 1
                return nc.scalar if i in scalar_set else nc.sync

            boot_sb = sing.tile([128, 4608], BF16, name="boot_sb")
            for lo, hi in ((0, 1024), (1024, 2048), (2048, 3072),
                           (3072, 4096), (4096, 4608)):
                ldq().dma_start(out=boot_sb[:, lo:hi], in_=boot[:, lo:hi])
            wkt_ob = []
            wkt_ob.append(boot_sb[:, 0:512].rearrange("p (db c) -> p db c", db=4))
            for ob in range(1, 4):
                t = sing.tile([128, 4, 128], BF16, tag=f"wktob{ob}",
                              name=f"wktob{ob}")
                wkt_ob.append(t)
            kq0 = boot_sb[:, 512:4608].rearrange("p (db two t) -> p two db t",
                                                 two=2, t=512)
            kt_c[0] = kq0[:, 0]
            qt_c[0] = kq0[:, 1]
            # all remaining loads on the single sync queue in strict
            # deadline order: SP issues every ~650ns (never holding waits)
            # and DMA_ENGINES FIFO == emission order, so nothing competes
            # with the startup-critical boot DMAs
            for ob in range(1, 4):
                ldq().dma_start(out=wkt_ob[ob], in_=wkt[ob, :, :, :])
            wvt_sb = sing.tile([128, 4, D], BF16)
            ldq().dma_start(out=wvt_sb, in_=wvt[:, :, :])

            def load_chunk(which, ch):
                t = sing.tile([128, 4, 512], BF16, tag=f"{which}tc{ch}",
                              name=f"{which}tc{ch}")
                src = kt if which == "k" else (qt if which == "q" else vt)
                ldq().dma_start(out=t, in_=src[:, :, ch * 512:ch * 512 + 512])
                (kt_c if which == "k" else (qt_c if which == "q" else vt_c))[ch] = t

            load_chunk("v", 0)
            mix_sb = sing.tile([128, max(nmix, 1), 128], BF16)
            ldq().dma_start(out=mix_sb, in_=mixmul.rearrange("m t s -> t m s"))
            load_chunk("k", 1)
            load_chunk("q", 1)
            wot_sb = sing.tile([128, 4, D], BF16)
            ldq().dma_start(out=wot_sb, in_=wot[:, :, :])
            load_chunk("v", 1)
            load_chunk("k", 3)
            load_chunk("q", 3)
            load_chunk("v", 3)
            load_chunk("k", 2)
            load_chunk("q", 2)
            load_chunk("v", 2)
            ident_sb = sing.tile([128, 128], BF16)
            ldq().dma_start(out=ident_sb, in_=ident[:, :])
            bk_sb = None
            if has_bk:
                bk_sb = sing.tile([128, 4], F32)
                nc.sync.dma_start(out=bk_sb, in_=bk32[:, :])
            bvb_sb = bob_sb = ones_k1 = None
            if has_bv or has_bo:
                ones_k1 = sing.tile([1, 128], BF16)
                nc.vector.memset(ones_k1, 1.0)
            if has_bv:
                bvb_sb = sing.tile([1, D], BF16)
                nc.sync.dma_start(out=bvb_sb, in_=bvb[:, :])
            if has_bo:
                bob_sb = sing.tile([1, D], BF16)
                nc.sync.dma_start(out=bob_sb, in_=bob[:, :])

            kp_sb = sing.tile([128, 4, N], BF16)
            qp_sb = sing.tile([128, 4, N], BF16)
            vp_sb = sing.tile([128, N // 128, 520], BF16)

            # PE p-state warmup: dummy matmuls on memset scratch from ~1.4us
            # so the clock has ramped by the time the first input DMA lands
            # (PE runs at 0.65/1.2GHz for the first 3us of a busy stretch)
            nwarm = TUNE.get("nwarm", 10)
            if nwarm:
                warm_a = sing.tile([128, 128], BF16, name="warm_a")
                warm_b = sing.tile([128, 130], BF16, name="warm_b")
                nc.vector.memset(warm_a, 0.0)
                nc.vector.memset(warm_b, 0.0)
                warm_ps = avp.tile([128, 2, 65], F32, tag="av", name="warm_ps")
                for _ in range(nwarm):
                    nc.tensor.matmul(warm_ps.rearrange("p a b -> p (a b)"),
                                     warm_a, warm_b, start=True, stop=True,
                                     skip_group_check=True)

            # ones columns of vp (denominator trick)
            for nt in range(N // 128):
                nc.vector.memset(
                    vp_sb[:, nt, :].rearrange("p (h u) -> p h u", u=65)[:, :, 64:65],
                    1.0)

            fillers = []
            staged = []    # (pop_stamp, thunk): finals wait out their
                           # transpose latency before becoming poppable
            pop_ctr = [0]
            copy_rr = [0]

            def psum_copy(dst, src):
                # PSUM->SBUF copies on DVE; ACT stays exp-only and GPSIMD
                # cannot access PSUM
                nc.vector.tensor_copy(out=dst, in_=src)

            def kq_half(ob, ch, which):
                csl = slice(ch * 512, ch * 512 + 512)
                src = kt_c[ch] if which == "k" else qt_c[ch]
                dst = kp_sb if which == "k" else qp_sb
                ps = shp.tile([128, 512], F32, tag="sh", name=f"ps{which}")
                for db in range(4):
                    nc.tensor.matmul(
                        ps, wkt_ob[ob][:, db, :],
                        src[:, db, :], start=(db == 0), stop=(db == 3))
                if has_bk:
                    if which == "k":
                        nc.scalar.add(dst[:, ob, csl], ps, bk_sb[:, ob:ob + 1])
                    else:
                        nc.vector.tensor_scalar_add(
                            dst[:, ob, csl], ps, bk_sb[:, ob:ob + 1])
                else:
                    psum_copy(dst[:, ob, csl], ps)

            def kq_group(ob, ch):
                kq_half(ob, ch, "k")
                kq_half(ob, ch, "q")

            def v_proj(nt):
                psV = shp.tile([128, 512], F32, tag="sh")
                for db in range(4):
                    nc.tensor.matmul(
                        psV, vt_c[nt // 4][:, db, (nt % 4) * 128:(nt % 4) * 128 + 128],
                        wvt_sb[:, db, :], start=(db == 0),
                        stop=(db == 3 and not has_bv))
                if has_bv:
                    nc.tensor.matmul(psV, ones_k1, bvb_sb[0:1, :],
                                     start=False, stop=True)
                dst = vp_sb[:, nt, :].rearrange("p (h u) -> p h u", u=65)[:, :, 0:64]
                src = psV.rearrange("p (h u) -> p h u", u=64)
                psum_copy(dst, src)

            def pop_filler(k=1):
                for _ in range(k):
                    pop_ctr[0] += 1
                    while staged and staged[0][0] + TUNE["stage_delay"] <= pop_ctr[0]:
                        fillers.append((("fin",), staged.pop(0)[1]))
                    if fillers:
                        fillers.pop(0)[1]()

            def need_filler(key):
                """Force-emit a specific filler now (dependency deadline)."""
                for fi, (k, thunk) in enumerate(fillers):
                    if k == key:
                        fillers.pop(fi)
                        thunk()
                        return

            ct_t_tiles = {}

            def attention_packs(b, hp, c, ex_t):
                """Thunks: scores+exp+mask, one per pack of j blocks."""
                packs = _pack_js(plan, c)
                thunks = []

                def do_pack(pack):
                    # PSUM bank rule: each matmul output must stay inside one
                    # 2KB bank.  h0 segments stack downward from col 512
                    # (bank 0), h1 segments upward from col 512 (bank 1); the
                    # exp covers the contiguous union [512-tw, 512+tw).
                    tw = sum(plan[c][j][0] for j in pack)
                    st = stp.tile([128, 1024], F32, tag="st")
                    ex = expp.tile([128, 1024], BF16, tag="ex")
                    pre = 0
                    for j in pack:
                        w, _ = plan[c][j]
                        tsl = slice(b * S + j * 128, b * S + j * 128 + 128)
                        ssl = slice(b * S + c * 512 + 512 - w,
                                    b * S + c * 512 + 512)
                        h0s = 512 - pre - w
                        h1s = 512 + pre
                        nc.tensor.matmul(st[:, h0s:h0s + w],
                                         kp_sb[0:64, hp, tsl],
                                         qp_sb[0:64, hp, ssl],
                                         start=True, stop=True,
                                         skip_group_check=True)
                        nc.tensor.matmul(st[:, h1s:h1s + w],
                                         kp_sb[64:128, hp, tsl],
                                         qp_sb[64:128, hp, ssl],
                                         start=True, stop=True,
                                         skip_group_check=True)
                        ex_t[j] = (ex, h0s, h1s, w)
                        pre += w
                    nc.scalar.activation(
                        ex[:, 512 - tw:512 + tw], st[:, 512 - tw:512 + tw],
                        mybir.ActivationFunctionType.Exp, scale=0.125)
                    # 0/1 mask multiply for mixed blocks (per head half)
                    mask_eng = (nc.gpsimd if TUNE.get("mask_on_pool")
                                else nc.vector)
                    for j in pack:
                        w, mixes = plan[c][j]
                        _, h0s, h1s, _ = ex_t[j]
                        for moff, pid in mixes:
                            for hs in (h0s, h1s):
                                sl = ex[:, hs + moff:hs + moff + 128]
                                mask_eng.tensor_mul(
                                    sl, sl, mix_sb[:, pid, :])

                for pack in packs:
                    import functools
                    thunks.append(functools.partial(do_pack, pack))
                return thunks

            def attention_avs(b, hp, c, ex_t, last_hp, tail=False):
                """Thunks: AV + normalize, one per query block i (swapped
                operands: ex stationary, vp moving)."""
                h0 = 2 * hp
                thunks = []

                def do_av(i):
                    js_i = []
                    for j, (ex, h0s, h1s, w) in ex_t.items():
                        i_start = 4 * c + 4 - w // 128
                        if i >= i_start:
                            o = (i - i_start) * 128
                            js_i.append((j, ex, (h0s + o, h1s + o)))
                    if not js_i:
                        return
                    for j, _, _ in js_i:
                        need_filler(("vp", b * NB + j))
                    av = avp.tile([128, 2, 65], F32, tag="av")
                    nmm = len(js_i) * 2
                    mi = 0
                    for j, ex, hss in js_i:
                        vrow = b * NB + j
                        for h in range(2):
                            # single accumulation group per av tile: PSUM
                            # zeroing is bank-granular (start marks the whole
                            # bank pending-zero; first write to each address
                            # assigns, later writes accumulate)
                            nc.tensor.matmul(
                                av[:, h, :],
                                ex[:, hss[h]:hss[h] + 128],
                                vp_sb[:, vrow,
                                      65 * (h0 + h):65 * (h0 + h) + 65],
                                start=(mi == 0), stop=(mi == nmm - 1),
                                skip_group_check=True)
                            mi += 1
                    # normalize: per-partition reciprocal + broadcast mul
                    key = (b, i)
                    if key not in ct_t_tiles:
                        ct_t_tiles[key] = ctp.tile([128, 512], BF16,
                                                   name=f"ctt{b}_{i}",
                                                   tag=f"ctt{b}_{i % 4}")
                    ct_t = ct_t_tiles[key]
                    rc = rcp.tile([128, 2], BF16, tag="rc")
                    with nc.allow_low_precision(reason="softmax recip bf16"):
                        nc.vector.reciprocal(out=rc, in_=av[:, :, 64])
                    dst = ct_t[:, 128 * hp:128 * hp + 128].rearrange(
                        "p (h w) -> p h w", h=2)
                    nc.vector.tensor_mul(
                        dst, av[:, :, 0:64],
                        rc[:, :, None].broadcast_to([128, 2, 64]))
                    if last_hp:
                        finish_block(b, i, ct_t, tail=tail)

                import functools
                for i in range(4 * c, 4 * c + 4):
                    thunks.append(functools.partial(do_av, i))
                return thunks

            def finish_block(b, i, ct_t, tail=False):
                """transpose ct_t -> feature-major, then queue out-proj.

                tail=True blocks (the last c-group) use PE transposes + an
                ACT copy instead of a DMA transpose: ~1us latency instead of
                ~3.3us of DMA issue+transfer+sem-prop on the critical tail.
                """
                ct_i = cti.tile([128, 4, 128], BF16, tag="cti")
                if tail:
                    stq = stp.tile([128, 1024], F32, tag="st",
                                   name=f"pst{b}_{i}")
                    psT = stq.bitcast(BF16)
                    for q4 in range(4):
                        nc.tensor.transpose(
                            psT[:, q4 * 128:q4 * 128 + 128],
                            ct_t[:, q4 * 128:q4 * 128 + 128], ident_sb)
                    # DVE, not ACT: ACT is still draining exps here (in-order
                    # engine => a copy there queues behind them), and the
                    # bf16->bf16 copy gets DVE's 2x mode
                    nc.vector.tensor_copy(
                        out=ct_i.rearrange("p a c -> p (a c)"),
                        in_=psT[:, 0:512])
                else:
                    nc.sync.dma_start_transpose(ct_i[:, :, :], ct_t[:, :])
                del ct_t_tiles[(b, i)]

                def final(b=b, i=i, ct_i=ct_i, tail=tail):
                    psO = shp.tile([128, 512], F32, tag="sh")
                    if tail and i == 3 and TUNE.get("split_mm_last"):
                        # column-split the accumulation so the first half's
                        # copy overlaps the second half's matmuls
                        for half in range(2):
                            cs = slice(half * 256, half * 256 + 256)
                            for db in range(4):
                                nc.tensor.matmul(
                                    psO[:, cs], ct_i[:, db, :],
                                    wot_sb[:, db, cs],
                                    start=(db == 0), stop=(db == 3),
                                    skip_group_check=True)
                        ot = outp.tile([128, 512], BF16)
                        ca, cb, dq = TUNE.get("last_engs", ("v", "v", "sync"))
                        for half, ce in ((0, ca), (1, cb)):
                            hs = slice(half * 256, half * 256 + 256)
                            if ce == "v":
                                nc.vector.tensor_copy(out=ot[:, hs],
                                                      in_=psO[:, hs])
                            else:
                                nc.scalar.copy(ot[:, hs], psO[:, hs])
                        row = b * S + i * 128
                        eng = dict(sync=nc.sync, scalar=nc.scalar)[dq]
                        eng.dma_start(out=out[row:row + 128, :], in_=ot)
                        return
                    for db in range(4):
                        nc.tensor.matmul(
                            psO, ct_i[:, db, :], wot_sb[:, db, :],
                            start=(db == 0), stop=(db == 3 and not has_bo))
                    if has_bo:
                        nc.tensor.matmul(psO, ones_k1, bob_sb[0:1, :],
                                         start=False, stop=True)
                    ot = outp.tile([128, 512], BF16)
                    row = b * S + i * 128
                    if tail and i == 3 and TUNE.get("split_last"):
                        # very last block: split copy+DMA into halves across
                        # DVE/ACT and sync/scalar so the end chain pipelines
                        nc.vector.tensor_copy(out=ot[:, 0:256],
                                              in_=psO[:, 0:256])
                        nc.scalar.copy(ot[:, 256:512], psO[:, 256:512])
                        nc.sync.dma_start(out=out[row:row + 128, 0:256],
                                          in_=ot[:, 0:256])
                        nc.scalar.dma_start(out=out[row:row + 128, 256:512],
                                            in_=ot[:, 256:512])
                    elif tail:
                        # alternate copy engine and DMA queue so the last
                        # finals' copies and issues don't serialize
                        par = (i + TUNE.get("tail_parity", 0)) % 2
                        if par == 0:
                            nc.scalar.copy(ot, psO)
                        else:
                            psum_copy(ot, psO)
                        eng = nc.sync if par == 0 else nc.scalar
                        eng.dma_start(out=out[row:row + 128, :], in_=ot)
                    else:
                        psum_copy(ot, psO)
                        nc.sync.dma_start(out=out[row:row + 128, :], in_=ot)

                staged.append((pop_ctr[0] if not tail else
                               pop_ctr[0] - TUNE["stage_delay"]
                               + TUNE["tail_stage_delay"], final))

            # ---- emission schedule (software-pipelined) ----
            # upfront: only what iteration 0's scores need; the rest of the
            # projections become ordered fillers consumed during attention.
            import functools
            # upfront: iteration 0 (b0, hp0, c0) needs only ob0/ch0.
            # k/q matmuls interleaved at db granularity to match the boot
            # DMA arrival order (psk/psq accumulate in separate PSUM banks)
            psk = shp.tile([128, 512], F32, tag="sh", name="psk")
            psq = shp.tile([128, 512], F32, tag="sh", name="psq")
            for db in range(4):
                nc.tensor.matmul(psk, wkt_ob[0][:, db, :], kt_c[0][:, db, :],
                                 start=(db == 0), stop=(db == 3),
                                 skip_group_check=True)
                nc.tensor.matmul(psq, wkt_ob[0][:, db, :], qt_c[0][:, db, :],
                                 start=(db == 0), stop=(db == 3),
                                 skip_group_check=True)
            if has_bk:
                nc.scalar.add(kp_sb[:, 0, 0:512], psk, bk_sb[:, 0:1])
                nc.vector.tensor_scalar_add(qp_sb[:, 0, 0:512], psq,
                                            bk_sb[:, 0:1])
            else:
                psum_copy(kp_sb[:, 0, 0:512], psk)
                psum_copy(qp_sb[:, 0, 0:512], psq)
            # deadline-ordered fillers matching the b0c0,b0c1,b1c1,b1c0 seq;
            # keys let consumers force-emit their prerequisites in time
            def FK(ob, ch):
                fillers.append((("kq", ob, ch), functools.partial(kq_group, ob, ch)))

            def FV(nt):
                fillers.append((("vp", nt), functools.partial(v_proj, nt)))

            FK(1, 0)
            FK(2, 0)
            FV(0)
            FV(1)
            FK(3, 0)
            FV(2)
            FV(3)
            FK(0, 1)
            FK(1, 1)
            FV(4)
            FV(5)
            FK(2, 1)
            FV(6)
            FV(7)
            FK(3, 1)
            for ob in range(4):
                FK(ob, 3)
                FK(ob, 2)
            for nt in range(12, 16):
                FV(nt)
            for nt in range(8, 12):
                FV(nt)

            seq = []
            border = {0: (0, 1), 1: (1, 0)}
            for b in range(BL):
                if NCH == 2 and b % 2 == 1 and TUNE.get("b1_order"):
                    # interleave the light c0 iterations among the heavy c1
                    # ones so the tail's exp backlog on ACT is smaller
                    for hp, c in TUNE["b1_order"]:
                        seq.append((b, hp, c))
                else:
                    for c in border[b % 2] if NCH == 2 else range(NCH):
                        for hp in range(4):
                            seq.append((b, hp, c))

            # iteration k's AV phase is interleaved with iteration k+2's
            # scores/exp packs (2-deep software pipeline): by the time an AV
            # runs, its exps retired during iteration k+1, so PE never waits
            # on ACT across iteration boundaries
            pend = []      # queue of AV thunk lists
            nseq = len(seq)
            for it, (b, hp, c) in enumerate(seq):
                # scores need this iteration's kq projections emitted first
                for ch in ([2 * b] if c == 0 else [2 * b, 2 * b + 1]):
                    need_filler(("kq", hp, ch))
                ex_t = {}
                packs = attention_packs(b, hp, c, ex_t)
                avs = attention_avs(b, hp, c, ex_t, last_hp=(hp == 3),
                                    tail=(it >= nseq - TUNE["tail_n"]))
                ready = pend.pop(0) if (len(pend) >= 2 or
                                        (pend and it == nseq - 1)) else []
                pops = 0
                # cap pops in the first (b0) half so fillers remain for the
                # ACT-bound b1c1 phase; first iterations also delay pops so
                # a not-yet-loaded filler can't head-of-line block PE
                cap = (TUNE["cap1"] if it < nseq * TUNE["capfrac1"] else
                       (TUNE["cap2"] if it < nseq * TUNE["capfrac2"] else 99))
                if TUNE.get("packs_first") and it >= TUNE.get(
                        "packs_first_it", 8):
                    for x in range(len(packs)):
                        packs[x]()
                        if (it >= 2 or x >= 2) and pops < cap:
                            pop_filler(1)
                            pops += 1
                    for x in range(len(ready)):
                        ready[x]()
                        if pops < cap:
                            pop_filler(1)
                            pops += 1
                else:
                    for x in range(max(len(packs), len(ready))):
                        do_pop = (it >= 2 or x >= 2) and pops < cap
                        if x < len(packs):
                            packs[x]()
                            if do_pop:
                                pop_filler(1)
                                pops += 1
                        if x < len(ready):
                            ready[x]()
                            if do_pop and pops < cap:
                                pop_filler(1)
                                pops += 1
                pend.append(avs)
            for avs in pend:
                for av in avs:
                    av()
                    pop_filler(1)
            while fillers or staged:
                pop_filler(1)

    return nc


_prog_cache = {}


def kernel(q, k, v, mask, zero_pad, Wk, bk, Wv, bv, Wo, bo):
    global LAST_SIM_NS, LAST_EXEC_NS
    q = np.asarray(q, dtype=np.float32)
    k = np.asarray(k, dtype=np.float32)
    v = np.asarray(v, dtype=np.float32)
    Wk = np.asarray(Wk, dtype=np.float32)
    Wv = np.asarray(Wv, dtype=np.float32)
    Wo = np.asarray(Wo, dtype=np.float32)
    bk = np.asarray(bk, dtype=np.float32).reshape(D)
    bv = np.asarray(bv, dtype=np.float32).reshape(D)
    bo = np.asarray(bo, dtype=np.float32).reshape(D)
    mask2d = np.asarray(mask).reshape(S, S).astype(bool)
    zp = int(np.asarray(zero_pad))

    status, patterns = _classify_mask(mask2d)
    plan, first_j = _plan_chunks(status, patterns)
    nmix = len(patterns)
    has_bk = bool(np.any(bk))
    has_bv = bool(np.any(bv))
    has_bo = bool(np.any(bo))

    sig = (tuple(tuple(r) for r in status), nmix, has_bk, has_bv, has_bo)
    if sig not in _prog_cache:
        nc_new = _build(plan, first_j, nmix, has_bk, has_bv, has_bo)
        legalize_waits(nc_new)   # hardware-only pass (sim runs pre-legalized)
        _prog_cache[sig] = nc_new
    nc = _prog_cache[sig]

    def _sbuf_layout(wt):
        # [D, X] -> [128, 4, X]: row d = a*128+p  ->  [p, a, :]
        return np.ascontiguousarray(wt.reshape(4, 128, -1).transpose(1, 0, 2))

    # wkt grouped by ob block: [4, 128, 4, 128], wkt[ob][p, db, c] =
    # Wk.T[db*128+p, ob*128+c]
    wkt = np.ascontiguousarray(
        _sbuf_layout(Wk.T.astype(BF)).reshape(128, 4, 4, 128)
        .transpose(2, 0, 1, 3))
    wvt = _sbuf_layout(Wv.T.astype(BF))
    wot = _sbuf_layout(Wo.T.astype(BF))
    bk32 = np.ascontiguousarray(bk.reshape(4, 128).T).astype(np.float32)
    bvb = bv.reshape(1, D).astype(BF)
    bob = bo.reshape(1, D).astype(BF)
    mixmul = (np.stack(patterns) if patterns
              else np.zeros((1, 128, 128), np.float32)).astype(BF)

    common = dict(wkt=wkt, wvt=wvt, wot=wot, bk32=bk32, bvb=bvb, bob=bob,
                  mixmul=mixmul, ident=np.eye(128, dtype=BF))
    wkt_ob0_flat = wkt[0].reshape(128, 512)
    in_maps = []
    for ci in range(NCORES):
        sl = slice(ci * BL, (ci + 1) * BL)
        qt_h = _sbuf_layout(q[sl].reshape(N, D).T.astype(BF))
        kt_h = _sbuf_layout(k[sl].reshape(N, D).T.astype(BF))
        # [db, {k,q}, t] interleaved to match the boot DMA pipeline
        kq0_h = np.stack([kt_h[:, :, 0:512], qt_h[:, :, 0:512]],
                         axis=2).reshape(128, 4096)
        boot_h = np.ascontiguousarray(
            np.concatenate([wkt_ob0_flat, kq0_h], axis=1))
        in_maps.append(dict(
            boot=boot_h,
            qt=qt_h,
            kt=kt_h,
            vt=_sbuf_layout(v[sl].reshape(N, D).T.astype(BF)),
            **common))

    if os.environ.get("BASS_KERNEL_SIM_TIME"):
        from concourse.timeline_sim import TimelineSim
        LAST_SIM_NS = TimelineSim(nc).simulate()

    res = run_bass_kernel_spmd(nc, in_maps, list(range(NCORES)))
    LAST_EXEC_NS = res.exec_time_ns

    outs = [np.asarray(res.results[ci]["out"], dtype=np.float32)
            .reshape(BL, S, D) for ci in range(NCORES)]
    full = np.concatenate(outs, axis=0)
    if zp:
        full[:, 0, :] = bo
    return full



# revision 49
# speedup vs baseline: 1.0895x; 1.0043x over previous
"""Trainium2 Bass kernel for nn_MultiHeadAttention_47579647705431.

Multi-head attention (8 heads, dim 512, seq 1024, batch 16) with:
  - shared key/query linear (key_query_same=True: q and k both use Wk/bk)
  - causal (or arbitrary block-structured) mask
  - SimpleKT zero_pad: attention row 0 zeroed => out[:, 0, :] = bo

Sharding: data-parallel over batch across 8 NeuronCores (2 batches/core).

Per-core pipeline (all matmuls bf16, fp32 PSUM):
  1. kp/qp = Wk.T-stationary projections -> feature-major [o, n] bf16
  2. vp    = token-major projection [n, o] bf16 with interleaved ones
             columns (stride-65) providing the softmax denominator column
  3. per (b, hp, c): scores^T st [t, s] via K=64 row-packed matmuls;
     exp on ACT (scale 1/8 folded, several j-blocks packed per
     activation); causal/diagonal masking as a 0/1 multiply on DVE
  4. AV with SWAPPED operands: stationary = ex [t, s-block], moving =
     vp [t, 65] -> av PSUM [s, 2, 65] per (b, i, hp).  Cost = 65 free
     columns per (i, j, head) instead of 512 -- half the PE cycles of
     the stationary-vp form, and the denominator lands per-partition so
     normalization is a DVE reciprocal + broadcast multiply (no PE
     broadcast matmuls, no mask identity matmuls).
  5. ct_t token-major [s, 512] per (b, i) -> feature-major ct_i
     [128, 4, 128] via one XBAR dma_start_transpose
  6. out projection per (b, i) (ct_i-stationary) -> [128, 512] f32 -> DRAM

The walrus build here supports ONE sync wait per instruction; Tile emits
more. legalize_waits() hoists extra waits onto same-engine NoOps.
"""

import os
from contextlib import ExitStack

import numpy as np
import ml_dtypes

import concourse.bass as bass
import concourse.mybir as mybir
import concourse.tile as tile
from concourse.bass_utils import run_bass_kernel_spmd

F32 = mybir.dt.float32
BF16 = mybir.dt.bfloat16
BF = ml_dtypes.bfloat16

B, S, D, H, DH = 16, 1024, 512, 8, 64
NCORES = 8
BL = B // NCORES          # batches per core
N = BL * S                # tokens per core
NB = S // 128             # 128-blocks per sequence (8)
HP = H // 2               # head pairs (= o-blocks of 128)
NCH = S // 512            # 512-chunks per sequence (2)

LAST_SIM_NS = None
LAST_EXEC_NS = None

# schedule tuning knobs (sweepable via TimelineSim)
TUNE = dict(
    cap1=4,        # pop cap while it < nseq*capfrac1
    cap2=5,        # pop cap while it < nseq*capfrac2
    capfrac1=0.55,
    capfrac2=0.85,
    stage_delay=17,   # pops a staged final waits before becoming poppable
    tail_stage_delay=4,   # same for tail (PE-transpose) finals
    tail_n=4,      # trailing iterations whose finals use the PE-transpose path
    b1_order=None,
    mask_on_pool=True,  # diag-mask multiplies on the idle GPSIMD engine
    split_mm_last=True,  # column-split the very last out-proj + copies
    packs_first=True,    # from packs_first_it on, emit all score packs
    packs_first_it=12,   # before AVs (feeds ACT sooner when saturated)
)


def legalize_waits(nc):
    """Split multi-wait instructions: keep one wait, hoist the rest onto
    preceding same-engine NoOps (this walrus encodes 1 wait/instruction)."""
    for f in nc.m.functions:
        for blk in f.blocks:
            il = blk.instructions
            i = 0
            while i < len(il):
                inst = il[i]
                si = inst.sync_info
                if si is not None and si.on_wait and len(si.on_wait) > 1:
                    waits = list(si.on_wait)
                    for j, w in enumerate(waits[:-1]):
                        nop = mybir.InstNoOp(
                            name=f"{inst.name}-hw{j}",
                            sync_info=mybir.SyncInfo(on_wait=[w], on_update=[]),
                            bass_nofuse=True,
                            engine=inst.engine,
                        )
                        il.insert(i, nop)
                        i += 1
                    si.on_wait = waits[-1:]
                i += 1


def _classify_mask(mask2d):
    """Classify 128x128 blocks of the [S, S] bool mask (query s, key t).

    Returns (status[j][i], patterns) in scores-transposed coords:
    j = key(t) block, i = query(s) block. status: -1 skip, -2 full,
    >=0 index into patterns (multiplicative bf16 0/1 [t, s] blocks).
    """
    status = [[-1] * NB for _ in range(NB)]
    patterns = []
    pat_idx = {}
    for j in range(NB):
        for i in range(NB):
            blk = mask2d[i * 128:(i + 1) * 128, j * 128:(j + 1) * 128]  # [s, t]
            if blk.all():
                status[j][i] = -2
            elif not blk.any():
                status[j][i] = -1
            else:
                mul = np.where(blk.T, 1.0, 0.0).astype(BF)  # [t, s]
                key = mul.tobytes()
                if key not in pat_idx:
                    pat_idx[key] = len(patterns)
                    patterns.append(mul)
                status[j][i] = pat_idx[key]
    return status, patterns


def _plan_chunks(status, patterns):
    """Per (c, j): suffix run of non-skip query blocks within chunk c.

    Returns plan[c][j] = (w, mixes) where w = run width and mixes =
    [(col_offset_in_region, pattern_id), ...] for mixed blocks. Also
    first_j[c]. Asserts the suffix-nested structure the kernel relies on.
    """
    plan = [[None] * NB for _ in range(NCH)]
    first_j = [None] * NCH
    for c in range(NCH):
        i_lo, i_hi = 4 * c, 4 * c + 4
        prev_w = None
        for j in range(NB):
            sts = [status[j][i] for i in range(i_lo, i_hi)]
            nz = [k for k, s in enumerate(sts) if s != -1]
            if not nz:
                plan[c][j] = (0, [])
                continue
            # must be a contiguous suffix of the chunk
            if nz != list(range(nz[0], 4)):
                raise NotImplementedError("mask block structure not suffix-contiguous")
            w = 128 * len(nz)
            if prev_w is not None and w > prev_w:
                raise NotImplementedError("mask runs not nested over key blocks")
            prev_w = w
            mixes = [((k - nz[0]) * 128, sts[k]) for k in nz if sts[k] >= 0]
            plan[c][j] = (w, mixes)
            if first_j[c] is None:
                first_j[c] = j
    return plan, first_j


def _pack_js(plan, c):
    """Greedy-pack consecutive j runs so one st tile / one exp covers
    several j blocks.  Each pack's total 2w must fit 1024 f32 (4KB)."""
    js = [j for j in range(NB) if plan[c][j][0] > 0]
    packs = []
    cur, cur_sz = [], 0
    for j in js:
        sz = 2 * plan[c][j][0]
        if cur and cur_sz + sz > 1024:
            packs.append(cur)
            cur, cur_sz = [], 0
        cur.append(j)
        cur_sz += sz
    if cur:
        packs.append(cur)
    return packs


def _build(plan, first_j, nmix, has_bk, has_bv, has_bo):
    nc = bass.Bass()
    # boot: startup-critical inputs merged in compute order so the first
    # projection's operands stream in a few pipelined DMAs:
    #   [wkt_ob0 (512) | ktc0 db-major (2048) | qtc0 db-major (2048)]
    boot = nc.dram_tensor("boot", [128, 4608], BF16, kind="ExternalInput")
    qt = nc.dram_tensor("qt", [128, 4, N], BF16, kind="ExternalInput")
    kt = nc.dram_tensor("kt", [128, 4, N], BF16, kind="ExternalInput")
    vt = nc.dram_tensor("vt", [128, 4, N], BF16, kind="ExternalInput")
    wkt = nc.dram_tensor("wkt", [4, 128, 4, 128], BF16, kind="ExternalInput")
    wvt = nc.dram_tensor("wvt", [128, 4, D], BF16, kind="ExternalInput")
    wot = nc.dram_tensor("wot", [128, 4, D], BF16, kind="ExternalInput")
    bk32 = nc.dram_tensor("bk32", [128, 4], F32, kind="ExternalInput")
    bvb = nc.dram_tensor("bvb", [1, D], BF16, kind="ExternalInput")
    bob = nc.dram_tensor("bob", [1, D], BF16, kind="ExternalInput")
    mixmul = nc.dram_tensor("mixmul", [max(nmix, 1), 128, 128], BF16,
                            kind="ExternalInput")
    ident = nc.dram_tensor("ident", [128, 128], BF16, kind="ExternalInput")
    # bf16 output: halves output DMA transfer time; host converts to f32.
    # Rounding cost ~2^-9 relative, well inside the error budget.
    out = nc.dram_tensor("out", [N, D], BF16, kind="ExternalOutput")

    with tile.TileContext(nc) as tc:
        with ExitStack() as ctx:
            sing = ctx.enter_context(tc.tile_pool(name="sing", bufs=1))
            expp = ctx.enter_context(tc.tile_pool(name="expp", bufs=21))
            rcp = ctx.enter_context(tc.tile_pool(name="rcp", bufs=4))
            ctp = ctx.enter_context(tc.tile_pool(name="ctp", bufs=2))
            cti = ctx.enter_context(tc.tile_pool(name="cti", bufs=4))
            outp = ctx.enter_context(tc.tile_pool(name="outp", bufs=4))
            stp = ctx.enter_context(tc.tile_pool(name="stp", bufs=2, space="PSUM"))
            avp = ctx.enter_context(tc.tile_pool(name="avp", bufs=2, space="PSUM"))
            shp = ctx.enter_context(tc.tile_pool(name="shp", bufs=2, space="PSUM"))

            # ---- input loads: critical-path first.  Attention-phase DMAs
            # go on sync; startup loads spread over scalar/sync/gpsimd.
            # kt0 on sync and wkt on scalar so the first projection's two
            # inputs stream through HWDGE back-to-back instead of serially
            kt_c = [None] * 4
            qt_c = [None] * 4
            vt_c = [None] * 4
            # boot tile: wkt_ob0 + interleaved k/q chunk-0 db slices, loaded
            # by 5 pipelined DMAs whose arrival order matches the db-
            # interleaved first projection (each DMA unblocks the next 1-2
            # matmuls, so PE starts at ~4us and never re-stalls)
            # each engine's SEQ is held through the ~625ns HWDGE acquire, so
            # one queue caps issue at ~1.3us/DMA -- slower than the boot
            # chunks' 728ns transfers.  A few early loads go on the scalar
            # (ACT) queue to double the issue rate, but only ones that clear
            # ACT.SEQ before the first exp (~10us); the rest stay on sync so
            # DMA issues never delay exp decode.
            scalar_set = set(TUNE.get("scalar_set", (1, 3, 5, 8, 10)))
            qrr = [0]

            def ldq():
                i = qrr[0]
                qrr[0] += 1
                return nc.scalar if i in scalar_set else nc.sync

            boot_sb = sing.tile([128, 4608], BF16, name="boot_sb")
            for lo, hi in ((0, 1024), (1024, 2048), (2048, 3072),
                           (3072, 4096), (4096, 4608)):
                ldq().dma_start(out=boot_sb[:, lo:hi], in_=boot[:, lo:hi])
            wkt_ob = []
            wkt_ob.append(boot_sb[:, 0:512].rearrange("p (db c) -> p db c", db=4))
            for ob in range(1, 4):
                t = sing.tile([128, 4, 128], BF16, tag=f"wktob{ob}",
                              name=f"wktob{ob}")
                wkt_ob.append(t)
            kq0 = boot_sb[:, 512:4608].rearrange("p (db two t) -> p two db t",
                                                 two=2, t=512)
            kt_c[0] = kq0[:, 0]
            qt_c[0] = kq0[:, 1]
            # all remaining loads on the single sync queue in strict
            # deadline order: SP issues every ~650ns (never holding waits)
            # and DMA_ENGINES FIFO == emission order, so nothing competes
            # with the startup-critical boot DMAs
            for ob in range(1, 4):
                ldq().dma_start(out=wkt_ob[ob], in_=wkt[ob, :, :, :])
            wvt_sb = sing.tile([128, 4, D], BF16)
            ldq().dma_start(out=wvt_sb, in_=wvt[:, :, :])

            def load_chunk(which, ch):
                t = sing.tile([128, 4, 512], BF16, tag=f"{which}tc{ch}",
                              name=f"{which}tc{ch}")
                src = kt if which == "k" else (qt if which == "q" else vt)
                ldq().dma_start(out=t, in_=src[:, :, ch * 512:ch * 512 + 512])
                (kt_c if which == "k" else (qt_c if which == "q" else vt_c))[ch] = t

            load_chunk("v", 0)
            mix_sb = sing.tile([128, max(nmix, 1), 128], BF16)
            ldq().dma_start(out=mix_sb, in_=mixmul.rearrange("m t s -> t m s"))
            load_chunk("k", 1)
            load_chunk("q", 1)
            wot_sb = sing.tile([128, 4, D], BF16)
            ldq().dma_start(out=wot_sb, in_=wot[:, :, :])
            load_chunk("v", 1)
            load_chunk("k", 3)
            load_chunk("q", 3)
            load_chunk("v", 3)
            load_chunk("k", 2)
            load_chunk("q", 2)
            load_chunk("v", 2)
            ident_sb = sing.tile([128, 128], BF16)
            ldq().dma_start(out=ident_sb, in_=ident[:, :])
            bk_sb = None
            if has_bk:
                bk_sb = sing.tile([128, 4], F32)
                nc.sync.dma_start(out=bk_sb, in_=bk32[:, :])
            bvb_sb = bob_sb = ones_k1 = None
            if has_bv or has_bo:
                ones_k1 = sing.tile([1, 128], BF16)
                nc.vector.memset(ones_k1, 1.0)
            if has_bv:
                bvb_sb = sing.tile([1, D], BF16)
                nc.sync.dma_start(out=bvb_sb, in_=bvb[:, :])
            if has_bo:
                bob_sb = sing.tile([1, D], BF16)
                nc.sync.dma_start(out=bob_sb, in_=bob[:, :])

            kp_sb = sing.tile([128, 4, N], BF16)
            qp_sb = sing.tile([128, 4, N], BF16)
            vp_sb = sing.tile([128, N // 128, 520], BF16)

            # PE p-state warmup: dummy matmuls on memset scratch from ~1.4us
            # so the clock has ramped by the time the first input DMA lands
            # (PE runs at 0.65/1.2GHz for the first 3us of a busy stretch)
            nwarm = TUNE.get("nwarm", 10)
            if nwarm:
                warm_a = sing.tile([128, 128], BF16, name="warm_a")
                warm_b = sing.tile([128, 130], BF16, name="warm_b")
                nc.vector.memset(warm_a, 0.0)
                nc.vector.memset(warm_b, 0.0)
                warm_ps = avp.tile([128, 2, 65], F32, tag="av", name="warm_ps")
                for _ in range(nwarm):
                    nc.tensor.matmul(warm_ps.rearrange("p a b -> p (a b)"),
                                     warm_a, warm_b, start=True, stop=True,
                                     skip_group_check=True)

            # ones columns of vp (denominator trick)
            for nt in range(N // 128):
                nc.vector.memset(
                    vp_sb[:, nt, :].rearrange("p (h u) -> p h u", u=65)[:, :, 64:65],
                    1.0)

            fillers = []
            staged = []    # (pop_stamp, thunk): finals wait out their
                           # transpose latency before becoming poppable
            pop_ctr = [0]
            copy_rr = [0]

            def psum_copy(dst, src):
                # PSUM->SBUF copies on DVE; ACT stays exp-only and GPSIMD
                # cannot access PSUM
                nc.vector.tensor_copy(out=dst, in_=src)

            def kq_half(ob, ch, which):
                csl = slice(ch * 512, ch * 512 + 512)
                src = kt_c[ch] if which == "k" else qt_c[ch]
                dst = kp_sb if which == "k" else qp_sb
                ps = shp.tile([128, 512], F32, tag="sh", name=f"ps{which}")
                for db in range(4):
                    nc.tensor.matmul(
                        ps, wkt_ob[ob][:, db, :],
                        src[:, db, :], start=(db == 0), stop=(db == 3))
                if has_bk:
                    if which == "k":
                        nc.scalar.add(dst[:, ob, csl], ps, bk_sb[:, ob:ob + 1])
                    else:
                        nc.vector.tensor_scalar_add(
                            dst[:, ob, csl], ps, bk_sb[:, ob:ob + 1])
                else:
                    psum_copy(dst[:, ob, csl], ps)

            def kq_group(ob, ch):
                kq_half(ob, ch, "k")
                kq_half(ob, ch, "q")

            def v_proj(nt):
                psV = shp.tile([128, 512], F32, tag="sh")
                for db in range(4):
                    nc.tensor.matmul(
                        psV, vt_c[nt // 4][:, db, (nt % 4) * 128:(nt % 4) * 128 + 128],
                        wvt_sb[:, db, :], start=(db == 0),
                        stop=(db == 3 and not has_bv))
                if has_bv:
                    nc.tensor.matmul(psV, ones_k1, bvb_sb[0:1, :],
                                     start=False, stop=True)
                dst = vp_sb[:, nt, :].rearrange("p (h u) -> p h u", u=65)[:, :, 0:64]
                src = psV.rearrange("p (h u) -> p h u", u=64)
                psum_copy(dst, src)

            def pop_filler(k=1):
                for _ in range(k):
                    pop_ctr[0] += 1
                    while staged and staged[0][0] + TUNE["stage_delay"] <= pop_ctr[0]:
                        fillers.append((("fin",), staged.pop(0)[1]))
                    if fillers:
                        fillers.pop(0)[1]()

            def need_filler(key):
                """Force-emit a specific filler now (dependency deadline)."""
                for fi, (k, thunk) in enumerate(fillers):
                    if k == key:
                        fillers.pop(fi)
                        thunk()
                        return

            ct_t_tiles = {}

            def attention_packs(b, hp, c, ex_t):
                """Thunks: scores+exp+mask, one per pack of j blocks."""
                packs = _pack_js(plan, c)
                thunks = []

                def do_pack(pack):
                    # PSUM bank rule: each matmul output must stay inside one
                    # 2KB bank.  h0 segments stack downward from col 512
                    # (bank 0), h1 segments upward from col 512 (bank 1); the
                    # exp covers the contiguous union [512-tw, 512+tw).
                    tw = sum(plan[c][j][0] for j in pack)
                    st = stp.tile([128, 1024], F32, tag="st")
                    ex = expp.tile([128, 1024], BF16, tag="ex")
                    pre = 0
                    for j in pack:
                        w, _ = plan[c][j]
                        tsl = slice(b * S + j * 128, b * S + j * 128 + 128)
                        ssl = slice(b * S + c * 512 + 512 - w,
                                    b * S + c * 512 + 512)
                        h0s = 512 - pre - w
                        h1s = 512 + pre
                        nc.tensor.matmul(st[:, h0s:h0s + w],
                                         kp_sb[0:64, hp, tsl],
                                         qp_sb[0:64, hp, ssl],
                                         start=True, stop=True,
                                         skip_group_check=True)
                        nc.tensor.matmul(st[:, h1s:h1s + w],
                                         kp_sb[64:128, hp, tsl],
                                         qp_sb[64:128, hp, ssl],
                                         start=True, stop=True,
                                         skip_group_check=True)
                        ex_t[j] = (ex, h0s, h1s, w)
                        pre += w
                    nc.scalar.activation(
                        ex[:, 512 - tw:512 + tw], st[:, 512 - tw:512 + tw],
                        mybir.ActivationFunctionType.Exp, scale=0.125)
                    # 0/1 mask multiply for mixed blocks (per head half)
                    mask_eng = (nc.gpsimd if TUNE.get("mask_on_pool")
                                else nc.vector)
                    for j in pack:
                        w, mixes = plan[c][j]
                        _, h0s, h1s, _ = ex_t[j]
                        for moff, pid in mixes:
                            for hs in (h0s, h1s):
                                sl = ex[:, hs + moff:hs + moff + 128]
                                mask_eng.tensor_mul(
                                    sl, sl, mix_sb[:, pid, :])

                for pack in packs:
                    import functools
                    thunks.append(functools.partial(do_pack, pack))
                return thunks

            def attention_avs(b, hp, c, ex_t, last_hp, tail=False):
                """Thunks: AV + normalize, one per query block i (swapped
                operands: ex stationary, vp moving)."""
                h0 = 2 * hp
                thunks = []

                def do_av(i):
                    js_i = []
                    for j, (ex, h0s, h1s, w) in ex_t.items():
                        i_start = 4 * c + 4 - w // 128
                        if i >= i_start:
                            o = (i - i_start) * 128
                            js_i.append((j, ex, (h0s + o, h1s + o)))
                    if not js_i:
                        return
                    for j, _, _ in js_i:
                        need_filler(("vp", b * NB + j))
                    av = avp.tile([128, 2, 65], F32, tag="av")
                    nmm = len(js_i) * 2
                    mi = 0
                    for j, ex, hss in js_i:
                        vrow = b * NB + j
                        for h in range(2):
                            # single accumulation group per av tile: PSUM
                            # zeroing is bank-granular (start marks the whole
                            # bank pending-zero; first write to each address
                            # assigns, later writes accumulate)
                            nc.tensor.matmul(
                                av[:, h, :],
                                ex[:, hss[h]:hss[h] + 128],
                                vp_sb[:, vrow,
                                      65 * (h0 + h):65 * (h0 + h) + 65],
                                start=(mi == 0), stop=(mi == nmm - 1),
                                skip_group_check=True)
                            mi += 1
                    # normalize: per-partition reciprocal + broadcast mul
                    key = (b, i)
                    if key not in ct_t_tiles:
                        ct_t_tiles[key] = ctp.tile([128, 512], BF16,
                                                   name=f"ctt{b}_{i}",
                                                   tag=f"ctt{b}_{i % 4}")
                    ct_t = ct_t_tiles[key]
                    rc = rcp.tile([128, 2], BF16, tag="rc")
                    with nc.allow_low_precision(reason="softmax recip bf16"):
                        nc.vector.reciprocal(out=rc, in_=av[:, :, 64])
                    dst = ct_t[:, 128 * hp:128 * hp + 128].rearrange(
                        "p (h w) -> p h w", h=2)
                    nc.vector.tensor_mul(
                        dst, av[:, :, 0:64],
                        rc[:, :, None].broadcast_to([128, 2, 64]))
                    if last_hp:
                        finish_block(b, i, ct_t, tail=tail)

                import functools
                for i in range(4 * c, 4 * c + 4):
                    thunks.append(functools.partial(do_av, i))
                return thunks

            def finish_block(b, i, ct_t, tail=False):
                """transpose ct_t -> feature-major, then queue out-proj.

                tail=True blocks (the last c-group) use PE transposes + an
                ACT copy instead of a DMA transpose: ~1us latency instead of
                ~3.3us of DMA issue+transfer+sem-prop on the critical tail.
                """
                ct_i = cti.tile([128, 4, 128], BF16, tag="cti")
                if tail:
                    stq = stp.tile([128, 1024], F32, tag="st",
                                   name=f"pst{b}_{i}")
                    psT = stq.bitcast(BF16)
                    for q4 in range(4):
                        nc.tensor.transpose(
                            psT[:, q4 * 128:q4 * 128 + 128],
                            ct_t[:, q4 * 128:q4 * 128 + 128], ident_sb)
                    # DVE, not ACT: ACT is still draining exps here (in-order
                    # engine => a copy there queues behind them), and the
                    # bf16->bf16 copy gets DVE's 2x mode
                    nc.vector.tensor_copy(
                        out=ct_i.rearrange("p a c -> p (a c)"),
                        in_=psT[:, 0:512])
                else:
                    nc.sync.dma_start_transpose(ct_i[:, :, :], ct_t[:, :])
                del ct_t_tiles[(b, i)]

                def final(b=b, i=i, ct_i=ct_i, tail=tail):
                    psO = shp.tile([128, 512], F32, tag="sh")
                    if tail and i == 3 and TUNE.get("split_mm_last"):
                        # column-split the accumulation so the first half's
                        # copy overlaps the second half's matmuls
                        for half in range(2):
                            cs = slice(half * 256, half * 256 + 256)
                            for db in range(4):
                                nc.tensor.matmul(
                                    psO[:, cs], ct_i[:, db, :],
                                    wot_sb[:, db, cs],
                                    start=(db == 0), stop=(db == 3),
                                    skip_group_check=True)
                        ot = outp.tile([128, 512], BF16)
                        ca, cb, dq = TUNE.get("last_engs", ("v", "v", "sync"))
                        for half, ce in ((0, ca), (1, cb)):
                            hs = slice(half * 256, half * 256 + 256)
                            if ce == "v":
                                nc.vector.tensor_copy(out=ot[:, hs],
                                                      in_=psO[:, hs])
                            else:
                                nc.scalar.copy(ot[:, hs], psO[:, hs])
                        row = b * S + i * 128
                        eng = dict(sync=nc.sync, scalar=nc.scalar)[dq]
                        eng.dma_start(out=out[row:row + 128, :], in_=ot)
                        return
                    for db in range(4):
                        nc.tensor.matmul(
                            psO, ct_i[:, db, :], wot_sb[:, db, :],
                            start=(db == 0), stop=(db == 3 and not has_bo))
                    if has_bo:
                        nc.tensor.matmul(psO, ones_k1, bob_sb[0:1, :],
                                         start=False, stop=True)
                    ot = outp.tile([128, 512], BF16)
                    row = b * S + i * 128
                    if tail and i == 3 and TUNE.get("split_last"):
                        # very last block: split copy+DMA into halves across
                        # DVE/ACT and sync/scalar so the end chain pipelines
                        nc.vector.tensor_copy(out=ot[:, 0:256],
                                              in_=psO[:, 0:256])
                        nc.scalar.copy(ot[:, 256:512], psO[:, 256:512])
                        nc.sync.dma_start(out=out[row:row + 128, 0:256],
                                          in_=ot[:, 0:256])
                        nc.scalar.dma_start(out=out[row:row + 128, 256:512],
                                            in_=ot[:, 256:512])
                    elif tail:
                        # alternate copy engine and DMA queue so the last
                        # finals' copies and issues don't serialize
                        par = (i + TUNE.get("tail_parity", 0)) % 2
                        if par == 0:
                            nc.scalar.copy(ot, psO)
                        else:
                            psum_copy(ot, psO)
                        eng = nc.sync if par == 0 else nc.scalar
                        eng.dma_start(out=out[row:row + 128, :], in_=ot)
                    else:
                        psum_copy(ot, psO)
                        nc.sync.dma_start(out=out[row:row + 128, :], in_=ot)

                staged.append((pop_ctr[0] if not tail else
                               pop_ctr[0] - TUNE["stage_delay"]
                               + TUNE["tail_stage_delay"], final))

            # ---- emission schedule (software-pipelined) ----
            # upfront: only what iteration 0's scores need; the rest of the
            # projections become ordered fillers consumed during attention.
            import functools
            # upfront: iteration 0 (b0, hp0, c0) needs only ob0/ch0.
            # k/q matmuls interleaved at db granularity to match the boot
            # DMA arrival order (psk/psq accumulate in separate PSUM banks)
            psk = shp.tile([128, 512], F32, tag="sh", name="psk")
            psq = shp.tile([128, 512], F32, tag="sh", name="psq")
            for db in range(4):
                nc.tensor.matmul(psk, wkt_ob[0][:, db, :], kt_c[0][:, db, :],
                                 start=(db == 0), stop=(db == 3),
                                 skip_group_check=True)
                nc.tensor.matmul(psq, wkt_ob[0][:, db, :], qt_c[0][:, db, :],
                                 start=(db == 0), stop=(db == 3),
                                 skip_group_check=True)
            if has_bk:
                nc.scalar.add(kp_sb[:, 0, 0:512], psk, bk_sb[:, 0:1])
                nc.vector.tensor_scalar_add(qp_sb[:, 0, 0:512], psq,
                                            bk_sb[:, 0:1])
            else:
                psum_copy(kp_sb[:, 0, 0:512], psk)
                psum_copy(qp_sb[:, 0, 0:512], psq)
            # deadline-ordered fillers matching the b0c0,b0c1,b1c1,b1c0 seq;
            # keys let consumers force-emit their prerequisites in time
            def FK(ob, ch):
                fillers.append((("kq", ob, ch), functools.partial(kq_group, ob, ch)))

            def FV(nt):
                fillers.append((("vp", nt), functools.partial(v_proj, nt)))

            FK(1, 0)
            FK(2, 0)
            FV(0)
            FV(1)
            FK(3, 0)
            FV(2)
            FV(3)
            FK(0, 1)
            FK(1, 1)
            FV(4)
            FV(5)
            FK(2, 1)
            FV(6)
            FV(7)
            FK(3, 1)
            for ob in range(4):
                FK(ob, 3)
                FK(ob, 2)
            for nt in range(12, 16):
                FV(nt)
            for nt in range(8, 12):
                FV(nt)

            seq = []
            border = {0: (0, 1), 1: (1, 0)}
            for b in range(BL):
                if NCH == 2 and b % 2 == 1 and TUNE.get("b1_order"):
                    # interleave the light c0 iterations among the heavy c1
                    # ones so the tail's exp backlog on ACT is smaller
                    for hp, c in TUNE["b1_order"]:
                        seq.append((b, hp, c))
                else:
                    for c in border[b % 2] if NCH == 2 else range(NCH):
                        for hp in range(4):
                            seq.append((b, hp, c))

            # iteration k's AV phase is interleaved with iteration k+2's
            # scores/exp packs (2-deep software pipeline): by the time an AV
            # runs, its exps retired during iteration k+1, so PE never waits
            # on ACT across iteration boundaries
            pend = []      # queue of AV thunk lists
            nseq = len(seq)
            for it, (b, hp, c) in enumerate(seq):
                # scores need this iteration's kq projections emitted first
                for ch in ([2 * b] if c == 0 else [2 * b, 2 * b + 1]):
                    need_filler(("kq", hp, ch))
                ex_t = {}
                packs = attention_packs(b, hp, c, ex_t)
                avs = attention_avs(b, hp, c, ex_t, last_hp=(hp == 3),
                                    tail=(it >= nseq - TUNE["tail_n"]))
                ready = pend.pop(0) if (len(pend) >= 2 or
                                        (pend and it == nseq - 1)) else []
                pops = 0
                # cap pops in the first (b0) half so fillers remain for the
                # ACT-bound b1c1 phase; first iterations also delay pops so
                # a not-yet-loaded filler can't head-of-line block PE
                cap = (TUNE["cap1"] if it < nseq * TUNE["capfrac1"] else
                       (TUNE["cap2"] if it < nseq * TUNE["capfrac2"] else 99))
                if TUNE.get("packs_first") and it >= TUNE.get(
                        "packs_first_it", 8):
                    for x in range(len(packs)):
                        packs[x]()
                        if (it >= 2 or x >= 2) and pops < cap:
                            pop_filler(1)
                            pops += 1
                    for x in range(len(ready)):
                        ready[x]()
                        if pops < cap:
                            pop_filler(1)
                            pops += 1
                else:
                    for x in range(max(len(packs), len(ready))):
                        do_pop = (it >= 2 or x >= 2) and pops < cap
                        if x < len(packs):
                            packs[x]()
                            if do_pop:
                                pop_filler(1)
                                pops += 1
                        if x < len(ready):
                            ready[x]()
                            if do_pop and pops < cap:
                                pop_filler(1)
                                pops += 1
                pend.append(avs)
            for avs in pend:
                for av in avs:
                    av()
                    pop_filler(1)
            while fillers or staged:
                pop_filler(1)

    return nc


_prog_cache = {}


def kernel(q, k, v, mask, zero_pad, Wk, bk, Wv, bv, Wo, bo):
    global LAST_SIM_NS, LAST_EXEC_NS
    q = np.asarray(q, dtype=np.float32)
    k = np.asarray(k, dtype=np.float32)
    v = np.asarray(v, dtype=np.float32)
    Wk = np.asarray(Wk, dtype=np.float32)
    Wv = np.asarray(Wv, dtype=np.float32)
    Wo = np.asarray(Wo, dtype=np.float32)
    bk = np.asarray(bk, dtype=np.float32).reshape(D)
    bv = np.asarray(bv, dtype=np.float32).reshape(D)
    bo = np.asarray(bo, dtype=np.float32).reshape(D)
    mask2d = np.asarray(mask).reshape(S, S).astype(bool)
    zp = int(np.asarray(zero_pad))

    status, patterns = _classify_mask(mask2d)
    plan, first_j = _plan_chunks(status, patterns)
    nmix = len(patterns)
    has_bk = bool(np.any(bk))
    has_bv = bool(np.any(bv))
    has_bo = bool(np.any(bo))

    sig = (tuple(tuple(r) for r in status), nmix, has_bk, has_bv, has_bo)
    if sig not in _prog_cache:
        nc_new = _build(plan, first_j, nmix, has_bk, has_bv, has_bo)
        legalize_waits(nc_new)   # hardware-only pass (sim runs pre-legalized)
        _prog_cache[sig] = nc_new
    nc = _prog_cache[sig]

    def _sbuf_layout(wt):
        # [D, X] -> [128, 4, X]: row d = a*128+p  ->  [p, a, :]
        return np.ascontiguousarray(wt.reshape(4, 128, -1).transpose(1, 0, 2))

    # wkt grouped by ob block: [4, 128, 4, 128], wkt[ob][p, db, c] =
    # Wk.T[db*128+p, ob*128+c]
    wkt = np.ascontiguousarray(
        _sbuf_layout(Wk.T.astype(BF)).reshape(128, 4, 4, 128)
        .transpose(2, 0, 1, 3))
    wvt = _sbuf_layout(Wv.T.astype(BF))
    wot = _sbuf_layout(Wo.T.astype(BF))
    bk32 = np.ascontiguousarray(bk.reshape(4, 128).T).astype(np.float32)
    bvb = bv.reshape(1, D).astype(BF)
    bob = bo.reshape(1, D).astype(BF)
    mixmul = (np.stack(patterns) if patterns
              else np.zeros((1, 128, 128), np.float32)).astype(BF)

    common = dict(wkt=wkt, wvt=wvt, wot=wot, bk32=bk32, bvb=bvb, bob=bob,
                  mixmul=mixmul, ident=np.eye(128, dtype=BF))
    wkt_ob0_flat = wkt[0].reshape(128, 512)
    in_maps = []
    for ci in range(NCORES):
        sl = slice(ci * BL, (ci + 1) * BL)
        qt_h = _sbuf_layout(q[sl].reshape(N, D).T.astype(BF))
        kt_h = _sbuf_layout(k[sl].reshape(N, D).T.astype(BF))
        # [db, {k,q}, t] interleaved to match the boot DMA pipeline
        kq0_h = np.stack([kt_h[:, :, 0:512], qt_h[:, :, 0:512]],
                         axis=2).reshape(128, 4096)
        boot_h = np.ascontiguousarray(
            np.concatenate([wkt_ob0_flat, kq0_h], axis=1))
        in_maps.append(dict(
            boot=boot_h,
            qt=qt_h,
            kt=kt_h,
            vt=_sbuf_layout(v[sl].reshape(N, D).T.astype(BF)),
            **common))

    if os.environ.get("BASS_KERNEL_SIM_TIME"):
        from concourse.timeline_sim import TimelineSim
        LAST_SIM_NS = TimelineSim(nc).simulate()

    res = run_bass_kernel_spmd(nc, in_maps, list(range(NCORES)))
    LAST_EXEC_NS = res.exec_time_ns

    outs = [np.asarray(res.results[ci]["out"], dtype=np.float32)
            .reshape(BL, S, D) for ci in range(NCORES)]
    full = np.concatenate(outs, axis=0)
    if zp:
        full[:, 0, :] = bo
    return full

